# revision 1
# baseline (speedup 1.0000x reference)
"""DMR induction routing kernel for Trainium2 (Bass/Tile), 8-core data-parallel.

Problem: nn_DMRInduction. Full inputs:
  m [128, 768], q [256, 768], W [768, 765], b [765] -> out [256, 765] fp32.

Sharding: Q=256 split 8 ways (32 queries/core); m, W, b replicated.

Per-core layouts:
  - hat_m        [I=128, C*D=765]   (I on partitions)  - hv weights / final hv rhs
  - hmT aug      [D+1=154, I] per c (D on partitions)  - num/mdv weights;
      row 153 holds -mean_c(m) so the num matmul computes the centered
      correlation numerator directly (sum_d xm*tq = sum_d m*tq - mum*colsum).
  - tq, v        [D, C*Q=160] as two tiles [128,160] + [34,160]
      (tqB row 32 carries colsum for the augmented num matmul and the
       yn2 correction; vB rows 25..33 stay zero so mdv stays uncentered).
  - routing state a, p, dsp [I=128, C*Q=160].
  - final hat_v  [Q=32, C*D=765] -> squash -> contiguous DMA out.
"""
import os
import sys

for _p in ("/opt/trn_rl_repo", "/root/.axon_site/_ro/trn_rl_repo"):
    if os.path.isdir(_p) and _p not in sys.path:
        sys.path.insert(0, _p)

import numpy as np
import concourse.bass as bass
import concourse.bacc as bacc
import concourse.mybir as mybir
import concourse.tile as tile
from concourse.bass_utils import run_bass_kernel_spmd

F32 = mybir.dt.float32
# Matmul input dtype. float32 is exact (final scale-relative err ~2e-5);
# float32r uses the fast PE path (1 cyc/row at N>=256 vs 4) and cuts the
# projection phase ~14us, at ~2.5e-4 scale-relative output error. The
# rest of the kernel is dependency-latency-bound, so the dtype only
# affects the projection matmuls. Default to exact.
DT = getattr(mybir.dt, os.environ.get("KERNEL_MM_DT", "float32"))

NCORES = 8
I = 128         # memory capsules
C = 5           # capsule classes
D = 153         # dim per capsule
CD = C * D      # 765
K = 768         # input dim
KC = K // 128   # 6 contraction chunks
QL = 32         # queries per core
CQ = C * QL     # 160
NPAD = 768      # W padded to 768 cols so fp32r matmuls stream N>=256
EPS = 1e-8
AX = mybir.AxisListType.X
MUL = mybir.AluOpType.mult
ADD = mybir.AluOpType.add
SUB = mybir.AluOpType.subtract


def build(with_bias: bool, dbg: bool = False):
    nc = bacc.Bacc("TRN2", target_bir_lowering=False, debug=False)

    mT_d = nc.dram_tensor("mT", [K, I], F32, kind="ExternalInput")
    qT_d = nc.dram_tensor("qT", [K, QL], F32, kind="ExternalInput")
    W_d = nc.dram_tensor("Wp", [K, NPAD], F32, kind="ExternalInput")
    b_d = nc.dram_tensor("b", [1, CD], F32, kind="ExternalInput")
    eye_d = nc.dram_tensor("eye", [128, 128], F32, kind="ExternalInput")
    ones_d = nc.dram_tensor("onesv", [128, 1], F32, kind="ExternalInput")
    zeros_d = nc.dram_tensor("zerosv", [128, 644], F32, kind="ExternalInput")
    onesc_d = nc.dram_tensor("onescv", [34, 1], F32, kind="ExternalInput")
    out_d = nc.dram_tensor("out", [QL, CD], F32, kind="ExternalOutput")
    dbg_d = {}
    if dbg:
        for nm, shp in [("hatm", [128, CD]), ("hatq", [QL, CD]), ("tqA0", [128, CQ]),
                        ("p1", [128, CQ]), ("a1", [128, CQ]), ("p2", [128, CQ]),
                        ("a2", [128, CQ]), ("p3", [128, CQ]), ("mTc1d", [128, C * 128]),
                        ("mTc2d", [34, C * 128]), ("tqB0", [34, CQ])]:
            dbg_d[nm] = nc.dram_tensor("dbg_" + nm, shp, F32, kind="ExternalOutput")

    with tile.TileContext(nc) as tc:
        with (
            tc.tile_pool(name="sb", bufs=1) as sb,
            tc.tile_pool(name="sb2", bufs=3) as sb2,
        ):
            # ---------------- loads ----------------
            W_sb = sb.tile([128, KC, NPAD], DT, tag="W")
            mT_sb = sb.tile([128, KC, I], DT, tag="mT")
            qT_sb = sb.tile([128, KC, QL], DT, tag="qT")
            eye = sb.tile([128, 128], F32, tag="eye")
            nc.sync.dma_start(mT_sb[:], mT_d[:].rearrange("(k p) n -> p k n", p=128).bitcast(DT))
            nc.sync.dma_start(qT_sb[:], qT_d[:].rearrange("(k p) n -> p k n", p=128).bitcast(DT))
            nc.sync.dma_start(eye[:], eye_d[:])
            Wr = W_d[:].rearrange("(k p) n -> p k n", p=128).bitcast(DT)
            for k in range(KC):
                nc.sync.dma_start(W_sb[:, k, 0:512], Wr[:, k, 0:512])
            for k in range(KC):
                nc.sync.dma_start(W_sb[:, k, 512:768], Wr[:, k, 512:768])
            if with_bias:
                b_sb = sb.tile([1, CD], F32, tag="b")
                nc.sync.dma_start(b_sb[:], b_d[:])
            ones1 = sb.tile([1, 128], F32, tag="ones1")
            nc.vector.memset(ones1[:], 1.0)
            halfv = sb.tile([1, 128], F32, tag="halfv")
            nc.vector.memset(halfv[:], 0.5)
            onesD = sb.tile([128, 1], DT, tag="onesD")
            nc.sync.dma_start(onesD[:], ones_d[:].bitcast(DT))
            epsb = sb.tile([128, 1], F32, tag="epsb")
            nc.vector.memset(epsb[:], EPS)
            onesC = sb.tile([34, 1], DT, tag="onesC")
            nc.sync.dma_start(onesC[:], onesc_d[:].bitcast(DT))

            # ---------------- projections (hat-major) ----------------
            hat_m_r = sb.tile([128, CD + 1], DT, tag="hatmr")  # col 765 zero (even-N pad)
            hat_q32 = sb.tile([QL, CD], F32, tag="hatq32")

            with tc.tile_pool(name="ps1", bufs=1, space="PSUM") as ps1, \
                 tc.tile_pool(name="pstp", bufs=4, space="PSUM") as pstp:
                psA = ps1.tile([128, 512], F32, tag="psA")
                psB = ps1.tile([128, 256], F32, tag="psB")
                for k in range(KC):
                    nc.tensor.matmul(psA[:], mT_sb[:, k, :], W_sb[:, k, 0:512],
                                     start=(k == 0), stop=(k == KC - 1 and not with_bias))
                    nc.tensor.matmul(psB[:], mT_sb[:, k, :], W_sb[:, k, 512:768],
                                     start=(k == 0), stop=(k == KC - 1 and not with_bias))
                if with_bias:
                    nc.tensor.matmul(psA[:], ones1[:], b_sb[:, 0:512], start=False, stop=True)
                    nc.tensor.matmul(psB[:, 0:253], ones1[:], b_sb[:, 512:765],
                                     start=False, stop=True)
                nc.scalar.copy(hat_m_r[:, 0:512], psA[:])
                nc.vector.tensor_copy(hat_m_r[:, 512:765], psB[:, 0:253])
                nc.sync.dma_start(hat_m_r[:, 765:766], zeros_d[0:128, 640:641].bitcast(DT))

                psC = ps1.tile([QL, 512], F32, tag="psC")
                psD = ps1.tile([QL, 256], F32, tag="psD")
                for k in range(KC):
                    nc.tensor.matmul(psC[:], qT_sb[:, k, :], W_sb[:, k, 0:512],
                                     start=(k == 0), stop=(k == KC - 1 and not with_bias))
                    nc.tensor.matmul(psD[:], qT_sb[:, k, :], W_sb[:, k, 512:768],
                                     start=(k == 0), stop=(k == KC - 1 and not with_bias))
                if with_bias:
                    onesq = sb.tile([1, QL], F32, tag="onesq")
                    nc.vector.memset(onesq[:], 1.0)
                    nc.tensor.matmul(psC[:], onesq[:], b_sb[:, 0:512],
                                     start=False, stop=True)
                    nc.tensor.matmul(psD[:, 0:253], onesq[:], b_sb[:, 512:765],
                                     start=False, stop=True)
                # NOTE: bias-for-q path writes b broadcast over q? must be b per column:
                # out[q, n] += 1*b[n] -> lhsT = onesq [1, QL], rhs = b [1, n] OK.
                nc.scalar.copy(hat_q32[:, 0:512], psC[:])
                nc.scalar.copy(hat_q32[:, 512:765], psD[:, 0:253])

                # ---------------- m stats ----------------
                # mum [128, C], xn2 [128, C], inv_xn [128, C]
                hm32 = hat_m_r[:, 0:765].bitcast(F32)
                mum = sb.tile([128, C], F32, tag="mum")
                nc.vector.tensor_reduce(mum[:], hm32.rearrange("p (c d) -> p c d", c=C),
                                        axis=AX, op=ADD)  # holds D*mean
                sqm = sb.tile([128, CD], F32, tag="sqm")
                nc.vector.tensor_tensor(sqm[:], hm32, hm32, op=MUL)
                xn2 = sb.tile([128, C], F32, tag="xn2")
                nc.vector.tensor_reduce(xn2[:], sqm[:].rearrange("p (c d) -> p c d", c=C),
                                        axis=AX, op=ADD)
                # xn2 = sum(hm^2) - D*mum^2 ; inv_xn = 1/sqrt(xn2)
                mum2 = sb.tile([128, C], F32, tag="mum2")
                nc.vector.tensor_tensor(mum2[:], mum[:], mum[:], op=MUL)
                nc.vector.tensor_scalar(mum2[:], mum2[:], 1.0 / D, None, op0=MUL)
                nc.vector.tensor_tensor(xn2[:], xn2[:], mum2[:], op=SUB)
                lxn = sb.tile([128, C], F32, tag="lxn")
                nc.scalar.activation(lxn[:], xn2[:], mybir.ActivationFunctionType.Ln)
                inv_xn = sb.tile([128, C], F32, tag="invxn")
                nc.scalar.activation(inv_xn[:], lxn[:], mybir.ActivationFunctionType.Exp, scale=-0.5)

                # rows: [C, 128] transposes of mum and inv_xn
                tpm = pstp.tile([C, 128], F32, tag="tp")
                nc.tensor.transpose(tpm[:], mum[:], eye[:])
                mumT = sb.tile([C, 128], F32, tag="mumT")
                nc.scalar.copy(mumT[:], tpm[:])
                nmumT = sb.tile([C, 128], F32, tag="nmumT")
                nc.vector.tensor_scalar(nmumT[:], mumT[:], -1.0 / D, None, op0=MUL)
                tpx = pstp.tile([C, 128], F32, tag="tp")
                nc.tensor.transpose(tpx[:], inv_xn[:], eye[:])
                invxnT = sb.tile([C, 128], F32, tag="invxnT")
                nc.scalar.copy(invxnT[:], tpx[:])
                # matmul lhsT needs base_partition 0: stage each row at partition 0
                rowsX = sb.tile([1, C, 128], F32, tag="rowsX")
                for c in range(C):
                    nc.sync.dma_start(rowsX[:, c, :], invxnT[c:c + 1, :])

                # ---------------- transposes: hmT (aug) and tq ----------------
                mTc1 = sb.tile([128, C, 128], DT, tag="mTc1")   # rows d=0..127
                mTc2 = sb.tile([34, C, 128], DT, tag="mTc2")    # rows d=128..152, row32=-mum, rest 0
                tqA = sb.tile([128, C, QL], DT, tag="tqA")
                tqB = sb.tile([34, C, QL], DT, tag="tqB")       # row32 = colsum(tq), rows 25..31,33 zero
                vA = sb.tile([128, C, QL], DT, tag="vA")
                vB = sb.tile([34, C, QL], DT, tag="vB")         # rows 25..33 stay 0
                nc.sync.dma_start(vB[:], zeros_d[0:34, 0:CQ].rearrange("p (c q) -> p c q", c=C).bitcast(DT))
                nc.sync.dma_start(tqB[:], zeros_d[0:34, 0:CQ].rearrange("p (c q) -> p c q", c=C).bitcast(DT))
                nc.sync.dma_start(mTc2[:], zeros_d[0:34, 0:640].rearrange("p (c q) -> p c q", c=C).bitcast(DT))

                for c in range(C):
                    t1 = pstp.tile([128, 128], F32, tag="tp")
                    nc.tensor.transpose(t1[:], hat_m_r[:, D * c:D * c + 128].bitcast(F32), eye[:])
                    (nc.vector.tensor_copy if c % 2 else nc.scalar.copy)(mTc1[:, c, :], t1[:])
                    t2 = pstp.tile([25, 128], F32, tag="tp")
                    nc.tensor.transpose(t2[:], hat_m_r[:, D * c + 128:D * (c + 1)].bitcast(F32), eye[:])
                    (nc.scalar.copy if c % 2 else nc.vector.tensor_copy)(mTc2[0:25, c, :], t2[:])
                    nc.sync.dma_start(mTc2[32:33, c, :], nmumT[c:c + 1, :].bitcast(DT))

                    t3 = pstp.tile([128, QL], F32, tag="tp")
                    nc.tensor.transpose(t3[:], hat_q32[:, D * c:D * c + 128], eye[0:QL, 0:QL])
                    (nc.vector.tensor_copy if c % 2 else nc.scalar.copy)(tqA[:, c, :], t3[:])
                    t4 = pstp.tile([25, QL], F32, tag="tp")
                    nc.tensor.transpose(t4[:], hat_q32[:, D * c + 128:D * (c + 1)], eye[0:QL, 0:QL])
                    (nc.scalar.copy if c % 2 else nc.vector.tensor_copy)(tqB[0:25, c, :], t4[:])

            if dbg:
                nc.sync.dma_start(dbg_d["hatm"][:], hat_m_r[:, 0:765].bitcast(F32))
                nc.sync.dma_start(dbg_d["hatq"][:], hat_q32[:])
                nc.sync.dma_start(dbg_d["tqA0"][:], tqA[:].bitcast(F32).rearrange("p c q -> p (c q)"))
                nc.sync.dma_start(dbg_d["mTc1d"][:], mTc1[:].bitcast(F32).rearrange("p c q -> p (c q)"))
                nc.sync.dma_start(dbg_d["mTc2d"][:], mTc2[:].bitcast(F32).rearrange("p c q -> p (c q)"))
            # ---------------- routing ----------------
            with tc.tile_pool(name="ps2", bufs=1, space="PSUM") as ps2:
                p_t = None     # pearson tile [128, CQ] fp32
                a_t = None     # routing logits [128, CQ] fp32

                def pearson():
                    """colsum -> row32; yn2 via weighted ones-matmul; p = tanh(num*bc)."""
                    tqA32 = tqA[:].bitcast(F32).rearrange("p c q -> p (c q)")
                    sqA = sb2.tile([128, CQ], DT, tag="sqA")
                    nc.gpsimd.tensor_tensor(sqA[:], tqA32, tqA32, op=MUL)
                    colsum = ps2.tile([1, CQ], F32, tag="colsum")
                    nc.tensor.matmul(colsum[:], onesD[:, :], tqA[:].rearrange("p c q -> p (c q)"),
                                     start=True, stop=False)
                    nc.tensor.matmul(colsum[:], onesD[0:26, :], tqB[0:26].rearrange("p c q -> p (c q)"),
                                     start=False, stop=True)
                    # colsum into tqB row 32 (augmented num matmul + yn2 correction)
                    nc.scalar.copy(tqB[32:33, :, :].rearrange("p c q -> p (c q)"), colsum[:])
                    tqB34 = tqB[0:34].bitcast(F32).rearrange("p c q -> p (c q)")
                    sqB = sb2.tile([34, CQ], DT, tag="sqB")
                    nc.gpsimd.tensor_tensor(sqB[:], tqB34, tqB34, op=MUL)
                    # yn2 = 1'sqA + onesC'sqB  (onesC row32 = -1/D weights colsum^2)
                    yn2 = ps2.tile([1, CQ], F32, tag="colsum2")
                    nc.tensor.matmul(yn2[:], onesD[:, :], sqA[:], start=True, stop=False)
                    nc.tensor.matmul(yn2[:], onesC[:, :], sqB[:], start=False, stop=True)
                    lyn = sb2.tile([1, CQ], F32, tag="lyn")
                    nc.scalar.activation(lyn[:], yn2[:], mybir.ActivationFunctionType.Ln)
                    inv_yn = sb2.tile([1, CQ], F32, tag="invyn")
                    nc.scalar.activation(inv_yn[:], lyn[:], mybir.ActivationFunctionType.Exp, scale=-0.5)

                    # num[i, (c,q)]
                    num = ps2.tile([128, C, QL], F32, tag="num")
                    for c in range(C):
                        nc.tensor.matmul(num[:, c, :], mTc1[:, c, :], tqA[:, c, :],
                                         start=True, stop=False)
                        nc.tensor.matmul(num[:, c, :], mTc2[:, c, :], tqB[:, c, :],
                                         start=False, stop=True)
                    # nx[i,(c,q)] = num * inv_xn[i,c]  (early, parallel with yn chain)
                    nx = sb2.tile([128, C, QL], F32, tag="nx")
                    for c in range(C):
                        nc.vector.tensor_scalar(nx[:, c, :], num[:, c, :], inv_xn[:, c:c + 1],
                                                None, op0=MUL)
                    iyb = ps2.tile([128, CQ], F32, tag="bcast")
                    nc.tensor.matmul(iyb[:], ones1[:], inv_yn[:], start=True, stop=True)
                    pp = sb2.tile([128, CQ], F32, tag="pp")
                    nc.vector.tensor_tensor(pp[:], nx[:].rearrange("p c q -> p (c q)"), iyb[:], op=MUL)
                    # tanh(x) = 1 - 2/(1+exp(2x))
                    e2 = sb2.tile([128, CQ], F32, tag="e2")
                    nc.scalar.activation(e2[:], pp[:], mybir.ActivationFunctionType.Exp, scale=2.0)
                    den = sb2.tile([128, CQ], F32, tag="dent")
                    nc.vector.tensor_scalar(den[:], e2[:], 1.0, None, op0=ADD)
                    rr = sb2.tile([128, CQ], F32, tag="rr")
                    nc.vector.reciprocal(rr[:], den[:])
                    p_new = sb2.tile([128, CQ], F32, tag="p")
                    nc.vector.tensor_scalar(p_new[:], rr[:], -2.0, 1.0, op0=MUL, op1=ADD)
                    return p_new

                p_t = pearson()
                if dbg:
                    nc.sync.dma_start(dbg_d["p1"][:], p_t[:])
                    nc.sync.dma_start(dbg_d["tqB0"][:], tqB[:].bitcast(F32).rearrange("p c q -> p (c q)"))

                for it in range(2):
                    dsp = sb2.tile([128, C, QL], DT, tag="dsp")
                    if it == 0:
                        # softmax(0) = 1/C exactly
                        nc.vector.tensor_scalar(dsp[:].rearrange("p c q -> p (c q)"),
                                                p_t[:], 1.0 / C, None, op0=ADD)
                    else:
                        ea = sb2.tile([128, CQ], F32, tag="ea")
                        nc.scalar.activation(ea[:], a_t[:], mybir.ActivationFunctionType.Exp)
                        asum = sb2.tile([128, QL], F32, tag="asum")
                        nc.vector.tensor_reduce(asum[:], ea[:].rearrange("p (c q) -> p q c", c=C),
                                                axis=AX, op=ADD)
                        rs = sb2.tile([128, QL], F32, tag="rs")
                        nc.vector.reciprocal(rs[:], asum[:])
                        dd = sb2.tile([128, C, QL], F32, tag="dd")
                        nc.vector.tensor_tensor(
                            dd[:], ea[:].rearrange("p (c q) -> p c q", c=C),
                            rs[:].rearrange("p (a q) -> p a q", a=1).broadcast_to((128, C, QL)),
                            op=MUL)
                        nc.vector.tensor_tensor(dsp[:].rearrange("p c q -> p (c q)"),
                                                dd[:].rearrange("p c q -> p (c q)"), p_t[:], op=ADD)

                    # hv[d, (c,q)] in two D-chunks
                    hvA = ps2.tile([128, C, QL], F32, tag="hvA")
                    hvB = ps2.tile([26, C, QL], F32, tag="hvB")
                    for c in range(C):
                        nc.tensor.matmul(hvA[:, c, :], hat_m_r[:, D * c:D * c + 128], dsp[:, c, :],
                                         start=True, stop=True)
                        nc.tensor.matmul(hvB[:, c, :], hat_m_r[:, D * c + 128:D * c + 154], dsp[:, c, :],
                                         start=True, stop=True)
                    # squash scale s[(c,q)] = n2/(1+n2)/sqrt(n2+eps)
                    # stage raw hv into the v tiles (scaled-by-s only where needed)
                    vAf = vA[:].bitcast(F32).rearrange("p c q -> p (c q)")
                    vBf = vB[0:25].bitcast(F32).rearrange("p c q -> p (c q)")
                    nc.scalar.copy(vA[:].rearrange("p c q -> p (c q)"), hvA[:].rearrange("p c q -> p (c q)"))
                    nc.vector.tensor_copy(vB[0:25].rearrange("p c q -> p (c q)"), hvB[0:25].rearrange("p c q -> p (c q)"))
                    sqhA = sb2.tile([128, CQ], DT, tag="sqhA")
                    nc.gpsimd.tensor_tensor(sqhA[:], vAf, vAf, op=MUL)
                    sqhB = sb2.tile([25, CQ], DT, tag="sqhB")
                    nc.gpsimd.tensor_tensor(sqhB[:], vBf, vBf, op=MUL)
                    n2 = ps2.tile([1, CQ], F32, tag="colsum2")  # share slot with colsum2
                    nc.tensor.matmul(n2[:], onesD[:, :], sqhA[:], start=True, stop=False)
                    nc.tensor.matmul(n2[:], onesD[0:25, :], sqhB[:], start=False, stop=True)
                    n2p1 = sb2.tile([1, CQ], F32, tag="n2p1")
                    nc.vector.tensor_scalar(n2p1[:], n2[:], 1.0, None, op0=ADD)
                    r1 = sb2.tile([1, CQ], F32, tag="r1")
                    nc.vector.reciprocal(r1[:], n2p1[:])
                    ln2 = sb2.tile([1, CQ], F32, tag="ln2")
                    nc.scalar.activation(ln2[:], n2[:], mybir.ActivationFunctionType.Ln, bias=epsb[0:1, :])
                    r2 = sb2.tile([1, CQ], F32, tag="r2")
                    nc.scalar.activation(r2[:], ln2[:], mybir.ActivationFunctionType.Exp, scale=-0.5)
                    omr = sb2.tile([1, CQ], F32, tag="omr")
                    nc.vector.tensor_scalar(omr[:], r1[:], -1.0, 1.0, op0=MUL, op1=ADD)
                    srow = sb2.tile([1, CQ], F32, tag="srow")
                    nc.vector.tensor_tensor(srow[:], omr[:], r2[:], op=MUL)
                    # broadcast s to all partitions via ones-matmul
                    sB = ps2.tile([128, CQ], F32, tag="bcast")  # share slot with iyb
                    nc.tensor.matmul(sB[:], ones1[:], srow[:], start=True, stop=True)
                    sBh = ps2.tile([128, CQ], F32, tag="num")  # 0.5*s broadcast; reuses num slot
                    nc.tensor.matmul(sBh[:], halfv[:], srow[:], start=True, stop=True)

                    # mdv[i, (c,q)]
                    mdv = ps2.tile([128, C, QL], F32, tag="mdv")
                    for c in range(C):
                        nc.tensor.matmul(mdv[:, c, :], mTc1[:, c, :], vA[:, c, :],
                                         start=True, stop=False)
                        nc.tensor.matmul(mdv[:, c, :], mTc2[:, c, :], vB[:, c, :],
                                         start=False, stop=True)
                    # a += p * s * mdv_raw   (mdv computed on raw hv; s applied here)
                    pm = sb2.tile([128, CQ], F32, tag="pm")
                    nc.vector.tensor_tensor(pm[:], mdv[:].rearrange("p c q -> p (c q)"), p_t[:], op=MUL)
                    pms = sb2.tile([128, CQ], F32, tag="pms")
                    nc.vector.tensor_tensor(pms[:], pm[:], sB[:], op=MUL)
                    if it == 0:
                        a_t = pms
                    else:
                        a_new = sb2.tile([128, CQ], F32, tag="a")
                        nc.vector.tensor_tensor(a_new[:], a_t[:], pms[:], op=ADD)
                        a_t = a_new

                    # tq = 0.5*tq (computed early) + (0.5*s)*hv_raw
                    tqhA = sb2.tile([128, CQ], F32, tag="tqhA")
                    nc.vector.tensor_scalar(tqhA[:], tqA[:].bitcast(F32).rearrange("p c q -> p (c q)"),
                                            0.5, None, op0=MUL)
                    tqhB = sb2.tile([25, CQ], F32, tag="tqhB")
                    nc.vector.tensor_scalar(tqhB[:], tqB[0:25].bitcast(F32).rearrange("p c q -> p (c q)"),
                                            0.5, None, op0=MUL)
                    svA = sb2.tile([128, CQ], F32, tag="svA")
                    nc.vector.tensor_tensor(svA[:], vAf, sBh[:], op=MUL)
                    nc.vector.tensor_tensor(tqA[:].rearrange("p c q -> p (c q)"), tqhA[:], svA[:], op=ADD)
                    svB = sb2.tile([25, CQ], F32, tag="svB")
                    nc.vector.tensor_tensor(svB[:], vBf, sBh[0:25, :], op=MUL)
                    nc.vector.tensor_tensor(tqB[0:25].rearrange("p c q -> p (c q)"), tqhB[:], svB[:], op=ADD)

                    p_t = pearson()
                    if dbg:
                        nc.sync.dma_start(dbg_d["a1" if it == 0 else "a2"][:], a_t[:])
                        nc.sync.dma_start(dbg_d["p2" if it == 0 else "p3"][:], p_t[:])

                # ---------------- final ----------------
                ea = sb2.tile([128, CQ], F32, tag="ea")
                nc.scalar.activation(ea[:], a_t[:], mybir.ActivationFunctionType.Exp)
                asum = sb2.tile([128, QL], F32, tag="asum")
                nc.vector.tensor_reduce(asum[:], ea[:].rearrange("p (c q) -> p q c", c=C),
                                        axis=AX, op=ADD)
                rs = sb2.tile([128, QL], F32, tag="rs")
                nc.vector.reciprocal(rs[:], asum[:])
                dd = sb2.tile([128, C, QL], F32, tag="dd")
                nc.vector.tensor_tensor(
                    dd[:], ea[:].rearrange("p (c q) -> p c q", c=C),
                    rs[:].rearrange("p (a q) -> p a q", a=1).broadcast_to((128, C, QL)), op=MUL)
                dspF = sb2.tile([128, C, QL], DT, tag="dsp")
                nc.vector.tensor_tensor(dspF[:].rearrange("p c q -> p (c q)"),
                                        dd[:].rearrange("p c q -> p (c q)"), p_t[:], op=ADD)

                hvF = sb.tile([QL, CD], F32, tag="hvF")
                for c in range(C):
                    fps = ps2.tile([QL, D + 1], F32, tag=("hvA" if c % 2 == 0 else "mdv"))
                    nc.tensor.matmul(fps[:], dspF[:, c, :], hat_m_r[:, D * c:D * c + 154],
                                     start=True, stop=True)
                    (nc.vector.tensor_copy if c % 2 else nc.scalar.copy)(hvF[:, D * c:D * (c + 1)], fps[:, 0:153])

                n2q = sb2.tile([QL, C], F32, tag="n2q")
                sqf = sb2.tile([QL, CD], F32, tag="sqf")
                nc.vector.tensor_tensor(sqf[:], hvF[:], hvF[:], op=MUL)
                nc.vector.tensor_reduce(n2q[:], sqf[:].rearrange("p (c d) -> p c d", c=C),
                                        axis=AX, op=ADD)
                fp1 = sb2.tile([QL, C], F32, tag="fp1")
                nc.vector.tensor_scalar(fp1[:], n2q[:], 1.0, None, op0=ADD)
                fr1 = sb2.tile([QL, C], F32, tag="fr1")
                nc.vector.reciprocal(fr1[:], fp1[:])
                fln = sb2.tile([QL, C], F32, tag="fln")
                nc.scalar.activation(fln[:], n2q[:], mybir.ActivationFunctionType.Ln, bias=epsb[0:QL, :])
                fr2 = sb2.tile([QL, C], F32, tag="fr2")
                nc.scalar.activation(fr2[:], fln[:], mybir.ActivationFunctionType.Exp, scale=-0.5)
                fs1 = sb2.tile([QL, C], F32, tag="fs1")
                nc.vector.tensor_scalar(fs1[:], fr1[:], -1.0, 1.0, op0=MUL, op1=ADD)
                fs = sb2.tile([QL, C], F32, tag="fs")
                nc.vector.tensor_tensor(fs[:], fs1[:], fr2[:], op=MUL)
                outT = sb.tile([QL, CD], F32, tag="outT")
                nc.vector.tensor_tensor(
                    outT[:].rearrange("p (c d) -> p c d", c=C),
                    hvF[:].rearrange("p (c d) -> p c d", c=C),
                    fs[:].rearrange("p (c a) -> p c a", a=1).broadcast_to((QL, C, D)), op=MUL)
                nc.sync.dma_start(out_d[:], outT[:])

    # All activations use only {Ln, Exp, Copy}, which live together in act
    # func set 6 (natural_log_exp_and_others). The default solver alternates
    # sets 0/5, inserting ~15 table reloads (~1.3us each); one load suffices.
    def _single_act_table_load():
        inst = mybir.InstLoadActFuncSet(
            name=nc.get_next_instruction_name(), ins=[], outs=[],
            act_func_set_id=6,
        )
        inst.engine = mybir.EngineType.Activation
        nc.register_instruction(inst)
        for blk in nc.main_func.blocks:
            for idx, bi in enumerate(blk.instructions):
                if isinstance(bi, mybir.InstActivation):
                    blk.instructions.insert(idx, inst)
                    return
        raise AssertionError("no activation found")

    nc.insert_act_table_loads = _single_act_table_load
    nc.compile()
    return nc


_CACHE = {}
LAST_EXEC_NS = None
LAST_RESULTS = None


def kernel(m, q, W, b):
    m = np.asarray(m, dtype=np.float32)
    q = np.asarray(q, dtype=np.float32)
    W = np.asarray(W, dtype=np.float32)
    b = np.asarray(b, dtype=np.float32)
    assert m.shape == (I, K) and q.shape == (NCORES * QL, K) and W.shape == (K, CD)

    with_bias = bool(np.any(b))
    dbg = bool(int(os.environ.get("KERNEL_DBG", "0")))
    key = ("v1", with_bias, str(DT), dbg)
    if key not in _CACHE:
        _CACHE[key] = build(with_bias, dbg)
    nc = _CACHE[key]

    Wp = np.zeros((K, NPAD), dtype=np.float32)
    Wp[:, :CD] = W
    mT = np.ascontiguousarray(m.T)
    eye = np.eye(128, dtype=np.float32)
    b2 = b.reshape(1, CD)

    onesv = np.ones((128, 1), dtype=np.float32)
    zerosv = np.zeros((128, 644), dtype=np.float32)
    onescv = np.zeros((34, 1), dtype=np.float32)
    onescv[0:25] = 1.0
    onescv[32] = -1.0 / D
    in_maps = []
    for i in range(NCORES):
        qT = np.ascontiguousarray(q[QL * i:QL * (i + 1)].T)
        in_maps.append({"mT": mT, "qT": qT, "Wp": Wp, "b": b2, "eye": eye,
                        "onesv": onesv, "zerosv": zerosv, "onescv": onescv})

    res = run_bass_kernel_spmd(nc, in_maps, list(range(NCORES)))
    global LAST_EXEC_NS, LAST_RESULTS
    LAST_EXEC_NS = res.exec_time_ns
    LAST_RESULTS = res.results
    out = np.concatenate([res.results[i]["out"] for i in range(NCORES)], axis=0)
    return out.astype(np.float32)


if __name__ == "__main__":
    rng = np.random.default_rng(0)
    m = rng.standard_normal((I, K)).astype(np.float32)
    q = rng.standard_normal((NCORES * QL, K)).astype(np.float32)
    W = (rng.standard_normal((K, CD)) * 0.02).astype(np.float32)
    b = np.zeros((CD,), dtype=np.float32)
    out = kernel(m=m, q=q, W=W, b=b)
    print("out", out.shape, out.dtype, np.abs(out).mean())



# revision 17
# speedup vs baseline: 1.4075x; 1.4075x over previous
"""DMR induction routing kernel for Trainium2 (Bass/Tile), 8-core data-parallel.

Problem: nn_DMRInduction. Full inputs:
  m [128, 768], q [256, 768], W [768, 765], b [765] -> out [256, 765] fp32.

Sharding: Q=256 split 8 ways (32 queries/core); m, W, b replicated.

v2 layout/dataflow (per core):
  - hat_m_r  [I=128, 1024] (I on partitions; cols 0..764 = m @ W, cols 765+ zero
      so the final per-class matmuls can stream N=256 on the fp32r fast path)
  - mTc1 [128, C, 128] / mTc2 [34, C, 128]: per-class transposes of hat_m
      (d on partitions). mTc2 row 32 = -mean_d(hat_m) per (c,i), computed by a
      ones-matmul over mTc + scaled copy, so the pearson numerator matmul is
      centered for free (row 32 of tqB carries colsum(tq)).
  - tq [d, (c,q)]: computed DIRECTLY transposed from qT/W with 60 small
      matmuls into PSUM (no PE eye-transposes for the q side at all).
  - routing state a, p, dsp: [I=128, C*Q=160].
  - squash/pearson scalars on [1, 160] rows; broadcasts via 1-row matmuls.
  - iteration v is consumed straight from PSUM (svA = hv_psum * 0.5s); the
    m-dot-v matmul runs on the scaled v, so a += p*(2*mdv') needs one fused op.
"""
import os
import sys

for _p in ("/opt/trn_rl_repo", "/root/.axon_site/_ro/trn_rl_repo"):
    if os.path.isdir(_p) and _p not in sys.path:
        sys.path.insert(0, _p)

import numpy as np
import concourse.bass as bass
import concourse.bacc as bacc
import concourse.mybir as mybir
import concourse.tile as tile
from concourse.bass_utils import run_bass_kernel_spmd

F32 = mybir.dt.float32
# float32r uses the fast PE path (1 cyc/row at N>=256 vs 4) at ~2.5e-4
# scale-relative output error (tolerance is 2e-2). KERNEL_MM_DT=float32
# restores exact matmuls.
DT = getattr(mybir.dt, os.environ.get("KERNEL_MM_DT", "float32r"))

NCORES = 8
I = 128         # memory capsules
C = 5           # capsule classes
D = 153         # dim per capsule
CD = C * D      # 765
K = 768         # input dim
KC = K // 128   # 6 contraction chunks
QL = 32         # queries per core
CQ = C * QL     # 160
NPAD = 768      # W padded to 768 cols so fp32r matmuls stream N>=256
HM_W = 1024     # hat_m_r padded width (final matmuls read 256-wide windows)
EPS = 1e-8
AX = mybir.AxisListType.X
MUL = mybir.AluOpType.mult
ADD = mybir.AluOpType.add
SUB = mybir.AluOpType.subtract
ACT = mybir.ActivationFunctionType


def build(with_bias: bool, dbg: bool = False):
    nc = bacc.Bacc("TRN2", target_bir_lowering=False, debug=False)

    mT_d = nc.dram_tensor("mT", [K, I], F32, kind="ExternalInput")
    qT_d = nc.dram_tensor("qT", [K, QL], F32, kind="ExternalInput")
    W_d = nc.dram_tensor("Wp", [K, NPAD], F32, kind="ExternalInput")
    eye_d = nc.dram_tensor("eye", [128, 128], F32, kind="ExternalInput")
    if with_bias:
        b_d = nc.dram_tensor("b", [1, CD], F32, kind="ExternalInput")
    out_d = nc.dram_tensor("out", [QL, CD], F32, kind="ExternalOutput")
    dbg_d = {}
    if dbg:
        for nm, shp in [("hatm", [128, CD]), ("tqA0", [128, CQ]), ("tqB0", [34, CQ]),
                        ("mTc1d", [128, C * 128]), ("mTc2d", [34, C * 128]),
                        ("invxn", [128, C]), ("p1", [128, CQ]), ("a1", [128, CQ]),
                        ("p2", [128, CQ]), ("a2", [128, CQ]), ("p3", [128, CQ]),
                        ("hvFd", [QL, CD]), ("n2qd", [QL, C])]:
            dbg_d[nm] = nc.dram_tensor("dbg_" + nm, shp, F32, kind="ExternalOutput")

    with tile.TileContext(nc) as tc:
        with (
            tc.tile_pool(name="sb", bufs=1) as sb,
            tc.tile_pool(name="sb2", bufs=3) as sb2,
        ):
            # ---------------- input DMAs (order = HWDGE serial order) -------
            mT_sb = sb.tile([128, KC, I], DT, tag="mT")
            qT_sb = sb.tile([128, KC, QL], DT, tag="qT")
            W_sb = sb.tile([128, KC, NPAD], DT, tag="W")
            eye = sb.tile([128, 128], F32, tag="eye")
            nc.sync.dma_start(mT_sb[:], mT_d[:].rearrange("(k p) n -> p k n", p=128).bitcast(DT))
            nc.sync.dma_start(qT_sb[:], qT_d[:].rearrange("(k p) n -> p k n", p=128).bitcast(DT))
            nc.sync.dma_start(eye[:], eye_d[:])
            Wr = W_d[:].rearrange("(k p) n -> p k n", p=128).bitcast(DT)
            for k in range(KC):
                nc.sync.dma_start(W_sb[:, k, :], Wr[:, k, :])
            if with_bias:
                b_sb = sb.tile([1, CD], F32, tag="b")
                nc.sync.dma_start(b_sb[:], b_d[:])

            # ---------------- constants (no DMA) ----------------------------
            # float32r tiles cannot be memset directly; memset F32 staging and
            # copy through Act/DVE (engine writes perform the f32r rounding).
            zf = sb.tile([128, 640], F32, tag="zf")
            nc.vector.memset(zf[:], 0.0)
            of = sb.tile([128, 1], F32, tag="of")
            nc.vector.memset(of[:], 1.0)
            o1f = sb.tile([1, 128], F32, tag="o1f")
            nc.vector.memset(o1f[:], 1.0)
            nhf = sb.tile([1, 128], F32, tag="nhf")
            nc.vector.memset(nhf[:], -0.5)
            epsb = sb.tile([128, 1], F32, tag="epsb")
            nc.vector.memset(epsb[:], EPS)

            ones1 = sb.tile([1, 128], DT, tag="ones1")
            nc.scalar.copy(ones1[:], o1f[:])
            nhalf1 = sb.tile([1, 128], DT, tag="nhalf1")
            nc.vector.tensor_copy(nhalf1[:], nhf[:])
            onesF = sb.tile([128, 1], DT, tag="onesF")
            nc.scalar.copy(onesF[:], of[:])
            if with_bias:
                onesq = sb.tile([1, QL], DT, tag="onesq")
                nc.vector.tensor_copy(onesq[:], o1f[:, 0:QL])

            # persistent tiles that need zero rows
            hat_m_r = sb.tile([128, HM_W], DT, tag="hatmr")
            nc.vector.tensor_copy(hat_m_r[:, CD:HM_W], zf[:, 0:HM_W - CD])
            mTc1 = sb.tile([128, C, 128], DT, tag="mTc1")
            mTc2 = sb.tile([34, C, 128], DT, tag="mTc2")
            nc.scalar.copy(mTc2[:].rearrange("p c n -> p (c n)"), zf[0:34, 0:640])
            tqA = sb.tile([128, C, QL], DT, tag="tqA")
            tqB = sb.tile([34, C, QL], DT, tag="tqB")
            nc.vector.tensor_copy(tqB[:].rearrange("p c q -> p (c q)"), zf[0:34, 0:CQ])
            svA = sb.tile([128, C, QL], DT, tag="svA")
            svB = sb.tile([34, C, QL], DT, tag="svB")
            nc.scalar.copy(svB[:].rearrange("p c q -> p (c q)"), zf[0:34, 0:CQ])

            tqAf = tqA[:].bitcast(F32).rearrange("p c q -> p (c q)")
            tqBf25 = tqB[0:25].bitcast(F32).rearrange("p c q -> p (c q)")

            # ---------------- projections ----------------------------------
            with tc.tile_pool(name="ps1", bufs=1, space="PSUM") as ps1, \
                 tc.tile_pool(name="pstp", bufs=2, space="PSUM") as pstp, \
                 tc.tile_pool(name="psmu", bufs=1, space="PSUM") as psmu:
                psA = ps1.tile([128, 512], F32, tag="psA")
                psB = ps1.tile([128, 256], F32, tag="psB")
                psQA = ps1.tile([128, C, QL], F32, tag="psQA")
                psQB = ps1.tile([34, C, QL], F32, tag="psQB")

                # hat_m (row-major): accumulate m^T W over k-chunks. psA/psB own
                # their banks, so the k-interleaved accumulation is safe.
                # hat_q (DIRECTLY transposed: out[d,(c,q)] = sum_k W[k,cD+d] q[q,k])
                # shares one bank across 5 classes; start=True clears the whole
                # bank's has_written bits, so each (c, piece) group must run its
                # start..stop consecutively. Split k in halves: each half's groups
                # run back-to-back, halves are combined through SBUF.
                def q_half(h):
                    ks = range(3 * h, 3 * h + 3)
                    add_bias = with_bias and h == 1
                    for c in range(C):
                        for j, k in enumerate(ks):
                            nc.tensor.matmul(psQA[:, c, :], W_sb[:, k, D * c:D * c + 128],
                                             qT_sb[:, k, :], start=(j == 0),
                                             stop=(j == 2 and not add_bias))
                        if add_bias:
                            nc.tensor.matmul(psQA[:, c, :], b_sb[:, D * c:D * c + 128],
                                             onesq[:], start=False, stop=True)
                        for j, k in enumerate(ks):
                            nc.tensor.matmul(psQB[0:25, c, :], W_sb[:, k, D * c + 128:D * (c + 1)],
                                             qT_sb[:, k, :], start=(j == 0),
                                             stop=(j == 2 and not add_bias))
                        if add_bias:
                            nc.tensor.matmul(psQB[0:25, c, :], b_sb[:, D * c + 128:D * (c + 1)],
                                             onesq[:], start=False, stop=True)

                for k in range(3):
                    nc.tensor.matmul(psA[:], mT_sb[:, k, :], W_sb[:, k, 0:512],
                                     start=(k == 0), stop=False)
                    nc.tensor.matmul(psB[:], mT_sb[:, k, :], W_sb[:, k, 512:768],
                                     start=(k == 0), stop=False)
                q_half(0)
                nc.vector.tensor_copy(tqA[:].rearrange("p c q -> p (c q)"),
                                      psQA[:].rearrange("p c q -> p (c q)"))
                nc.vector.tensor_copy(tqB[0:25].rearrange("p c q -> p (c q)"),
                                      psQB[0:25].rearrange("p c q -> p (c q)"))
                for k in range(3, KC):
                    last = k == KC - 1
                    nc.tensor.matmul(psA[:], mT_sb[:, k, :], W_sb[:, k, 0:512],
                                     start=False, stop=(last and not with_bias))
                    nc.tensor.matmul(psB[:], mT_sb[:, k, :], W_sb[:, k, 512:768],
                                     start=False, stop=(last and not with_bias))
                q_half(1)
                if with_bias:
                    nc.tensor.matmul(psA[:], ones1[:], b_sb[:, 0:512], start=False, stop=True)
                    nc.tensor.matmul(psB[:, 0:253], ones1[:], b_sb[:, 512:765],
                                     start=False, stop=True)
                nc.scalar.copy(hat_m_r[:, 0:512], psA[:])
                nc.vector.tensor_copy(hat_m_r[:, 512:765], psB[:, 0:253])
                nc.vector.tensor_tensor(tqA[:].rearrange("p c q -> p (c q)"), tqAf,
                                        psQA[:].rearrange("p c q -> p (c q)"), op=ADD)
                nc.vector.tensor_tensor(tqB[0:25].rearrange("p c q -> p (c q)"), tqBf25,
                                        psQB[0:25].rearrange("p c q -> p (c q)"), op=ADD)

                hm32 = hat_m_r[:, 0:765].bitcast(F32)

                # ------------- m transposes (per class) ---------------------
                for c in range(C):
                    t1 = pstp.tile([128, 128], F32, tag="tp")
                    nc.tensor.transpose(t1[:], hat_m_r[:, D * c:D * c + 128].bitcast(F32), eye[:])
                    (nc.vector.tensor_copy if c % 2 else nc.scalar.copy)(mTc1[:, c, :], t1[:])
                    t2 = pstp.tile([25, 128], F32, tag="tp")
                    nc.tensor.transpose(t2[:], hat_m_r[:, D * c + 128:D * (c + 1)].bitcast(F32),
                                        eye[:])
                    (nc.scalar.copy if c % 2 else nc.vector.tensor_copy)(mTc2[0:25, c, :], t2[:])

                # mTc2 row 32 = -mean_d(hat_m)[c,i] via ones-matmul over mTc
                # (split 512+128: PSUM banks hold 512 fp32/partition max)
                psMuA = psmu.tile([1, 512], F32, tag="muA")
                psMuB = psmu.tile([1, 128], F32, tag="muB")
                mTc1f = mTc1[:].rearrange("p c n -> p (c n)")
                mTc2f = mTc2[0:25].rearrange("p c n -> p (c n)")
                nc.tensor.matmul(psMuA[:], onesF[:], mTc1f[:, 0:512],
                                 start=True, stop=False)
                nc.tensor.matmul(psMuA[:], onesF[0:25], mTc2f[:, 0:512],
                                 start=False, stop=True)
                nc.tensor.matmul(psMuB[:], onesF[:], mTc1f[:, 512:640],
                                 start=True, stop=False)
                nc.tensor.matmul(psMuB[:], onesF[0:25], mTc2f[:, 512:640],
                                 start=False, stop=True)
                mTc2r32 = mTc2[32:33, :, :].rearrange("p c n -> p (c n)")
                nc.scalar.activation(mTc2r32[:, 0:512], psMuA[:], ACT.Copy, scale=-1.0 / D)
                nc.scalar.activation(mTc2r32[:, 512:640], psMuB[:], ACT.Copy, scale=-1.0 / D)

                # ------------- m stats: inv_xn [128, C] ---------------------
                mum = sb.tile([128, C], F32, tag="mum")
                nc.vector.tensor_reduce(mum[:], hm32.rearrange("p (c d) -> p c d", c=C),
                                        axis=AX, op=ADD)  # sum_d
                xn2r = sb.tile([128, C], F32, tag="xn2r")
                sqs = sb.tile([128, D], F32, tag="sqs")
                for c in range(C):
                    nc.vector.scalar_tensor_tensor(
                        sqs[:], hm32[:, D * c:D * (c + 1)], 1.0,
                        hm32[:, D * c:D * (c + 1)], op0=MUL, op1=MUL,
                        accum_out=xn2r[:, c:c + 1])
                mum2 = sb.tile([128, C], F32, tag="mum2")
                nc.scalar.activation(mum2[:], mum[:], ACT.Square, scale=float(1.0 / np.sqrt(D)))
                xn2 = sb.tile([128, C], F32, tag="xn2")
                nc.vector.tensor_tensor(xn2[:], xn2r[:], mum2[:], op=SUB)
                lxn = sb.tile([128, C], F32, tag="lxn")
                nc.scalar.activation(lxn[:], xn2[:], ACT.Ln)
                inv_xn = sb.tile([128, C], F32, tag="invxn")
                nc.scalar.activation(inv_xn[:], lxn[:], ACT.Exp, scale=-0.5)

            if dbg:
                nc.sync.dma_start(dbg_d["hatm"][:], hm32)
                nc.sync.dma_start(dbg_d["tqA0"][:], tqAf)
                nc.sync.dma_start(dbg_d["tqB0"][:], tqB[:].bitcast(F32).rearrange("p c q -> p (c q)"))
                nc.sync.dma_start(dbg_d["mTc1d"][:], mTc1[:].bitcast(F32).rearrange("p c n -> p (c n)"))
                nc.sync.dma_start(dbg_d["mTc2d"][:], mTc2[:].bitcast(F32).rearrange("p c n -> p (c n)"))
                nc.sync.dma_start(dbg_d["invxn"][:], inv_xn[:])

            ixb = inv_xn[:].rearrange("p (c a) -> p c a", a=1).broadcast_to((128, C, QL))

            # ---------------- routing --------------------------------------
            with tc.tile_pool(name="ps2", bufs=1, space="PSUM") as ps2:

                def pearson():
                    """p = tanh(centered-corr(mT, tq)); returns p tile [128, CQ]."""
                    # squares (row 32 of tqB excluded; centering via csq)
                    sqA = sb2.tile([128, CQ], DT, tag="sqA")
                    nc.gpsimd.tensor_tensor(sqA[:], tqAf, tqAf, op=MUL)
                    sqB = sb2.tile([25, CQ], DT, tag="sqB")
                    nc.vector.tensor_tensor(sqB[:], tqBf25, tqBf25, op=MUL)
                    # colsum(tq) -> tqB row 32 (feeds the centered num matmul)
                    colsum = ps2.tile([1, CQ], F32, tag="colsum")
                    nc.tensor.matmul(colsum[:], onesF[:],
                                     tqA[:].rearrange("p c q -> p (c q)"),
                                     start=True, stop=False)
                    nc.tensor.matmul(colsum[:], onesF[0:25],
                                     tqB[0:25].rearrange("p c q -> p (c q)"),
                                     start=False, stop=True)
                    nc.scalar.copy(tqB[32:33, :, :].rearrange("p c q -> p (c q)"), colsum[:])
                    # yn2 = sum(tq^2) - colsum^2/D
                    yn2r = ps2.tile([1, CQ], F32, tag="cs2")
                    nc.tensor.matmul(yn2r[:], onesF[:], sqA[:], start=True, stop=False)
                    nc.tensor.matmul(yn2r[:], onesF[0:25], sqB[:], start=False, stop=True)
                    # num[i,(c,q)]: per-class A/B pairs back-to-back (groups share
                    # one bank; start=True clears the bank's has_written bits)
                    num = ps2.tile([128, C, QL], F32, tag="num")
                    for c in range(C):
                        nc.tensor.matmul(num[:, c, :], mTc1[:, c, :], tqA[:, c, :],
                                         start=True, stop=False)
                        nc.tensor.matmul(num[:, c, :], mTc2[:, c, :], tqB[:, c, :],
                                         start=False, stop=True)
                    csq = sb2.tile([1, CQ], F32, tag="csq")
                    nc.scalar.activation(csq[:], colsum[:], ACT.Square, scale=float(1.0 / np.sqrt(D)))
                    yn2 = sb2.tile([1, CQ], F32, tag="yn2")
                    nc.vector.tensor_tensor(yn2[:], yn2r[:], csq[:], op=SUB)
                    lyn = sb2.tile([1, CQ], F32, tag="lyn")
                    nc.scalar.activation(lyn[:], yn2[:], ACT.Ln)
                    inv_yn = sb2.tile([1, CQ], DT, tag="invyn")
                    nc.scalar.activation(inv_yn[:], lyn[:], ACT.Exp, scale=-0.5)
                    iyb = ps2.tile([128, CQ], F32, tag="bcast")
                    nc.tensor.matmul(iyb[:], ones1[:], inv_yn[:], start=True, stop=True)
                    # p = tanh(num * inv_xn * inv_yn); tanh(x) = 1 - 2/(1+exp(2x))
                    pp1 = sb2.tile([128, C, QL], F32, tag="pp1")
                    nc.vector.tensor_tensor(pp1[:], num[:], ixb, op=MUL)
                    pp = sb2.tile([128, CQ], F32, tag="pp")
                    nc.vector.tensor_tensor(pp[:], pp1[:].rearrange("p c q -> p (c q)"),
                                            iyb[:], op=MUL)
                    e2 = sb2.tile([128, CQ], F32, tag="e2")
                    nc.scalar.activation(e2[:], pp[:], ACT.Exp, scale=2.0)
                    den = sb2.tile([128, CQ], F32, tag="den")
                    nc.vector.tensor_scalar(den[:], e2[:], 1.0, None, op0=ADD)
                    rr = sb2.tile([128, CQ], F32, tag="rr")
                    nc.vector.reciprocal(rr[:], den[:])
                    return rr

                def p_from_rr(rr):
                    p_new = sb2.tile([128, CQ], F32, tag="p")
                    nc.vector.tensor_scalar(p_new[:], rr[:], -2.0, 1.0, op0=MUL, op1=ADD)
                    return p_new

                rr = pearson()
                a_t = None
                p_t = None

                for it in range(2):
                    dsp = sb2.tile([128, C, QL], DT, tag="dsp")
                    if it == 0:
                        # softmax(0) = 1/C exactly; dsp = p + 1/C straight from rr
                        nc.vector.tensor_scalar(dsp[:].rearrange("p c q -> p (c q)"),
                                                rr[:], -2.0, 1.0 + 1.0 / C, op0=MUL, op1=ADD)
                        p_t = p_from_rr(rr)
                        if dbg:
                            nc.sync.dma_start(dbg_d["p1"][:], p_t[:])
                    else:
                        p_t = p_from_rr(rr)
                        if dbg:
                            nc.sync.dma_start(dbg_d["p2"][:], p_t[:])
                        ea = sb2.tile([128, CQ], F32, tag="ea")
                        nc.scalar.activation(ea[:], a_t[:], ACT.Exp)
                        asum = sb2.tile([128, QL], F32, tag="asum")
                        nc.vector.tensor_reduce(asum[:], ea[:].rearrange("p (c q) -> p q c", c=C),
                                                axis=AX, op=ADD)
                        rs = sb2.tile([128, QL], F32, tag="rs")
                        nc.vector.reciprocal(rs[:], asum[:])
                        dd = sb2.tile([128, C, QL], F32, tag="dd")
                        nc.vector.tensor_tensor(
                            dd[:], ea[:].rearrange("p (c q) -> p c q", c=C),
                            rs[:].rearrange("p (a q) -> p a q", a=1).broadcast_to((128, C, QL)),
                            op=MUL)
                        nc.vector.tensor_tensor(dsp[:].rearrange("p c q -> p (c q)"),
                                                dd[:].rearrange("p c q -> p (c q)"), p_t[:], op=ADD)

                    # hv[d,(c,q)] in PSUM (consumed in place; never copied to SBUF)
                    hvA = ps2.tile([128, C, QL], F32, tag="hvA")
                    hvB = ps2.tile([26, C, QL], F32, tag="hvB")
                    for c in range(C):
                        nc.tensor.matmul(hvA[:, c, :], hat_m_r[:, D * c:D * c + 128], dsp[:, c, :],
                                         start=True, stop=True)
                        nc.tensor.matmul(hvB[:, c, :], hat_m_r[:, D * c + 128:D * c + 154], dsp[:, c, :],
                                         start=True, stop=True)
                    hvAf = hvA[:].rearrange("p c q -> p (c q)")
                    hvBf25 = hvB[0:25].rearrange("p c q -> p (c q)")
                    # n2 = sum_d hv^2 (raw); squash scale applied later
                    sqhA = sb2.tile([128, CQ], DT, tag="sqhA")
                    nc.scalar.activation(sqhA[:], hvAf, ACT.Square)
                    sqhB = sb2.tile([25, CQ], DT, tag="sqhB")
                    nc.scalar.activation(sqhB[:], hvBf25, ACT.Square)
                    n2 = ps2.tile([1, CQ], F32, tag="cs2")
                    nc.tensor.matmul(n2[:], onesF[:], sqhA[:], start=True, stop=False)
                    nc.tensor.matmul(n2[:], onesF[0:25], sqhB[:], start=False, stop=True)
                    # -s = (1/(1+n2) - 1) / sqrt(n2+eps)
                    n2p1 = sb2.tile([1, CQ], F32, tag="n2p1")
                    nc.vector.tensor_scalar(n2p1[:], n2[:], 1.0, None, op0=ADD)
                    r1 = sb2.tile([1, CQ], F32, tag="r1")
                    nc.vector.reciprocal(r1[:], n2p1[:])
                    ln2 = sb2.tile([1, CQ], F32, tag="ln2")
                    nc.scalar.activation(ln2[:], n2[:], ACT.Ln, bias=epsb[0:1, :])
                    r2 = sb2.tile([1, CQ], F32, tag="r2")
                    nc.scalar.activation(r2[:], ln2[:], ACT.Exp, scale=-0.5)
                    nsrow = sb2.tile([1, CQ], DT, tag="nsrow")
                    nc.vector.scalar_tensor_tensor(nsrow[:], r1[:], 1.0, r2[:],
                                                   op0=SUB, op1=MUL)
                    # 0.5*s broadcast to all partitions: (-0.5) x (-s)
                    sBh = ps2.tile([128, CQ], F32, tag="num")
                    nc.tensor.matmul(sBh[:], nhalf1[:], nsrow[:], start=True, stop=True)
                    # stage 0.5*s in SBUF (vector ops may read only one PSUM input)
                    sBhs = sb2.tile([128, CQ], F32, tag="sBhs")
                    nc.scalar.copy(sBhs[:], sBh[:])
                    # sv = (0.5*s)*hv, straight from PSUM; tq = 0.5*tq + sv
                    nc.vector.tensor_tensor(svA[:].rearrange("p c q -> p (c q)"),
                                            hvAf, sBhs[:], op=MUL)
                    nc.vector.tensor_tensor(svB[0:25].rearrange("p c q -> p (c q)"),
                                            hvBf25, sBhs[0:25, :], op=MUL)
                    nc.vector.scalar_tensor_tensor(tqA[:].rearrange("p c q -> p (c q)"),
                                                   tqAf, 0.5,
                                                   svA[:].rearrange("p c q -> p (c q)").bitcast(F32),
                                                   op0=MUL, op1=ADD)
                    nc.vector.scalar_tensor_tensor(tqB[0:25].rearrange("p c q -> p (c q)"),
                                                   tqBf25, 0.5,
                                                   svB[0:25].rearrange("p c q -> p (c q)").bitcast(F32),
                                                   op0=MUL, op1=ADD)
                    # mdv' = mT . (0.5*s*v)  (rows 25..33 of svB stay 0 -> uncentered)
                    mdv = ps2.tile([128, C, QL], F32, tag="mdv")
                    for c in range(C):
                        nc.tensor.matmul(mdv[:, c, :], mTc1[:, c, :], svA[:, c, :],
                                         start=True, stop=False)
                        nc.tensor.matmul(mdv[:, c, :], mTc2[:, c, :], svB[:, c, :],
                                         start=False, stop=True)
                    # a += p * s * mdv_raw = p * (2*mdv')
                    if it == 0:
                        a_t = sb2.tile([128, CQ], F32, tag="a")
                        nc.vector.scalar_tensor_tensor(a_t[:], mdv[:].rearrange("p c q -> p (c q)"),
                                                       2.0, p_t[:], op0=MUL, op1=MUL)
                    else:
                        pm2 = sb2.tile([128, CQ], F32, tag="pm2")
                        nc.vector.scalar_tensor_tensor(pm2[:], mdv[:].rearrange("p c q -> p (c q)"),
                                                       2.0, p_t[:], op0=MUL, op1=MUL)
                        a_new = sb2.tile([128, CQ], F32, tag="a")
                        nc.vector.tensor_tensor(a_new[:], a_t[:], pm2[:], op=ADD)
                        a_t = a_new
                    if dbg:
                        nc.sync.dma_start(dbg_d["a1" if it == 0 else "a2"][:], a_t[:])

                    rr = pearson()

                # ---------------- final ------------------------------------
                p_t = p_from_rr(rr)
                if dbg:
                    nc.sync.dma_start(dbg_d["p3"][:], p_t[:])
                ea = sb2.tile([128, CQ], F32, tag="ea")
                nc.scalar.activation(ea[:], a_t[:], ACT.Exp)
                asum = sb2.tile([128, QL], F32, tag="asum")
                nc.vector.tensor_reduce(asum[:], ea[:].rearrange("p (c q) -> p q c", c=C),
                                        axis=AX, op=ADD)
                rs = sb2.tile([128, QL], F32, tag="rs")
                nc.vector.reciprocal(rs[:], asum[:])
                dd = sb2.tile([128, C, QL], F32, tag="dd")
                nc.vector.tensor_tensor(
                    dd[:], ea[:].rearrange("p (c q) -> p c q", c=C),
                    rs[:].rearrange("p (a q) -> p a q", a=1).broadcast_to((128, C, QL)), op=MUL)
                dspF = sb2.tile([128, C, QL], DT, tag="dsp")
                nc.vector.tensor_tensor(dspF[:].rearrange("p c q -> p (c q)"),
                                        dd[:].rearrange("p c q -> p (c q)"), p_t[:], op=ADD)

                # final hv: per-class matmul, N=256 window (cols 765+ are zero)
                hvF = sb.tile([QL, CD], F32, tag="hvF")
                n2q = sb2.tile([QL, C], F32, tag="n2q")
                sqf = sb2.tile([QL, D], F32, tag="sqf")
                for c in range(C):
                    fps = ps2.tile([QL, 256], F32, tag=("hvA" if c % 2 == 0 else "mdv"))
                    nc.tensor.matmul(fps[:], dspF[:, c, :], hat_m_r[:, D * c:D * c + 256],
                                     start=True, stop=True)
                    nc.scalar.copy(hvF[:, D * c:D * (c + 1)], fps[:, 0:D])
                    nc.vector.scalar_tensor_tensor(sqf[:], hvF[:, D * c:D * (c + 1)], 1.0,
                                                   hvF[:, D * c:D * (c + 1)],
                                                   op0=MUL, op1=MUL,
                                                   accum_out=n2q[:, c:c + 1])
                if dbg:
                    nc.sync.dma_start(dbg_d["hvFd"][:], hvF[:])
                    nc.sync.dma_start(dbg_d["n2qd"][:], n2q[:])
                # fs = squash scale [QL, C]
                fp1 = sb2.tile([QL, C], F32, tag="fp1")
                nc.vector.tensor_scalar(fp1[:], n2q[:], 1.0, None, op0=ADD)
                fr1 = sb2.tile([QL, C], F32, tag="fr1")
                nc.vector.reciprocal(fr1[:], fp1[:])
                fln = sb2.tile([QL, C], F32, tag="fln")
                nc.scalar.activation(fln[:], n2q[:], ACT.Ln, bias=epsb[0:QL, :])
                fr2 = sb2.tile([QL, C], F32, tag="fr2")
                nc.scalar.activation(fr2[:], fln[:], ACT.Exp, scale=-0.5)
                nfs = sb2.tile([QL, C], F32, tag="nfs")
                nc.vector.scalar_tensor_tensor(nfs[:], fr1[:], 1.0, fr2[:], op0=SUB, op1=MUL)
                outT = sb.tile([QL, CD], F32, tag="outT")
                for c in range(C):
                    nc.vector.tensor_scalar(outT[:, D * c:D * (c + 1)],
                                            hvF[:, D * c:D * (c + 1)],
                                            nfs[:, c:c + 1], -1.0, op0=MUL, op1=MUL)
                nc.sync.dma_start(out_d[:], outT[:])

    # All activations use only {Ln, Exp, Copy, Square}, which live together in
    # act func set 6 (natural_log_exp_and_others). The default solver alternates
    # sets, inserting table reloads (~1.3us each); one load suffices.
    def _single_act_table_load():
        inst = mybir.InstLoadActFuncSet(
            name=nc.get_next_instruction_name(), ins=[], outs=[],
            act_func_set_id=6,
        )
        inst.engine = mybir.EngineType.Activation
        nc.register_instruction(inst)
        for blk in nc.main_func.blocks:
            for idx, bi in enumerate(blk.instructions):
                if isinstance(bi, mybir.InstActivation):
                    blk.instructions.insert(idx, inst)
                    return
        raise AssertionError("no activation found")

    nc.insert_act_table_loads = _single_act_table_load
    nc.compile()
    return nc


_CACHE = {}
LAST_EXEC_NS = None
LAST_RESULTS = None


def kernel(m, q, W, b):
    m = np.asarray(m, dtype=np.float32)
    q = np.asarray(q, dtype=np.float32)
    W = np.asarray(W, dtype=np.float32)
    b = np.asarray(b, dtype=np.float32)
    assert m.shape == (I, K) and q.shape == (NCORES * QL, K) and W.shape == (K, CD)

    with_bias = bool(np.any(b))
    dbg = bool(int(os.environ.get("KERNEL_DBG", "0")))
    key = ("v2", with_bias, str(DT), dbg)
    if key not in _CACHE:
        _CACHE[key] = build(with_bias, dbg)
    nc = _CACHE[key]

    Wp = np.zeros((K, NPAD), dtype=np.float32)
    Wp[:, :CD] = W
    mT = np.ascontiguousarray(m.T)
    eye = np.eye(128, dtype=np.float32)

    in_maps = []
    for i in range(NCORES):
        qT = np.ascontiguousarray(q[QL * i:QL * (i + 1)].T)
        im = {"mT": mT, "qT": qT, "Wp": Wp, "eye": eye}
        if with_bias:
            im["b"] = b.reshape(1, CD)
        in_maps.append(im)

    res = run_bass_kernel_spmd(nc, in_maps, list(range(NCORES)))
    global LAST_EXEC_NS, LAST_RESULTS
    LAST_EXEC_NS = res.exec_time_ns
    LAST_RESULTS = res.results
    out = np.concatenate([res.results[i]["out"] for i in range(NCORES)], axis=0)
    return out.astype(np.float32)


if __name__ == "__main__":
    rng = np.random.default_rng(0)
    m = rng.standard_normal((I, K)).astype(np.float32)
    q = rng.standard_normal((NCORES * QL, K)).astype(np.float32)
    W = (rng.standard_normal((K, CD)) * 0.02).astype(np.float32)
    b = np.zeros((CD,), dtype=np.float32)
    out = kernel(m=m, q=q, W=W, b=b)
    print("out", out.shape, out.dtype, np.abs(out).mean())


# revision 23
# speedup vs baseline: 1.4099x; 1.0017x over previous
"""DMR induction routing kernel for Trainium2 (Bass/Tile), 8-core data-parallel.

Problem: nn_DMRInduction. Full inputs:
  m [128, 768], q [256, 768], W [768, 765], b [765] -> out [256, 765] fp32.

Sharding: Q=256 split 8 ways (32 queries/core); m, W, b replicated.

v2 layout/dataflow (per core):
  - hat_m_r  [I=128, 1024] (I on partitions; cols 0..764 = m @ W, cols 765+ zero
      so the final per-class matmuls can stream N=256 on the fp32r fast path)
  - mTc1 [128, C, 128] / mTc2 [34, C, 128]: per-class transposes of hat_m
      (d on partitions). mTc2 row 32 = -mean_d(hat_m) per (c,i), computed by a
      ones-matmul over mTc + scaled copy, so the pearson numerator matmul is
      centered for free (row 32 of tqB carries colsum(tq)).
  - tq [d, (c,q)]: computed DIRECTLY transposed from qT/W with 60 small
      matmuls into PSUM (no PE eye-transposes for the q side at all).
  - routing state a, p, dsp: [I=128, C*Q=160].
  - squash/pearson scalars on [1, 160] rows; broadcasts via 1-row matmuls.
  - iteration v is consumed straight from PSUM (svA = hv_psum * 0.5s); the
    m-dot-v matmul runs on the scaled v, so a += p*(2*mdv') needs one fused op.
"""
import os
import sys

for _p in ("/opt/trn_rl_repo", "/root/.axon_site/_ro/trn_rl_repo"):
    if os.path.isdir(_p) and _p not in sys.path:
        sys.path.insert(0, _p)

import numpy as np
import concourse.bass as bass
import concourse.bacc as bacc
import concourse.mybir as mybir
import concourse.tile as tile
from concourse.bass_utils import run_bass_kernel_spmd

F32 = mybir.dt.float32
# float32r uses the fast PE path (1 cyc/row at N>=256 vs 4) at ~2.5e-4
# scale-relative output error (tolerance is 2e-2). KERNEL_MM_DT=float32
# restores exact matmuls.
DT = getattr(mybir.dt, os.environ.get("KERNEL_MM_DT", "float32r"))

NCORES = 8
I = 128         # memory capsules
C = 5           # capsule classes
D = 153         # dim per capsule
CD = C * D      # 765
K = 768         # input dim
KC = K // 128   # 6 contraction chunks
QL = 32         # queries per core
CQ = C * QL     # 160
NPAD = 768      # W padded to 768 cols so fp32r matmuls stream N>=256
HM_W = 1024     # hat_m_r padded width (final matmuls read 256-wide windows)
EPS = 1e-8
AX = mybir.AxisListType.X
MUL = mybir.AluOpType.mult
ADD = mybir.AluOpType.add
SUB = mybir.AluOpType.subtract
ACT = mybir.ActivationFunctionType


def build(with_bias: bool, dbg: bool = False):
    nc = bacc.Bacc("TRN2", target_bir_lowering=False, debug=False)

    mT_d = nc.dram_tensor("mT", [K, I], F32, kind="ExternalInput")
    qT_d = nc.dram_tensor("qT", [K, QL], F32, kind="ExternalInput")
    W_d = nc.dram_tensor("Wp", [K, NPAD], F32, kind="ExternalInput")
    eye_d = nc.dram_tensor("eye", [128, 128], F32, kind="ExternalInput")
    if with_bias:
        b_d = nc.dram_tensor("b", [1, CD], F32, kind="ExternalInput")
    out_d = nc.dram_tensor("out", [QL, CD], F32, kind="ExternalOutput")
    dbg_d = {}
    if dbg:
        for nm, shp in [("hatm", [128, CD]), ("tqA0", [128, CQ]), ("tqB0", [34, CQ]),
                        ("mTc1d", [128, C * 128]), ("mTc2d", [34, C * 128]),
                        ("invxn", [128, C]), ("p1", [128, CQ]), ("a1", [128, CQ]),
                        ("p2", [128, CQ]), ("a2", [128, CQ]), ("p3", [128, CQ]),
                        ("hvFd", [QL, CD]), ("n2qd", [QL, C])]:
            dbg_d[nm] = nc.dram_tensor("dbg_" + nm, shp, F32, kind="ExternalOutput")

    with tile.TileContext(nc) as tc:
        with (
            tc.tile_pool(name="sb", bufs=1) as sb,
            tc.tile_pool(name="sb2", bufs=3) as sb2,
        ):
            # ---------------- input DMAs (order = HWDGE serial order) -------
            mT_sb = sb.tile([128, KC, I], DT, tag="mT")
            qT_sb = sb.tile([128, KC, QL], DT, tag="qT")
            W_sb = sb.tile([128, KC, NPAD], DT, tag="W")
            eye = sb.tile([128, 128], F32, tag="eye")
            nc.sync.dma_start(mT_sb[:], mT_d[:].rearrange("(k p) n -> p k n", p=128).bitcast(DT))
            nc.sync.dma_start(qT_sb[:], qT_d[:].rearrange("(k p) n -> p k n", p=128).bitcast(DT))
            Wr = W_d[:].rearrange("(k p) n -> p k n", p=128).bitcast(DT)
            for k in range(KC):
                nc.sync.dma_start(W_sb[:, k, :], Wr[:, k, :])
            nc.sync.dma_start(eye[:], eye_d[:])
            if with_bias:
                b_sb = sb.tile([1, CD], F32, tag="b")
                nc.sync.dma_start(b_sb[:], b_d[:])

            # ---------------- constants (no DMA) ----------------------------
            # float32r tiles cannot be memset directly; memset F32 staging and
            # copy through Act/DVE (engine writes perform the f32r rounding).
            zf = sb.tile([128, 640], F32, tag="zf")
            nc.vector.memset(zf[:], 0.0)
            of = sb.tile([128, 1], F32, tag="of")
            nc.vector.memset(of[:], 1.0)
            o1f = sb.tile([1, 128], F32, tag="o1f")
            nc.vector.memset(o1f[:], 1.0)
            nhf = sb.tile([1, 128], F32, tag="nhf")
            nc.vector.memset(nhf[:], -0.5)
            epsb = sb.tile([128, 1], F32, tag="epsb")
            nc.vector.memset(epsb[:], EPS)

            ones1 = sb.tile([1, 128], DT, tag="ones1")
            nc.scalar.copy(ones1[:], o1f[:])
            nhalf1 = sb.tile([1, 128], DT, tag="nhalf1")
            nc.vector.tensor_copy(nhalf1[:], nhf[:])
            onesF = sb.tile([128, 1], DT, tag="onesF")
            nc.scalar.copy(onesF[:], of[:])
            if with_bias:
                onesq = sb.tile([1, QL], DT, tag="onesq")
                nc.vector.tensor_copy(onesq[:], o1f[:, 0:QL])

            # persistent tiles that need zero rows
            hat_m_r = sb.tile([128, HM_W], DT, tag="hatmr")
            nc.vector.tensor_copy(hat_m_r[:, CD:HM_W], zf[:, 0:HM_W - CD])
            mTc1 = sb.tile([128, C, 128], DT, tag="mTc1")
            mTc2 = sb.tile([34, C, 128], DT, tag="mTc2")
            nc.scalar.copy(mTc2[:].rearrange("p c n -> p (c n)"), zf[0:34, 0:640])
            tqA = sb.tile([128, C, QL], DT, tag="tqA")
            tqB = sb.tile([34, C, QL], DT, tag="tqB")
            nc.vector.tensor_copy(tqB[:].rearrange("p c q -> p (c q)"), zf[0:34, 0:CQ])
            svA = sb.tile([128, C, QL], DT, tag="svA")
            svB = sb.tile([34, C, QL], DT, tag="svB")
            nc.scalar.copy(svB[:].rearrange("p c q -> p (c q)"), zf[0:34, 0:CQ])

            tqAf = tqA[:].bitcast(F32).rearrange("p c q -> p (c q)")
            tqBf25 = tqB[0:25].bitcast(F32).rearrange("p c q -> p (c q)")

            # ---------------- projections ----------------------------------
            with tc.tile_pool(name="ps1", bufs=1, space="PSUM") as ps1, \
                 tc.tile_pool(name="pstp", bufs=2, space="PSUM") as pstp, \
                 tc.tile_pool(name="psmu", bufs=1, space="PSUM") as psmu:
                psA = ps1.tile([128, 512], F32, tag="psA")
                psB = ps1.tile([128, 256], F32, tag="psB")
                psQA = ps1.tile([128, C, QL], F32, tag="psQA")
                psQB = ps1.tile([34, C, QL], F32, tag="psQB")

                # hat_m (row-major): accumulate m^T W over k-chunks. psA/psB own
                # their banks, so the k-interleaved accumulation is safe.
                # hat_q (DIRECTLY transposed: out[d,(c,q)] = sum_k W[k,cD+d] q[q,k])
                # shares one bank across 5 classes; start=True clears the whole
                # bank's has_written bits, so each (c, piece) group must run its
                # start..stop consecutively. Split k in halves: each half's groups
                # run back-to-back, halves are combined through SBUF.
                def q_half(h):
                    ks = range(3 * h, 3 * h + 3)
                    add_bias = with_bias and h == 1
                    for c in range(C):
                        for j, k in enumerate(ks):
                            nc.tensor.matmul(psQA[:, c, :], W_sb[:, k, D * c:D * c + 128],
                                             qT_sb[:, k, :], start=(j == 0),
                                             stop=(j == 2 and not add_bias))
                        if add_bias:
                            nc.tensor.matmul(psQA[:, c, :], b_sb[:, D * c:D * c + 128],
                                             onesq[:], start=False, stop=True)
                        for j, k in enumerate(ks):
                            nc.tensor.matmul(psQB[0:25, c, :], W_sb[:, k, D * c + 128:D * (c + 1)],
                                             qT_sb[:, k, :], start=(j == 0),
                                             stop=(j == 2 and not add_bias))
                        if add_bias:
                            nc.tensor.matmul(psQB[0:25, c, :], b_sb[:, D * c + 128:D * (c + 1)],
                                             onesq[:], start=False, stop=True)

                for k in range(3):
                    nc.tensor.matmul(psA[:], mT_sb[:, k, :], W_sb[:, k, 0:512],
                                     start=(k == 0), stop=False)
                    nc.tensor.matmul(psB[:], mT_sb[:, k, :], W_sb[:, k, 512:768],
                                     start=(k == 0), stop=False)
                q_half(0)
                nc.vector.tensor_copy(tqA[:].rearrange("p c q -> p (c q)"),
                                      psQA[:].rearrange("p c q -> p (c q)"))
                nc.vector.tensor_copy(tqB[0:25].rearrange("p c q -> p (c q)"),
                                      psQB[0:25].rearrange("p c q -> p (c q)"))
                for k in range(3, KC):
                    last = k == KC - 1
                    nc.tensor.matmul(psA[:], mT_sb[:, k, :], W_sb[:, k, 0:512],
                                     start=False, stop=(last and not with_bias))
                    nc.tensor.matmul(psB[:], mT_sb[:, k, :], W_sb[:, k, 512:768],
                                     start=False, stop=(last and not with_bias))
                q_half(1)
                if with_bias:
                    nc.tensor.matmul(psA[:], ones1[:], b_sb[:, 0:512], start=False, stop=True)
                    nc.tensor.matmul(psB[:, 0:253], ones1[:], b_sb[:, 512:765],
                                     start=False, stop=True)
                nc.scalar.copy(hat_m_r[:, 0:256], psA[:, 0:256])
                nc.scalar.copy(hat_m_r[:, 256:512], psA[:, 256:512])
                nc.vector.tensor_copy(hat_m_r[:, 512:765], psB[:, 0:253])
                nc.vector.tensor_tensor(tqA[:].rearrange("p c q -> p (c q)"), tqAf,
                                        psQA[:].rearrange("p c q -> p (c q)"), op=ADD)
                nc.vector.tensor_tensor(tqB[0:25].rearrange("p c q -> p (c q)"), tqBf25,
                                        psQB[0:25].rearrange("p c q -> p (c q)"), op=ADD)

                hm32 = hat_m_r[:, 0:765].bitcast(F32)

                # ------------- m transposes (per class) ---------------------
                for c in range(C):
                    t1 = pstp.tile([128, 128], F32, tag="tp")
                    nc.tensor.transpose(t1[:], hat_m_r[:, D * c:D * c + 128].bitcast(F32), eye[:])
                    (nc.vector.tensor_copy if c % 2 else nc.scalar.copy)(mTc1[:, c, :], t1[:])
                    t2 = pstp.tile([25, 128], F32, tag="tp")
                    nc.tensor.transpose(t2[:], hat_m_r[:, D * c + 128:D * (c + 1)].bitcast(F32),
                                        eye[:])
                    (nc.scalar.copy if c % 2 else nc.vector.tensor_copy)(mTc2[0:25, c, :], t2[:])

                # mTc2 row 32 = -mean_d(hat_m)[c,i] via ones-matmul over mTc
                # (split 512+128: PSUM banks hold 512 fp32/partition max)
                psMuA = psmu.tile([1, 512], F32, tag="muA")
                psMuB = psmu.tile([1, 128], F32, tag="muB")
                mTc1f = mTc1[:].rearrange("p c n -> p (c n)")
                mTc2f = mTc2[0:25].rearrange("p c n -> p (c n)")
                nc.tensor.matmul(psMuA[:], onesF[:], mTc1f[:, 0:512],
                                 start=True, stop=False)
                nc.tensor.matmul(psMuA[:], onesF[0:25], mTc2f[:, 0:512],
                                 start=False, stop=True)
                nc.tensor.matmul(psMuB[:], onesF[:], mTc1f[:, 512:640],
                                 start=True, stop=False)
                nc.tensor.matmul(psMuB[:], onesF[0:25], mTc2f[:, 512:640],
                                 start=False, stop=True)
                mTc2r32 = mTc2[32:33, :, :].rearrange("p c n -> p (c n)")
                nc.scalar.activation(mTc2r32[:, 0:512], psMuA[:], ACT.Copy, scale=-1.0 / D)
                nc.scalar.activation(mTc2r32[:, 512:640], psMuB[:], ACT.Copy, scale=-1.0 / D)

                # ------------- m stats part 1: sum_d hat_m^2 ----------------
                xn2r = sb.tile([128, C], F32, tag="xn2r")
                sqs = sb.tile([128, D], F32, tag="sqs")
                for c in range(C):
                    nc.vector.scalar_tensor_tensor(
                        sqs[:], hm32[:, D * c:D * (c + 1)], 1.0,
                        hm32[:, D * c:D * (c + 1)], op0=MUL, op1=MUL,
                        accum_out=xn2r[:, c:c + 1])

            if dbg:
                nc.sync.dma_start(dbg_d["hatm"][:], hm32)
                nc.sync.dma_start(dbg_d["tqA0"][:], tqAf)
                nc.sync.dma_start(dbg_d["tqB0"][:], tqB[:].bitcast(F32).rearrange("p c q -> p (c q)"))
                nc.sync.dma_start(dbg_d["mTc1d"][:], mTc1[:].bitcast(F32).rearrange("p c n -> p (c n)"))
                nc.sync.dma_start(dbg_d["mTc2d"][:], mTc2[:].bitcast(F32).rearrange("p c n -> p (c n)"))

            inv_xn = sb.tile([128, C], F32, tag="invxn")
            ixb = inv_xn[:].rearrange("p (c a) -> p c a", a=1).broadcast_to((128, C, QL))

            # ---------------- routing --------------------------------------
            with tc.tile_pool(name="ps2", bufs=1, space="PSUM") as ps2:

                tqB32f = tqB[32:33, :, :].bitcast(F32).rearrange("p c q -> p (c q)")

                def pearson(extra=None):
                    """p = tanh(centered-corr(mT, tq)); returns rr tile [128, CQ]."""
                    # squares (row 32 of tqB excluded; centering via csq)
                    sqA = sb2.tile([128, CQ], DT, tag="sqA")
                    nc.gpsimd.tensor_tensor(sqA[:], tqAf, tqAf, op=MUL)
                    sqB = sb2.tile([25, CQ], DT, tag="sqB")
                    nc.vector.tensor_tensor(sqB[:], tqBf25, tqBf25, op=MUL)
                    # colsum(tq) -> tqB row 32 (feeds the centered num matmul)
                    colsum = ps2.tile([1, CQ], F32, tag="colsum")
                    nc.tensor.matmul(colsum[:], onesF[:],
                                     tqA[:].rearrange("p c q -> p (c q)"),
                                     start=True, stop=False)
                    nc.tensor.matmul(colsum[:], onesF[0:25],
                                     tqB[0:25].rearrange("p c q -> p (c q)"),
                                     start=False, stop=True)
                    nc.scalar.copy(tqB[32:33, :, :].rearrange("p c q -> p (c q)"), colsum[:])
                    # yn2 = sum(tq^2) - colsum^2/D, centered variance of tq
                    yn2r = ps2.tile([1, CQ], F32, tag="cs2")
                    nc.tensor.matmul(yn2r[:], onesF[:], sqA[:], start=True, stop=False)
                    nc.tensor.matmul(yn2r[:], onesF[0:25], sqB[:], start=False, stop=True)
                    csqv = sb2.tile([1, CQ], F32, tag="csqv")
                    nc.vector.tensor_tensor(csqv[:], tqB32f, tqB32f, op=MUL)
                    yn2 = sb2.tile([1, CQ], F32, tag="yn2")
                    nc.vector.scalar_tensor_tensor(yn2[:], csqv[:], -1.0 / D, yn2r[:],
                                                   op0=MUL, op1=ADD)
                    lyn = sb2.tile([1, CQ], F32, tag="lyn")
                    nc.scalar.activation(lyn[:], yn2[:], ACT.Ln)
                    inv_yn = sb2.tile([1, CQ], DT, tag="invyn")
                    nc.scalar.activation(inv_yn[:], lyn[:], ACT.Exp, scale=-0.5)
                    iyb = ps2.tile([128, CQ], F32, tag="bcast")
                    nc.tensor.matmul(iyb[:], ones1[:], inv_yn[:], start=True, stop=True)
                    if extra is not None:
                        extra()  # setup-only work that must precede the num matmuls
                    # num[i,(c,q)]: per-class A/B pairs back-to-back (groups share
                    # one bank; start=True clears the bank's has_written bits)
                    num = ps2.tile([128, C, QL], F32, tag="num")
                    for c in range(C):
                        nc.tensor.matmul(num[:, c, :], mTc1[:, c, :], tqA[:, c, :],
                                         start=True, stop=False)
                        nc.tensor.matmul(num[:, c, :], mTc2[:, c, :], tqB[:, c, :],
                                         start=False, stop=True)
                    # p = tanh(num * inv_xn * inv_yn); tanh(x) = 1 - 2/(1+exp(2x))
                    pp1 = sb2.tile([128, C, QL], F32, tag="pp1")
                    nc.vector.tensor_tensor(pp1[:], num[:], ixb, op=MUL)
                    pp = sb2.tile([128, CQ], F32, tag="pp")
                    nc.vector.tensor_tensor(pp[:], pp1[:].rearrange("p c q -> p (c q)"),
                                            iyb[:], op=MUL)
                    e2 = sb2.tile([128, CQ], F32, tag="e2")
                    nc.scalar.activation(e2[:], pp[:], ACT.Exp, scale=2.0)
                    den = sb2.tile([128, CQ], F32, tag="den")
                    nc.vector.tensor_scalar(den[:], e2[:], 1.0, None, op0=ADD)
                    rr = sb2.tile([128, CQ], F32, tag="rr")
                    nc.vector.reciprocal(rr[:], den[:])
                    return rr

                def stats_tail():
                    # m stats part 2: -mean[i,c] from mTc2 row 32 via 5 tiny
                    # transposes; then inv_xn = 1/sqrt(sum hm^2 - D*mean^2).
                    t_nm = ps2.tile([128, C], F32, tag="nm")
                    for c in range(C):
                        nc.tensor.transpose(t_nm[:, c:c + 1],
                                            mTc2[32:33, c, :].bitcast(F32), eye[32:33, 32:33])
                    nmean = sb.tile([128, C], F32, tag="nmean")
                    nc.scalar.copy(nmean[:], t_nm[:])
                    mum2 = sb.tile([128, C], F32, tag="mum2")
                    nc.scalar.activation(mum2[:], nmean[:], ACT.Square,
                                         scale=float(np.sqrt(D)))
                    xn2 = sb.tile([128, C], F32, tag="xn2")
                    nc.vector.tensor_tensor(xn2[:], xn2r[:], mum2[:], op=SUB)
                    lxn = sb.tile([128, C], F32, tag="lxn")
                    nc.scalar.activation(lxn[:], xn2[:], ACT.Ln)
                    nc.scalar.activation(inv_xn[:], lxn[:], ACT.Exp, scale=-0.5)

                def p_from_rr(rr):
                    p_new = sb2.tile([128, CQ], F32, tag="p")
                    nc.vector.tensor_scalar(p_new[:], rr[:], -2.0, 1.0, op0=MUL, op1=ADD)
                    return p_new

                rr = pearson(extra=stats_tail)
                a_t = None
                p_t = None

                for it in range(2):
                    dsp = sb2.tile([128, C, QL], DT, tag="dsp")
                    if it == 0:
                        # softmax(0) = 1/C exactly; dsp = p + 1/C straight from rr
                        nc.vector.tensor_scalar(dsp[:].rearrange("p c q -> p (c q)"),
                                                rr[:], -2.0, 1.0 + 1.0 / C, op0=MUL, op1=ADD)
                        p_t = p_from_rr(rr)
                        if dbg:
                            nc.sync.dma_start(dbg_d["p1"][:], p_t[:])
                            nc.sync.dma_start(dbg_d["invxn"][:], inv_xn[:])
                    else:
                        p_t = p_from_rr(rr)
                        if dbg:
                            nc.sync.dma_start(dbg_d["p2"][:], p_t[:])
                        ea = sb2.tile([128, CQ], F32, tag="ea")
                        nc.scalar.activation(ea[:], a_t[:], ACT.Exp)
                        asum = sb2.tile([128, QL], F32, tag="asum")
                        nc.vector.tensor_reduce(asum[:], ea[:].rearrange("p (c q) -> p q c", c=C),
                                                axis=AX, op=ADD)
                        rs = sb2.tile([128, QL], F32, tag="rs")
                        nc.vector.reciprocal(rs[:], asum[:])
                        dd = sb2.tile([128, C, QL], F32, tag="dd")
                        nc.vector.tensor_tensor(
                            dd[:], ea[:].rearrange("p (c q) -> p c q", c=C),
                            rs[:].rearrange("p (a q) -> p a q", a=1).broadcast_to((128, C, QL)),
                            op=MUL)
                        nc.vector.tensor_tensor(dsp[:].rearrange("p c q -> p (c q)"),
                                                dd[:].rearrange("p c q -> p (c q)"), p_t[:], op=ADD)

                    # hv[d,(c,q)] in PSUM (consumed in place; never copied to SBUF)
                    hvA = ps2.tile([128, C, QL], F32, tag="hvA")
                    hvB = ps2.tile([26, C, QL], F32, tag="hvB")
                    for c in range(C):
                        nc.tensor.matmul(hvA[:, c, :], hat_m_r[:, D * c:D * c + 128], dsp[:, c, :],
                                         start=True, stop=True)
                        nc.tensor.matmul(hvB[:, c, :], hat_m_r[:, D * c + 128:D * c + 154], dsp[:, c, :],
                                         start=True, stop=True)
                    hvAf = hvA[:].rearrange("p c q -> p (c q)")
                    hvBf25 = hvB[0:25].rearrange("p c q -> p (c q)")
                    # n2 = sum_d hv^2 (raw); squash scale applied later
                    sqhA = sb2.tile([128, CQ], DT, tag="sqhA")
                    nc.scalar.activation(sqhA[:], hvAf, ACT.Square)
                    sqhB = sb2.tile([25, CQ], DT, tag="sqhB")
                    nc.scalar.activation(sqhB[:], hvBf25, ACT.Square)
                    # hv staged to SBUF so sv ops can read the sBh broadcast
                    # straight from PSUM (one-PSUM-input rule); 0.5*tqB is
                    # pre-scaled on Pool while DVE runs the squash chain
                    vA = sb2.tile([128, CQ], F32, tag="vA")
                    nc.scalar.copy(vA[:], hvAf)
                    vB = sb2.tile([25, CQ], F32, tag="vB")
                    nc.vector.tensor_copy(vB[:], hvBf25)
                    tqhB = sb2.tile([25, CQ], F32, tag="tqhB")
                    nc.gpsimd.tensor_scalar(tqhB[:], tqBf25, 0.5, None, op0=MUL)
                    n2 = ps2.tile([1, CQ], F32, tag="cs2")
                    nc.tensor.matmul(n2[:], onesF[:], sqhA[:], start=True, stop=False)
                    nc.tensor.matmul(n2[:], onesF[0:25], sqhB[:], start=False, stop=True)
                    # -s = (1/(1+n2) - 1) / sqrt(n2+eps)
                    n2p1 = sb2.tile([1, CQ], F32, tag="n2p1")
                    nc.vector.tensor_scalar(n2p1[:], n2[:], 1.0, None, op0=ADD)
                    r1 = sb2.tile([1, CQ], F32, tag="r1")
                    nc.vector.reciprocal(r1[:], n2p1[:])
                    ln2 = sb2.tile([1, CQ], F32, tag="ln2")
                    nc.scalar.activation(ln2[:], n2[:], ACT.Ln, bias=epsb[0:1, :])
                    r2 = sb2.tile([1, CQ], F32, tag="r2")
                    nc.scalar.activation(r2[:], ln2[:], ACT.Exp, scale=-0.5)
                    nsrow = sb2.tile([1, CQ], DT, tag="nsrow")
                    nc.vector.scalar_tensor_tensor(nsrow[:], r1[:], 1.0, r2[:],
                                                   op0=SUB, op1=MUL)
                    # 0.5*s broadcast to all partitions: (-0.5) x (-s)
                    sBh = ps2.tile([128, CQ], F32, tag="num")
                    nc.tensor.matmul(sBh[:], nhalf1[:], nsrow[:], start=True, stop=True)
                    # sv = (0.5*s)*hv; tq = 0.5*tq + sv
                    nc.vector.tensor_tensor(svA[:].rearrange("p c q -> p (c q)"),
                                            vA[:], sBh[:], op=MUL)
                    nc.vector.tensor_tensor(svB[0:25].rearrange("p c q -> p (c q)"),
                                            vB[:], sBh[0:25, :], op=MUL)
                    nc.vector.scalar_tensor_tensor(tqA[:].rearrange("p c q -> p (c q)"),
                                                   tqAf, 0.5,
                                                   svA[:].rearrange("p c q -> p (c q)").bitcast(F32),
                                                   op0=MUL, op1=ADD)
                    nc.gpsimd.tensor_tensor(tqB[0:25].rearrange("p c q -> p (c q)"),
                                            tqhB[:],
                                            svB[0:25].rearrange("p c q -> p (c q)").bitcast(F32),
                                            op=ADD)
                    # mdv' = mT . (0.5*s*v)  (rows 25..33 of svB stay 0 -> uncentered)
                    mdv = ps2.tile([128, C, QL], F32, tag="mdv")
                    for c in range(C):
                        nc.tensor.matmul(mdv[:, c, :], mTc1[:, c, :], svA[:, c, :],
                                         start=True, stop=False)
                        nc.tensor.matmul(mdv[:, c, :], mTc2[:, c, :], svB[:, c, :],
                                         start=False, stop=True)
                    # a += p * s * mdv_raw = p * (2*mdv')
                    if it == 0:
                        a_t = sb2.tile([128, CQ], F32, tag="a")
                        nc.vector.scalar_tensor_tensor(a_t[:], mdv[:].rearrange("p c q -> p (c q)"),
                                                       2.0, p_t[:], op0=MUL, op1=MUL)
                    else:
                        pm2 = sb2.tile([128, CQ], F32, tag="pm2")
                        nc.vector.scalar_tensor_tensor(pm2[:], mdv[:].rearrange("p c q -> p (c q)"),
                                                       2.0, p_t[:], op0=MUL, op1=MUL)
                        a_new = sb2.tile([128, CQ], F32, tag="a")
                        nc.vector.tensor_tensor(a_new[:], a_t[:], pm2[:], op=ADD)
                        a_t = a_new
                    if dbg:
                        nc.sync.dma_start(dbg_d["a1" if it == 0 else "a2"][:], a_t[:])

                    rr = pearson()

                # ---------------- final ------------------------------------
                p_t = p_from_rr(rr)
                if dbg:
                    nc.sync.dma_start(dbg_d["p3"][:], p_t[:])
                ea = sb2.tile([128, CQ], F32, tag="ea")
                nc.scalar.activation(ea[:], a_t[:], ACT.Exp)
                asum = sb2.tile([128, QL], F32, tag="asum")
                nc.vector.tensor_reduce(asum[:], ea[:].rearrange("p (c q) -> p q c", c=C),
                                        axis=AX, op=ADD)
                rs = sb2.tile([128, QL], F32, tag="rs")
                nc.vector.reciprocal(rs[:], asum[:])
                dd = sb2.tile([128, C, QL], F32, tag="dd")
                nc.vector.tensor_tensor(
                    dd[:], ea[:].rearrange("p (c q) -> p c q", c=C),
                    rs[:].rearrange("p (a q) -> p a q", a=1).broadcast_to((128, C, QL)), op=MUL)
                dspF = sb2.tile([128, C, QL], DT, tag="dsp")
                nc.vector.tensor_tensor(dspF[:].rearrange("p c q -> p (c q)"),
                                        dd[:].rearrange("p c q -> p (c q)"), p_t[:], op=ADD)

                # final hv: per-class matmul, N=256 window (cols 765+ are zero).
                # n2q via Act Square+accum straight from PSUM; hvF copies on DVE.
                hvF = sb.tile([QL, CD], F32, tag="hvF")
                n2q = sb2.tile([QL, C], F32, tag="n2q")
                for c in range(C):
                    fps = ps2.tile([QL, 256], F32, tag=("hvA" if c % 2 == 0 else "mdv"))
                    nc.tensor.matmul(fps[:], dspF[:, c, :], hat_m_r[:, D * c:D * c + 256],
                                     start=True, stop=True)
                    sqf = sb2.tile([QL, D], F32, tag="sqf")
                    nc.scalar.activation(sqf[:], fps[:, 0:D], ACT.Square,
                                         accum_out=n2q[:, c:c + 1])
                    nc.vector.tensor_copy(hvF[:, D * c:D * (c + 1)], fps[:, 0:D])
                if dbg:
                    nc.sync.dma_start(dbg_d["hvFd"][:], hvF[:])
                    nc.sync.dma_start(dbg_d["n2qd"][:], n2q[:])
                # fs = squash scale [QL, C] (positive)
                fp1 = sb2.tile([QL, C], F32, tag="fp1")
                nc.vector.tensor_scalar(fp1[:], n2q[:], 1.0, None, op0=ADD)
                fr1 = sb2.tile([QL, C], F32, tag="fr1")
                nc.vector.reciprocal(fr1[:], fp1[:])
                fln = sb2.tile([QL, C], F32, tag="fln")
                nc.scalar.activation(fln[:], n2q[:], ACT.Ln, bias=epsb[0:QL, :])
                fr2 = sb2.tile([QL, C], F32, tag="fr2")
                nc.scalar.activation(fr2[:], fln[:], ACT.Exp, scale=-0.5)
                omr = sb2.tile([QL, C], F32, tag="omr")
                nc.vector.tensor_scalar(omr[:], fr1[:], -1.0, 1.0, op0=MUL, op1=ADD)
                fs = sb2.tile([QL, C], F32, tag="fs")
                nc.vector.tensor_tensor(fs[:], omr[:], fr2[:], op=MUL)
                outT = sb.tile([QL, CD], F32, tag="outT")
                for c in range(C):
                    if c % 2 == 0:
                        nc.vector.tensor_scalar(outT[:, D * c:D * (c + 1)],
                                                hvF[:, D * c:D * (c + 1)],
                                                fs[:, c:c + 1], None, op0=MUL)
                    else:
                        nc.scalar.activation(outT[:, D * c:D * (c + 1)],
                                             hvF[:, D * c:D * (c + 1)], ACT.Copy,
                                             scale=fs[:, c:c + 1])
                nc.sync.dma_start(out_d[:], outT[:])

    # All activations use only {Ln, Exp, Copy, Square}, which live together in
    # act func set 6 (natural_log_exp_and_others). The default solver alternates
    # sets, inserting table reloads (~1.3us each); one load suffices.
    def _single_act_table_load():
        inst = mybir.InstLoadActFuncSet(
            name=nc.get_next_instruction_name(), ins=[], outs=[],
            act_func_set_id=6,
        )
        inst.engine = mybir.EngineType.Activation
        nc.register_instruction(inst)
        for blk in nc.main_func.blocks:
            for idx, bi in enumerate(blk.instructions):
                if isinstance(bi, mybir.InstActivation):
                    blk.instructions.insert(idx, inst)
                    return
        raise AssertionError("no activation found")

    nc.insert_act_table_loads = _single_act_table_load
    nc.compile()
    return nc


_CACHE = {}
LAST_EXEC_NS = None
LAST_RESULTS = None


def kernel(m, q, W, b):
    m = np.asarray(m, dtype=np.float32)
    q = np.asarray(q, dtype=np.float32)
    W = np.asarray(W, dtype=np.float32)
    b = np.asarray(b, dtype=np.float32)
    assert m.shape == (I, K) and q.shape == (NCORES * QL, K) and W.shape == (K, CD)

    with_bias = bool(np.any(b))
    dbg = bool(int(os.environ.get("KERNEL_DBG", "0")))
    key = ("v2", with_bias, str(DT), dbg)
    if key not in _CACHE:
        _CACHE[key] = build(with_bias, dbg)
    nc = _CACHE[key]

    Wp = np.zeros((K, NPAD), dtype=np.float32)
    Wp[:, :CD] = W
    mT = np.ascontiguousarray(m.T)
    eye = np.eye(128, dtype=np.float32)

    in_maps = []
    for i in range(NCORES):
        qT = np.ascontiguousarray(q[QL * i:QL * (i + 1)].T)
        im = {"mT": mT, "qT": qT, "Wp": Wp, "eye": eye}
        if with_bias:
            im["b"] = b.reshape(1, CD)
        in_maps.append(im)

    res = run_bass_kernel_spmd(nc, in_maps, list(range(NCORES)))
    global LAST_EXEC_NS, LAST_RESULTS
    LAST_EXEC_NS = res.exec_time_ns
    LAST_RESULTS = res.results
    out = np.concatenate([res.results[i]["out"] for i in range(NCORES)], axis=0)
    return out.astype(np.float32)


if __name__ == "__main__":
    rng = np.random.default_rng(0)
    m = rng.standard_normal((I, K)).astype(np.float32)
    q = rng.standard_normal((NCORES * QL, K)).astype(np.float32)
    W = (rng.standard_normal((K, CD)) * 0.02).astype(np.float32)
    b = np.zeros((CD,), dtype=np.float32)
    out = kernel(m=m, q=q, W=W, b=b)
    print("out", out.shape, out.dtype, np.abs(out).mean())


# revision 24
# speedup vs baseline: 1.4441x; 1.0242x over previous
"""DMR induction routing kernel for Trainium2 (Bass/Tile), 8-core data-parallel.

Problem: nn_DMRInduction. Full inputs:
  m [128, 768], q [256, 768], W [768, 765], b [765] -> out [256, 765] fp32.

Sharding: Q=256 split 8 ways (32 queries/core); m, W, b replicated.

v2 layout/dataflow (per core):
  - hat_m_r  [I=128, 1024] (I on partitions; cols 0..764 = m @ W, cols 765+ zero
      so the final per-class matmuls can stream N=256 on the fp32r fast path)
  - mTc1 [128, C, 128] / mTc2 [34, C, 128]: per-class transposes of hat_m
      (d on partitions). mTc2 row 32 = -mean_d(hat_m) per (c,i), computed by a
      ones-matmul over mTc + scaled copy, so the pearson numerator matmul is
      centered for free (row 32 of tqB carries colsum(tq)).
  - tq [d, (c,q)]: computed DIRECTLY transposed from qT/W with 60 small
      matmuls into PSUM (no PE eye-transposes for the q side at all).
  - routing state a, p, dsp: [I=128, C*Q=160].
  - squash/pearson scalars on [1, 160] rows; broadcasts via 1-row matmuls.
  - iteration v is consumed straight from PSUM (svA = hv_psum * 0.5s); the
    m-dot-v matmul runs on the scaled v, so a += p*(2*mdv') needs one fused op.
"""
import os
import sys

for _p in ("/opt/trn_rl_repo", "/root/.axon_site/_ro/trn_rl_repo"):
    if os.path.isdir(_p) and _p not in sys.path:
        sys.path.insert(0, _p)

import numpy as np
import concourse.bass as bass
import concourse.bacc as bacc
import concourse.mybir as mybir
import concourse.tile as tile
from concourse.bass_utils import run_bass_kernel_spmd

F32 = mybir.dt.float32
# float32r uses the fast PE path (1 cyc/row at N>=256 vs 4) at ~2.5e-4
# scale-relative output error (tolerance is 2e-2). KERNEL_MM_DT=float32
# restores exact matmuls.
DT = getattr(mybir.dt, os.environ.get("KERNEL_MM_DT", "float32r"))

NCORES = 8
I = 128         # memory capsules
C = 5           # capsule classes
D = 153         # dim per capsule
CD = C * D      # 765
K = 768         # input dim
KC = K // 128   # 6 contraction chunks
QL = 32         # queries per core
CQ = C * QL     # 160
NPAD = 768      # W padded to 768 cols so fp32r matmuls stream N>=256
HM_W = 1024     # hat_m_r padded width (final matmuls read 256-wide windows)
EPS = 1e-8
AX = mybir.AxisListType.X
MUL = mybir.AluOpType.mult
ADD = mybir.AluOpType.add
SUB = mybir.AluOpType.subtract
ACT = mybir.ActivationFunctionType


def build(with_bias: bool, dbg: bool = False):
    nc = bacc.Bacc("TRN2", target_bir_lowering=False, debug=False)

    mT_d = nc.dram_tensor("mT", [K, I], F32, kind="ExternalInput")
    qT_d = nc.dram_tensor("qT", [K, QL], F32, kind="ExternalInput")
    W_d = nc.dram_tensor("Wp", [K, NPAD], F32, kind="ExternalInput")
    eye_d = nc.dram_tensor("eye", [128, 128], F32, kind="ExternalInput")
    if with_bias:
        b_d = nc.dram_tensor("b", [1, CD], F32, kind="ExternalInput")
    out_d = nc.dram_tensor("out", [QL, CD], F32, kind="ExternalOutput")
    dbg_d = {}
    if dbg:
        for nm, shp in [("hatm", [128, CD]), ("tqA0", [128, CQ]), ("tqB0", [34, CQ]),
                        ("mTc1d", [128, C * 128]), ("mTc2d", [34, C * 128]),
                        ("invxn", [128, C]), ("p1", [128, CQ]), ("a1", [128, CQ]),
                        ("p2", [128, CQ]), ("a2", [128, CQ]), ("p3", [128, CQ]),
                        ("hvFd", [QL, CD]), ("n2qd", [QL, C])]:
            dbg_d[nm] = nc.dram_tensor("dbg_" + nm, shp, F32, kind="ExternalOutput")

    with tile.TileContext(nc) as tc:
        with (
            tc.tile_pool(name="sb", bufs=1) as sb,
            tc.tile_pool(name="sb2", bufs=3) as sb2,
        ):
            # ---------------- input DMAs (order = HWDGE serial order) -------
            mT_sb = sb.tile([128, KC, I], DT, tag="mT")
            qT_sb = sb.tile([128, KC, QL], DT, tag="qT")
            W_sb = sb.tile([128, KC, NPAD], DT, tag="W")
            eye = sb.tile([128, 128], F32, tag="eye")
            nc.sync.dma_start(mT_sb[:], mT_d[:].rearrange("(k p) n -> p k n", p=128).bitcast(DT))
            nc.sync.dma_start(qT_sb[:], qT_d[:].rearrange("(k p) n -> p k n", p=128).bitcast(DT))
            Wr = W_d[:].rearrange("(k p) n -> p k n", p=128).bitcast(DT)
            for k in range(KC):
                nc.sync.dma_start(W_sb[:, k, :], Wr[:, k, :])
            nc.sync.dma_start(eye[:], eye_d[:])
            if with_bias:
                b_sb = sb.tile([1, CD], F32, tag="b")
                nc.sync.dma_start(b_sb[:], b_d[:])

            # ---------------- constants (no DMA) ----------------------------
            # float32r tiles cannot be memset directly; memset F32 staging and
            # copy through Act/DVE (engine writes perform the f32r rounding).
            zf = sb.tile([128, 640], F32, tag="zf")
            nc.vector.memset(zf[:], 0.0)
            of = sb.tile([128, 1], F32, tag="of")
            nc.vector.memset(of[:], 1.0)
            o1f = sb.tile([1, 128], F32, tag="o1f")
            nc.vector.memset(o1f[:], 1.0)
            nhf = sb.tile([1, 128], F32, tag="nhf")
            nc.vector.memset(nhf[:], -0.5)
            epsb = sb.tile([128, 1], F32, tag="epsb")
            nc.vector.memset(epsb[:], EPS)

            ones1 = sb.tile([1, 128], DT, tag="ones1")
            nc.scalar.copy(ones1[:], o1f[:])
            nhalf1 = sb.tile([1, 128], DT, tag="nhalf1")
            nc.vector.tensor_copy(nhalf1[:], nhf[:])
            onesF = sb.tile([128, 1], DT, tag="onesF")
            nc.scalar.copy(onesF[:], of[:])
            if with_bias:
                onesq = sb.tile([1, QL], DT, tag="onesq")
                nc.vector.tensor_copy(onesq[:], o1f[:, 0:QL])

            # persistent tiles that need zero rows
            hat_m_r = sb.tile([128, HM_W], DT, tag="hatmr")
            nc.vector.tensor_copy(hat_m_r[:, CD:HM_W], zf[:, 0:HM_W - CD])
            mTc1 = sb.tile([128, C, 128], DT, tag="mTc1")
            mTc2 = sb.tile([34, C, 128], DT, tag="mTc2")
            nc.scalar.copy(mTc2[:].rearrange("p c n -> p (c n)"), zf[0:34, 0:640])
            tqA = sb.tile([128, C, QL], DT, tag="tqA")
            tqB = sb.tile([34, C, QL], DT, tag="tqB")
            nc.vector.tensor_copy(tqB[:].rearrange("p c q -> p (c q)"), zf[0:34, 0:CQ])
            svA = sb.tile([128, C, QL], DT, tag="svA")
            svB = sb.tile([34, C, QL], DT, tag="svB")
            nc.scalar.copy(svB[:].rearrange("p c q -> p (c q)"), zf[0:34, 0:CQ])

            tqAf = tqA[:].bitcast(F32).rearrange("p c q -> p (c q)")
            tqBf25 = tqB[0:25].bitcast(F32).rearrange("p c q -> p (c q)")

            # ---------------- projections ----------------------------------
            with tc.tile_pool(name="ps1", bufs=1, space="PSUM") as ps1, \
                 tc.tile_pool(name="pstp", bufs=4, space="PSUM") as pstp:
                psA = ps1.tile([128, 512], F32, tag="psA")
                psB = ps1.tile([128, 256], F32, tag="psB")
                psQA = ps1.tile([128, C, QL], F32, tag="psQA")
                psQB = ps1.tile([34, C, QL], F32, tag="psQB")

                # hat_m (row-major): accumulate m^T W over k-chunks. psA/psB own
                # their banks, so the k-interleaved accumulation is safe.
                # hat_q (DIRECTLY transposed: out[d,(c,q)] = sum_k W[k,cD+d] q[q,k])
                # shares one bank across 5 classes; start=True clears the whole
                # bank's has_written bits, so each (c, piece) group must run its
                # start..stop consecutively. Split k in halves: each half's groups
                # run back-to-back, halves are combined through SBUF.
                def q_half(h):
                    ks = range(3 * h, 3 * h + 3)
                    add_bias = with_bias and h == 1
                    for c in range(C):
                        for j, k in enumerate(ks):
                            nc.tensor.matmul(psQA[:, c, :], W_sb[:, k, D * c:D * c + 128],
                                             qT_sb[:, k, :], start=(j == 0),
                                             stop=(j == 2 and not add_bias))
                        if add_bias:
                            nc.tensor.matmul(psQA[:, c, :], b_sb[:, D * c:D * c + 128],
                                             onesq[:], start=False, stop=True)
                        for j, k in enumerate(ks):
                            nc.tensor.matmul(psQB[0:25, c, :], W_sb[:, k, D * c + 128:D * (c + 1)],
                                             qT_sb[:, k, :], start=(j == 0),
                                             stop=(j == 2 and not add_bias))
                        if add_bias:
                            nc.tensor.matmul(psQB[0:25, c, :], b_sb[:, D * c + 128:D * (c + 1)],
                                             onesq[:], start=False, stop=True)

                for k in range(3):
                    nc.tensor.matmul(psA[:], mT_sb[:, k, :], W_sb[:, k, 0:512],
                                     start=(k == 0), stop=False)
                    nc.tensor.matmul(psB[:], mT_sb[:, k, :], W_sb[:, k, 512:768],
                                     start=(k == 0), stop=False)
                q_half(0)
                nc.vector.tensor_copy(tqA[:].rearrange("p c q -> p (c q)"),
                                      psQA[:].rearrange("p c q -> p (c q)"))
                nc.vector.tensor_copy(tqB[0:25].rearrange("p c q -> p (c q)"),
                                      psQB[0:25].rearrange("p c q -> p (c q)"))
                for k in range(3, KC):
                    last = k == KC - 1
                    nc.tensor.matmul(psA[:], mT_sb[:, k, :], W_sb[:, k, 0:512],
                                     start=False, stop=(last and not with_bias))
                    nc.tensor.matmul(psB[:], mT_sb[:, k, :], W_sb[:, k, 512:768],
                                     start=False, stop=(last and not with_bias))
                q_half(1)
                if with_bias:
                    nc.tensor.matmul(psA[:], ones1[:], b_sb[:, 0:512], start=False, stop=True)
                    nc.tensor.matmul(psB[:, 0:253], ones1[:], b_sb[:, 512:765],
                                     start=False, stop=True)
                nc.scalar.copy(hat_m_r[:, 0:256], psA[:, 0:256])
                nc.scalar.copy(hat_m_r[:, 256:512], psA[:, 256:512])
                nc.vector.tensor_copy(hat_m_r[:, 512:765], psB[:, 0:253])
                nc.vector.tensor_tensor(tqA[:].rearrange("p c q -> p (c q)"), tqAf,
                                        psQA[:].rearrange("p c q -> p (c q)"), op=ADD)
                nc.vector.tensor_tensor(tqB[0:25].rearrange("p c q -> p (c q)"), tqBf25,
                                        psQB[0:25].rearrange("p c q -> p (c q)"), op=ADD)

                hm32 = hat_m_r[:, 0:765].bitcast(F32)

                # ------------- m transposes (per class) ---------------------
                for c in range(C):
                    t1 = pstp.tile([128, 128], F32, tag="tp")
                    nc.tensor.transpose(t1[:], hat_m_r[:, D * c:D * c + 128].bitcast(F32), eye[:])
                    (nc.vector.tensor_copy if c % 2 else nc.scalar.copy)(mTc1[:, c, :], t1[:])
                    t2 = pstp.tile([25, 128], F32, tag="tp")
                    nc.tensor.transpose(t2[:], hat_m_r[:, D * c + 128:D * (c + 1)].bitcast(F32),
                                        eye[:])
                    (nc.scalar.copy if c % 2 else nc.vector.tensor_copy)(mTc2[0:25, c, :], t2[:])

                # mTc2 row 32 = -mean_d(hat_m)[c,i] via ones-matmul over mTc
                # (split 512+128: PSUM banks hold 512 fp32/partition max)
                psMuA = ps1.tile([1, 512], F32, tag="psA")
                psMuB = ps1.tile([1, 128], F32, tag="psB")
                mTc1f = mTc1[:].rearrange("p c n -> p (c n)")
                mTc2f = mTc2[0:25].rearrange("p c n -> p (c n)")
                nc.tensor.matmul(psMuA[:], onesF[:], mTc1f[:, 0:512],
                                 start=True, stop=False)
                nc.tensor.matmul(psMuA[:], onesF[0:25], mTc2f[:, 0:512],
                                 start=False, stop=True)
                nc.tensor.matmul(psMuB[:], onesF[:], mTc1f[:, 512:640],
                                 start=True, stop=False)
                nc.tensor.matmul(psMuB[:], onesF[0:25], mTc2f[:, 512:640],
                                 start=False, stop=True)
                mTc2r32 = mTc2[32:33, :, :].rearrange("p c n -> p (c n)")
                nc.scalar.activation(mTc2r32[:, 0:512], psMuA[:], ACT.Copy, scale=-1.0 / D)
                nc.scalar.activation(mTc2r32[:, 512:640], psMuB[:], ACT.Copy, scale=-1.0 / D)

                # ------------- m stats part 1: sum_d hat_m^2 ----------------
                xn2r = sb.tile([128, C], F32, tag="xn2r")
                sqs = sb.tile([128, D], F32, tag="sqs")
                for c in range(C):
                    nc.vector.scalar_tensor_tensor(
                        sqs[:], hm32[:, D * c:D * (c + 1)], 1.0,
                        hm32[:, D * c:D * (c + 1)], op0=MUL, op1=MUL,
                        accum_out=xn2r[:, c:c + 1])

            if dbg:
                nc.sync.dma_start(dbg_d["hatm"][:], hm32)
                nc.sync.dma_start(dbg_d["tqA0"][:], tqAf)
                nc.sync.dma_start(dbg_d["tqB0"][:], tqB[:].bitcast(F32).rearrange("p c q -> p (c q)"))
                nc.sync.dma_start(dbg_d["mTc1d"][:], mTc1[:].bitcast(F32).rearrange("p c n -> p (c n)"))
                nc.sync.dma_start(dbg_d["mTc2d"][:], mTc2[:].bitcast(F32).rearrange("p c n -> p (c n)"))

            inv_xn = sb.tile([128, C], F32, tag="invxn")
            ixb = inv_xn[:].rearrange("p (c a) -> p c a", a=1).broadcast_to((128, C, QL))

            # ---------------- routing --------------------------------------
            with tc.tile_pool(name="ps2", bufs=1, space="PSUM") as ps2:

                tqB32f = tqB[32:33, :, :].bitcast(F32).rearrange("p c q -> p (c q)")

                def pearson(extra=None):
                    """p = tanh(centered-corr(mT, tq)); returns rr tile [128, CQ]."""
                    # squares (row 32 of tqB excluded; centering via csq)
                    sqA = sb2.tile([128, CQ], DT, tag="sqA")
                    nc.gpsimd.tensor_tensor(sqA[:], tqAf, tqAf, op=MUL)
                    sqB = sb2.tile([25, CQ], DT, tag="sqB")
                    nc.vector.tensor_tensor(sqB[:], tqBf25, tqBf25, op=MUL)
                    # colsum(tq) -> tqB row 32 (feeds the centered num matmul)
                    colsum = ps2.tile([1, CQ], F32, tag="colsum")
                    nc.tensor.matmul(colsum[:], onesF[:],
                                     tqA[:].rearrange("p c q -> p (c q)"),
                                     start=True, stop=False)
                    nc.tensor.matmul(colsum[:], onesF[0:25],
                                     tqB[0:25].rearrange("p c q -> p (c q)"),
                                     start=False, stop=True)
                    nc.scalar.copy(tqB[32:33, :, :].rearrange("p c q -> p (c q)"), colsum[:])
                    # yn2 = sum(tq^2) - colsum^2/D, centered variance of tq
                    yn2r = ps2.tile([1, CQ], F32, tag="cs2")
                    nc.tensor.matmul(yn2r[:], onesF[:], sqA[:], start=True, stop=False)
                    nc.tensor.matmul(yn2r[:], onesF[0:25], sqB[:], start=False, stop=True)
                    csqv = sb2.tile([1, CQ], F32, tag="csqv")
                    nc.vector.tensor_tensor(csqv[:], tqB32f, tqB32f, op=MUL)
                    yn2 = sb2.tile([1, CQ], F32, tag="yn2")
                    nc.vector.scalar_tensor_tensor(yn2[:], csqv[:], -1.0 / D, yn2r[:],
                                                   op0=MUL, op1=ADD)
                    lyn = sb2.tile([1, CQ], F32, tag="lyn")
                    nc.scalar.activation(lyn[:], yn2[:], ACT.Ln)
                    inv_yn = sb2.tile([1, CQ], DT, tag="invyn")
                    nc.scalar.activation(inv_yn[:], lyn[:], ACT.Exp, scale=-0.5)
                    iyb = ps2.tile([128, CQ], F32, tag="bcast")
                    nc.tensor.matmul(iyb[:], ones1[:], inv_yn[:], start=True, stop=True)
                    if extra is not None:
                        extra()  # setup-only work that must precede the num matmuls
                    # num[i,(c,q)]: per-class A/B pairs back-to-back (groups share
                    # one bank; start=True clears the bank's has_written bits)
                    num = ps2.tile([128, C, QL], F32, tag="num")
                    for c in range(C):
                        nc.tensor.matmul(num[:, c, :], mTc1[:, c, :], tqA[:, c, :],
                                         start=True, stop=False)
                        nc.tensor.matmul(num[:, c, :], mTc2[:, c, :], tqB[:, c, :],
                                         start=False, stop=True)
                    # p = tanh(num * inv_xn * inv_yn); tanh(x) = 1 - 2/(1+exp(2x))
                    pp1 = sb2.tile([128, C, QL], F32, tag="pp1")
                    nc.vector.tensor_tensor(pp1[:], num[:], ixb, op=MUL)
                    pp = sb2.tile([128, CQ], F32, tag="pp")
                    nc.vector.tensor_tensor(pp[:], pp1[:].rearrange("p c q -> p (c q)"),
                                            iyb[:], op=MUL)
                    e2 = sb2.tile([128, CQ], F32, tag="e2")
                    nc.scalar.activation(e2[:], pp[:], ACT.Exp, scale=2.0)
                    den = sb2.tile([128, CQ], F32, tag="den")
                    nc.vector.tensor_scalar(den[:], e2[:], 1.0, None, op0=ADD)
                    rr = sb2.tile([128, CQ], F32, tag="rr")
                    nc.vector.reciprocal(rr[:], den[:])
                    return rr

                def stats_tail():
                    # m stats part 2: -mean[i,c] from mTc2 row 32 via 5 tiny
                    # transposes; then inv_xn = 1/sqrt(sum hm^2 - D*mean^2).
                    t_nm = ps2.tile([128, C], F32, tag="nm")
                    for c in range(C):
                        nc.tensor.transpose(t_nm[:, c:c + 1],
                                            mTc2[32:33, c, :].bitcast(F32), eye[32:33, 32:33])
                    nmean = sb.tile([128, C], F32, tag="nmean")
                    nc.vector.tensor_copy(nmean[:], t_nm[:])
                    nm2 = sb.tile([128, C], F32, tag="nm2")
                    nc.vector.tensor_tensor(nm2[:], nmean[:], nmean[:], op=MUL)
                    xn2 = sb.tile([128, C], F32, tag="xn2")
                    nc.vector.scalar_tensor_tensor(xn2[:], nm2[:], -float(D), xn2r[:],
                                                   op0=MUL, op1=ADD)
                    lxn = sb.tile([128, C], F32, tag="lxn")
                    nc.scalar.activation(lxn[:], xn2[:], ACT.Ln)
                    nc.scalar.activation(inv_xn[:], lxn[:], ACT.Exp, scale=-0.5)

                def p_from_rr(rr):
                    p_new = sb2.tile([128, CQ], F32, tag="p")
                    nc.vector.tensor_scalar(p_new[:], rr[:], -2.0, 1.0, op0=MUL, op1=ADD)
                    return p_new

                rr = pearson(extra=stats_tail)
                a_t = None
                p_t = None

                for it in range(2):
                    dsp = sb2.tile([128, C, QL], DT, tag="dsp")
                    if it == 0:
                        # softmax(0) = 1/C exactly; dsp = p + 1/C straight from rr
                        nc.vector.tensor_scalar(dsp[:].rearrange("p c q -> p (c q)"),
                                                rr[:], -2.0, 1.0 + 1.0 / C, op0=MUL, op1=ADD)
                        p_t = p_from_rr(rr)
                        if dbg:
                            nc.sync.dma_start(dbg_d["p1"][:], p_t[:])
                            nc.sync.dma_start(dbg_d["invxn"][:], inv_xn[:])
                    else:
                        p_t = p_from_rr(rr)
                        if dbg:
                            nc.sync.dma_start(dbg_d["p2"][:], p_t[:])
                        ea = sb2.tile([128, CQ], F32, tag="ea")
                        nc.scalar.activation(ea[:], a_t[:], ACT.Exp)
                        asum = sb2.tile([128, QL], F32, tag="asum")
                        nc.vector.tensor_reduce(asum[:], ea[:].rearrange("p (c q) -> p q c", c=C),
                                                axis=AX, op=ADD)
                        rs = sb2.tile([128, QL], F32, tag="rs")
                        nc.vector.reciprocal(rs[:], asum[:])
                        dd = sb2.tile([128, C, QL], F32, tag="dd")
                        nc.vector.tensor_tensor(
                            dd[:], ea[:].rearrange("p (c q) -> p c q", c=C),
                            rs[:].rearrange("p (a q) -> p a q", a=1).broadcast_to((128, C, QL)),
                            op=MUL)
                        nc.vector.tensor_tensor(dsp[:].rearrange("p c q -> p (c q)"),
                                                dd[:].rearrange("p c q -> p (c q)"), p_t[:], op=ADD)

                    # hv[d,(c,q)] in PSUM (consumed in place; never copied to SBUF)
                    hvA = ps2.tile([128, C, QL], F32, tag="hvA")
                    hvB = ps2.tile([26, C, QL], F32, tag="hvB")
                    for c in range(C):
                        nc.tensor.matmul(hvA[:, c, :], hat_m_r[:, D * c:D * c + 128], dsp[:, c, :],
                                         start=True, stop=True)
                        nc.tensor.matmul(hvB[:, c, :], hat_m_r[:, D * c + 128:D * c + 154], dsp[:, c, :],
                                         start=True, stop=True)
                    hvAf = hvA[:].rearrange("p c q -> p (c q)")
                    hvBf25 = hvB[0:25].rearrange("p c q -> p (c q)")
                    # n2 = sum_d hv^2 (raw); squash scale applied later
                    sqhA = sb2.tile([128, CQ], DT, tag="sqhA")
                    nc.scalar.activation(sqhA[:], hvAf, ACT.Square)
                    sqhB = sb2.tile([25, CQ], DT, tag="sqhB")
                    nc.scalar.activation(sqhB[:], hvBf25, ACT.Square)
                    # hv staged to SBUF so sv ops can read the sBh broadcast
                    # straight from PSUM (one-PSUM-input rule); 0.5*tqB is
                    # pre-scaled on Pool while DVE runs the squash chain
                    vA = sb2.tile([128, CQ], F32, tag="vA")
                    nc.scalar.copy(vA[:], hvAf)
                    vB = sb2.tile([25, CQ], F32, tag="vB")
                    nc.vector.tensor_copy(vB[:], hvBf25)
                    tqhB = sb2.tile([25, CQ], F32, tag="tqhB")
                    nc.gpsimd.tensor_scalar(tqhB[:], tqBf25, 0.5, None, op0=MUL)
                    n2 = ps2.tile([1, CQ], F32, tag="cs2")
                    nc.tensor.matmul(n2[:], onesF[:], sqhA[:], start=True, stop=False)
                    nc.tensor.matmul(n2[:], onesF[0:25], sqhB[:], start=False, stop=True)
                    # -s = (1/(1+n2) - 1) / sqrt(n2+eps)
                    n2p1 = sb2.tile([1, CQ], F32, tag="n2p1")
                    nc.vector.tensor_scalar(n2p1[:], n2[:], 1.0, None, op0=ADD)
                    r1 = sb2.tile([1, CQ], F32, tag="r1")
                    nc.vector.reciprocal(r1[:], n2p1[:])
                    ln2 = sb2.tile([1, CQ], F32, tag="ln2")
                    nc.scalar.activation(ln2[:], n2[:], ACT.Ln, bias=epsb[0:1, :])
                    r2 = sb2.tile([1, CQ], F32, tag="r2")
                    nc.scalar.activation(r2[:], ln2[:], ACT.Exp, scale=-0.5)
                    nsrow = sb2.tile([1, CQ], DT, tag="nsrow")
                    nc.vector.scalar_tensor_tensor(nsrow[:], r1[:], 1.0, r2[:],
                                                   op0=SUB, op1=MUL)
                    # 0.5*s broadcast to all partitions: (-0.5) x (-s)
                    sBh = ps2.tile([128, CQ], F32, tag="num")
                    nc.tensor.matmul(sBh[:], nhalf1[:], nsrow[:], start=True, stop=True)
                    # sv = (0.5*s)*hv; tq = 0.5*tq + sv
                    nc.vector.tensor_tensor(svA[:].rearrange("p c q -> p (c q)"),
                                            vA[:], sBh[:], op=MUL)
                    nc.vector.tensor_tensor(svB[0:25].rearrange("p c q -> p (c q)"),
                                            vB[:], sBh[0:25, :], op=MUL)
                    nc.vector.scalar_tensor_tensor(tqA[:].rearrange("p c q -> p (c q)"),
                                                   tqAf, 0.5,
                                                   svA[:].rearrange("p c q -> p (c q)").bitcast(F32),
                                                   op0=MUL, op1=ADD)
                    nc.gpsimd.tensor_tensor(tqB[0:25].rearrange("p c q -> p (c q)"),
                                            tqhB[:],
                                            svB[0:25].rearrange("p c q -> p (c q)").bitcast(F32),
                                            op=ADD)
                    # mdv' = mT . (0.5*s*v)  (rows 25..33 of svB stay 0 -> uncentered)
                    mdv = ps2.tile([128, C, QL], F32, tag="mdv")
                    for c in range(C):
                        nc.tensor.matmul(mdv[:, c, :], mTc1[:, c, :], svA[:, c, :],
                                         start=True, stop=False)
                        nc.tensor.matmul(mdv[:, c, :], mTc2[:, c, :], svB[:, c, :],
                                         start=False, stop=True)
                    # a += p * s * mdv_raw = p * (2*mdv')
                    if it == 0:
                        a_t = sb2.tile([128, CQ], F32, tag="a")
                        nc.vector.scalar_tensor_tensor(a_t[:], mdv[:].rearrange("p c q -> p (c q)"),
                                                       2.0, p_t[:], op0=MUL, op1=MUL)
                    else:
                        pm2 = sb2.tile([128, CQ], F32, tag="pm2")
                        nc.vector.scalar_tensor_tensor(pm2[:], mdv[:].rearrange("p c q -> p (c q)"),
                                                       2.0, p_t[:], op0=MUL, op1=MUL)
                        a_new = sb2.tile([128, CQ], F32, tag="a")
                        nc.vector.tensor_tensor(a_new[:], a_t[:], pm2[:], op=ADD)
                        a_t = a_new
                    if dbg:
                        nc.sync.dma_start(dbg_d["a1" if it == 0 else "a2"][:], a_t[:])

                    rr = pearson()

                # ---------------- final ------------------------------------
                p_t = p_from_rr(rr)
                if dbg:
                    nc.sync.dma_start(dbg_d["p3"][:], p_t[:])
                ea = sb2.tile([128, CQ], F32, tag="ea")
                nc.scalar.activation(ea[:], a_t[:], ACT.Exp)
                asum = sb2.tile([128, QL], F32, tag="asum")
                nc.vector.tensor_reduce(asum[:], ea[:].rearrange("p (c q) -> p q c", c=C),
                                        axis=AX, op=ADD)
                rs = sb2.tile([128, QL], F32, tag="rs")
                nc.vector.reciprocal(rs[:], asum[:])
                dd = sb2.tile([128, C, QL], F32, tag="dd")
                nc.vector.tensor_tensor(
                    dd[:], ea[:].rearrange("p (c q) -> p c q", c=C),
                    rs[:].rearrange("p (a q) -> p a q", a=1).broadcast_to((128, C, QL)), op=MUL)
                dspF = sb2.tile([128, C, QL], DT, tag="dsp")
                nc.vector.tensor_tensor(dspF[:].rearrange("p c q -> p (c q)"),
                                        dd[:].rearrange("p c q -> p (c q)"), p_t[:], op=ADD)

                # final hv: per-class matmul, N=256 window (cols 765+ are zero).
                # n2q via Act Square+accum straight from PSUM; hvF copies on DVE.
                hvF = sb.tile([QL, CD], F32, tag="hvF")
                n2q = sb2.tile([QL, C], F32, tag="n2q")
                for c in range(C):
                    fps = ps2.tile([QL, 256], F32, tag=("hvA" if c % 2 == 0 else "mdv"))
                    nc.tensor.matmul(fps[:], dspF[:, c, :], hat_m_r[:, D * c:D * c + 256],
                                     start=True, stop=True)
                    sqf = sb2.tile([QL, D], F32, tag="sqf")
                    nc.scalar.activation(sqf[:], fps[:, 0:D], ACT.Square,
                                         accum_out=n2q[:, c:c + 1])
                    nc.vector.tensor_copy(hvF[:, D * c:D * (c + 1)], fps[:, 0:D])
                if dbg:
                    nc.sync.dma_start(dbg_d["hvFd"][:], hvF[:])
                    nc.sync.dma_start(dbg_d["n2qd"][:], n2q[:])
                # fs = squash scale [QL, C] (positive)
                fp1 = sb2.tile([QL, C], F32, tag="fp1")
                nc.vector.tensor_scalar(fp1[:], n2q[:], 1.0, None, op0=ADD)
                fr1 = sb2.tile([QL, C], F32, tag="fr1")
                nc.vector.reciprocal(fr1[:], fp1[:])
                fln = sb2.tile([QL, C], F32, tag="fln")
                nc.scalar.activation(fln[:], n2q[:], ACT.Ln, bias=epsb[0:QL, :])
                fr2 = sb2.tile([QL, C], F32, tag="fr2")
                nc.scalar.activation(fr2[:], fln[:], ACT.Exp, scale=-0.5)
                omr = sb2.tile([QL, C], F32, tag="omr")
                nc.vector.tensor_scalar(omr[:], fr1[:], -1.0, 1.0, op0=MUL, op1=ADD)
                fs = sb2.tile([QL, C], F32, tag="fs")
                nc.vector.tensor_tensor(fs[:], omr[:], fr2[:], op=MUL)
                outT = sb.tile([QL, CD], F32, tag="outT")
                for c in range(C):
                    if c % 2 == 0:
                        nc.vector.tensor_scalar(outT[:, D * c:D * (c + 1)],
                                                hvF[:, D * c:D * (c + 1)],
                                                fs[:, c:c + 1], None, op0=MUL)
                    else:
                        nc.scalar.activation(outT[:, D * c:D * (c + 1)],
                                             hvF[:, D * c:D * (c + 1)], ACT.Copy,
                                             scale=fs[:, c:c + 1])
                nc.sync.dma_start(out_d[:], outT[:])

    # All activations use only {Ln, Exp, Copy, Square}, which live together in
    # act func set 6 (natural_log_exp_and_others). The default solver alternates
    # sets, inserting table reloads (~1.3us each); one load suffices.
    def _single_act_table_load():
        inst = mybir.InstLoadActFuncSet(
            name=nc.get_next_instruction_name(), ins=[], outs=[],
            act_func_set_id=6,
        )
        inst.engine = mybir.EngineType.Activation
        nc.register_instruction(inst)
        for blk in nc.main_func.blocks:
            for idx, bi in enumerate(blk.instructions):
                if isinstance(bi, mybir.InstActivation):
                    blk.instructions.insert(idx, inst)
                    return
        raise AssertionError("no activation found")

    nc.insert_act_table_loads = _single_act_table_load
    nc.compile()
    return nc


_CACHE = {}
LAST_EXEC_NS = None
LAST_RESULTS = None


def kernel(m, q, W, b):
    m = np.asarray(m, dtype=np.float32)
    q = np.asarray(q, dtype=np.float32)
    W = np.asarray(W, dtype=np.float32)
    b = np.asarray(b, dtype=np.float32)
    assert m.shape == (I, K) and q.shape == (NCORES * QL, K) and W.shape == (K, CD)

    with_bias = bool(np.any(b))
    dbg = bool(int(os.environ.get("KERNEL_DBG", "0")))
    key = ("v2", with_bias, str(DT), dbg)
    if key not in _CACHE:
        _CACHE[key] = build(with_bias, dbg)
    nc = _CACHE[key]

    Wp = np.zeros((K, NPAD), dtype=np.float32)
    Wp[:, :CD] = W
    mT = np.ascontiguousarray(m.T)
    eye = np.eye(128, dtype=np.float32)

    in_maps = []
    for i in range(NCORES):
        qT = np.ascontiguousarray(q[QL * i:QL * (i + 1)].T)
        im = {"mT": mT, "qT": qT, "Wp": Wp, "eye": eye}
        if with_bias:
            im["b"] = b.reshape(1, CD)
        in_maps.append(im)

    res = run_bass_kernel_spmd(nc, in_maps, list(range(NCORES)))
    global LAST_EXEC_NS, LAST_RESULTS
    LAST_EXEC_NS = res.exec_time_ns
    LAST_RESULTS = res.results
    out = np.concatenate([res.results[i]["out"] for i in range(NCORES)], axis=0)
    return out.astype(np.float32)


if __name__ == "__main__":
    rng = np.random.default_rng(0)
    m = rng.standard_normal((I, K)).astype(np.float32)
    q = rng.standard_normal((NCORES * QL, K)).astype(np.float32)
    W = (rng.standard_normal((K, CD)) * 0.02).astype(np.float32)
    b = np.zeros((CD,), dtype=np.float32)
    out = kernel(m=m, q=q, W=W, b=b)
    print("out", out.shape, out.dtype, np.abs(out).mean())


# revision 27
# speedup vs baseline: 1.4921x; 1.0333x over previous
"""DMR induction routing kernel for Trainium2 (Bass/Tile), 8-core data-parallel.

Problem: nn_DMRInduction. Full inputs:
  m [128, 768], q [256, 768], W [768, 765], b [765] -> out [256, 765] fp32.

Sharding: Q=256 split 8 ways (32 queries/core); m, W, b replicated.

v2 layout/dataflow (per core):
  - hat_m_r  [I=128, 1024] (I on partitions; cols 0..764 = m @ W, cols 765+ zero
      so the final per-class matmuls can stream N=256 on the fp32r fast path)
  - mTc1 [128, C, 128] / mTc2 [34, C, 128]: per-class transposes of hat_m
      (d on partitions). mTc2 row 32 = -mean_d(hat_m) per (c,i), computed by a
      ones-matmul over mTc + scaled copy, so the pearson numerator matmul is
      centered for free (row 32 of tqB carries colsum(tq)).
  - tq [d, (c,q)]: computed DIRECTLY transposed from qT/W with 60 small
      matmuls into PSUM (no PE eye-transposes for the q side at all).
  - routing state a, p, dsp: [I=128, C*Q=160].
  - squash/pearson scalars on [1, 160] rows; broadcasts via 1-row matmuls.
  - iteration v is consumed straight from PSUM (svA = hv_psum * 0.5s); the
    m-dot-v matmul runs on the scaled v, so a += p*(2*mdv') needs one fused op.
"""
import os
import sys

for _p in ("/opt/trn_rl_repo", "/root/.axon_site/_ro/trn_rl_repo"):
    if os.path.isdir(_p) and _p not in sys.path:
        sys.path.insert(0, _p)

import numpy as np
import concourse.bass as bass
import concourse.bacc as bacc
import concourse.mybir as mybir
import concourse.tile as tile
from concourse.bass_utils import run_bass_kernel_spmd

F32 = mybir.dt.float32
# float32r uses the fast PE path (1 cyc/row at N>=256 vs 4) at ~2.5e-4
# scale-relative output error (tolerance is 2e-2). KERNEL_MM_DT=float32
# restores exact matmuls.
DT = getattr(mybir.dt, os.environ.get("KERNEL_MM_DT", "float32r"))

NCORES = 8
I = 128         # memory capsules
C = 5           # capsule classes
D = 153         # dim per capsule
CD = C * D      # 765
K = 768         # input dim
KC = K // 128   # 6 contraction chunks
QL = 32         # queries per core
CQ = C * QL     # 160
NPAD = 768      # W padded to 768 cols so fp32r matmuls stream N>=256
HM_W = 1024     # hat_m_r padded width (final matmuls read 256-wide windows)
EPS = 1e-8
AX = mybir.AxisListType.X
MUL = mybir.AluOpType.mult
ADD = mybir.AluOpType.add
SUB = mybir.AluOpType.subtract
ACT = mybir.ActivationFunctionType


def build(with_bias: bool, dbg: bool = False):
    nc = bacc.Bacc("TRN2", target_bir_lowering=False, debug=False)

    mT_d = nc.dram_tensor("mT", [K, I], F32, kind="ExternalInput")
    qT_d = nc.dram_tensor("qT", [K, QL], F32, kind="ExternalInput")
    W_d = nc.dram_tensor("Wp", [K, NPAD], F32, kind="ExternalInput")
    eye_d = nc.dram_tensor("eye", [128, 128], F32, kind="ExternalInput")
    if with_bias:
        b_d = nc.dram_tensor("b", [1, CD], F32, kind="ExternalInput")
    out_d = nc.dram_tensor("out", [QL, CD], F32, kind="ExternalOutput")
    dbg_d = {}
    if dbg:
        for nm, shp in [("hatm", [128, CD]), ("tqA0", [128, CQ]), ("tqB0", [34, CQ]),
                        ("mTc1d", [128, C * 128]), ("mTc2d", [34, C * 128]),
                        ("invxn", [128, C]), ("p1", [128, CQ]), ("a1", [128, CQ]),
                        ("p2", [128, CQ]), ("a2", [128, CQ]), ("p3", [128, CQ]),
                        ("hvFd", [QL, CD]), ("n2qd", [QL, C])]:
            dbg_d[nm] = nc.dram_tensor("dbg_" + nm, shp, F32, kind="ExternalOutput")

    with tile.TileContext(nc) as tc:
        with (
            tc.tile_pool(name="sb", bufs=1) as sb,
            tc.tile_pool(name="sb2", bufs=3) as sb2,
        ):
            # ---------------- input DMAs (order = HWDGE serial order) -------
            mT_sb = sb.tile([128, KC, I], DT, tag="mT")
            qT_sb = sb.tile([128, KC, QL], DT, tag="qT")
            W_sb = sb.tile([128, KC, NPAD], DT, tag="W")
            eye = sb.tile([128, 128], F32, tag="eye")
            nc.sync.dma_start(mT_sb[:], mT_d[:].rearrange("(k p) n -> p k n", p=128).bitcast(DT))
            nc.sync.dma_start(qT_sb[:], qT_d[:].rearrange("(k p) n -> p k n", p=128).bitcast(DT))
            Wr = W_d[:].rearrange("(k p) n -> p k n", p=128).bitcast(DT)
            for k in range(KC):
                nc.sync.dma_start(W_sb[:, k, 0:512], Wr[:, k, 0:512])
            for k in range(KC):
                nc.sync.dma_start(W_sb[:, k, 512:768], Wr[:, k, 512:768])
            nc.sync.dma_start(eye[:], eye_d[:])
            if with_bias:
                b_sb = sb.tile([1, CD], F32, tag="b")
                nc.sync.dma_start(b_sb[:], b_d[:])

            # ---------------- constants (no DMA) ----------------------------
            # float32r tiles cannot be memset directly; memset F32 staging and
            # copy through Act/DVE (engine writes perform the f32r rounding).
            zf = sb.tile([128, 640], F32, tag="zf")
            nc.vector.memset(zf[:], 0.0)
            of = sb.tile([128, 1], F32, tag="of")
            nc.vector.memset(of[:], 1.0)
            o1f = sb.tile([1, 128], F32, tag="o1f")
            nc.vector.memset(o1f[:], 1.0)
            nhf = sb.tile([1, 128], F32, tag="nhf")
            nc.vector.memset(nhf[:], -0.5)
            epsb = sb.tile([128, 1], F32, tag="epsb")
            nc.vector.memset(epsb[:], EPS)

            ones1 = sb.tile([1, 128], DT, tag="ones1")
            nc.scalar.copy(ones1[:], o1f[:])
            twos1 = sb.tile([1, 128], DT, tag="twos1")
            nc.scalar.activation(twos1[:], o1f[:], ACT.Copy, scale=2.0)
            nhalf1 = sb.tile([1, 128], DT, tag="nhalf1")
            nc.vector.tensor_copy(nhalf1[:], nhf[:])
            onesF = sb.tile([128, 1], DT, tag="onesF")
            nc.scalar.copy(onesF[:], of[:])
            if with_bias:
                onesq = sb.tile([1, QL], DT, tag="onesq")
                nc.vector.tensor_copy(onesq[:], o1f[:, 0:QL])

            # persistent tiles that need zero rows
            hat_m_r = sb.tile([128, HM_W], DT, tag="hatmr")
            nc.vector.tensor_copy(hat_m_r[:, CD:HM_W], zf[:, 0:HM_W - CD])
            mTc1 = sb.tile([128, C, 128], DT, tag="mTc1")
            mTc2 = sb.tile([34, C, 128], DT, tag="mTc2")
            nc.scalar.copy(mTc2[:].rearrange("p c n -> p (c n)"), zf[0:34, 0:640])
            tqA = sb.tile([128, C, QL], DT, tag="tqA")
            tqB = sb.tile([34, C, QL], DT, tag="tqB")
            nc.vector.tensor_copy(tqB[:].rearrange("p c q -> p (c q)"), zf[0:34, 0:CQ])
            svA = sb.tile([128, C, QL], DT, tag="svA")
            svB = sb.tile([34, C, QL], DT, tag="svB")
            nc.scalar.copy(svB[:].rearrange("p c q -> p (c q)"), zf[0:34, 0:CQ])

            tqAf = tqA[:].bitcast(F32).rearrange("p c q -> p (c q)")
            tqBf25 = tqB[0:25].bitcast(F32).rearrange("p c q -> p (c q)")

            # ---------------- projections ----------------------------------
            with tc.tile_pool(name="ps1", bufs=1, space="PSUM") as ps1, \
                 tc.tile_pool(name="pstp", bufs=4, space="PSUM") as pstp:
                psA = ps1.tile([128, 512], F32, tag="psA")
                psB = ps1.tile([128, 256], F32, tag="psB")
                psQA = ps1.tile([128, C, QL], F32, tag="psQA")
                psQB = ps1.tile([34, C, QL], F32, tag="psQB")

                # hat_q (DIRECTLY transposed: out[d,(c,q)] = sum_k W[k,cD+d] q[q,k])
                # shares one bank across classes; start=True clears the whole
                # bank's has_written bits, so each (c, piece) group runs its
                # start..stop back-to-back. k is split in halves combined via
                # SBUF; classes are split c0-2 / c3-4 because c0-2 only read the
                # A columns (<512) of W, which stream in first.
                def q_cs(h, cs):
                    ks = range(3 * h, 3 * h + 3)
                    add_bias = with_bias and h == 1
                    for c in cs:
                        for j, k in enumerate(ks):
                            nc.tensor.matmul(psQA[:, c, :], W_sb[:, k, D * c:D * c + 128],
                                             qT_sb[:, k, :], start=(j == 0),
                                             stop=(j == 2 and not add_bias))
                        if add_bias:
                            nc.tensor.matmul(psQA[:, c, :], b_sb[:, D * c:D * c + 128],
                                             onesq[:], start=False, stop=True)
                        for j, k in enumerate(ks):
                            nc.tensor.matmul(psQB[0:25, c, :], W_sb[:, k, D * c + 128:D * (c + 1)],
                                             qT_sb[:, k, :], start=(j == 0),
                                             stop=(j == 2 and not add_bias))
                        if add_bias:
                            nc.tensor.matmul(psQB[0:25, c, :], b_sb[:, D * c + 128:D * (c + 1)],
                                             onesq[:], start=False, stop=True)

                def q_copy(cs):
                    c0, c1 = cs[0], cs[-1] + 1
                    nc.vector.tensor_copy(tqA[:, c0:c1, :].rearrange("p c q -> p (c q)"),
                                          psQA[:, c0:c1, :].rearrange("p c q -> p (c q)"))
                    nc.vector.tensor_copy(tqB[0:25, c0:c1, :].rearrange("p c q -> p (c q)"),
                                          psQB[0:25, c0:c1, :].rearrange("p c q -> p (c q)"))

                def q_add(cs):
                    c0, c1 = cs[0], cs[-1] + 1
                    nc.vector.tensor_tensor(
                        tqA[:, c0:c1, :].rearrange("p c q -> p (c q)"),
                        tqA[:, c0:c1, :].bitcast(F32).rearrange("p c q -> p (c q)"),
                        psQA[:, c0:c1, :].rearrange("p c q -> p (c q)"), op=ADD)
                    nc.vector.tensor_tensor(
                        tqB[0:25, c0:c1, :].rearrange("p c q -> p (c q)"),
                        tqB[0:25, c0:c1, :].bitcast(F32).rearrange("p c q -> p (c q)"),
                        psQB[0:25, c0:c1, :].rearrange("p c q -> p (c q)"), op=ADD)

                def tposes(cs):
                    for c in cs:
                        t1 = pstp.tile([128, 128], F32, tag="tp")
                        nc.tensor.transpose(t1[:], hat_m_r[:, D * c:D * c + 128].bitcast(F32), eye[:])
                        (nc.vector.tensor_copy if c % 2 else nc.scalar.copy)(mTc1[:, c, :], t1[:])
                        t2 = pstp.tile([25, 128], F32, tag="tp")
                        nc.tensor.transpose(t2[:], hat_m_r[:, D * c + 128:D * (c + 1)].bitcast(F32),
                                            eye[:])
                        (nc.scalar.copy if c % 2 else nc.vector.tensor_copy)(mTc2[0:25, c, :], t2[:])

                # ---- A-column half: hat_m cols 0:512, q classes 0-2 ----------
                for k in range(KC):
                    nc.tensor.matmul(psA[:], mT_sb[:, k, :], W_sb[:, k, 0:512],
                                     start=(k == 0), stop=(k == KC - 1 and not with_bias))
                if with_bias:
                    nc.tensor.matmul(psA[:], ones1[:], b_sb[:, 0:512], start=False, stop=True)
                q_cs(0, [0, 1, 2])
                q_copy([0, 1, 2])
                q_cs(1, [0, 1, 2])
                q_add([0, 1, 2])
                nc.scalar.copy(hat_m_r[:, 0:256], psA[:, 0:256])
                nc.vector.tensor_copy(hat_m_r[:, 256:512], psA[:, 256:512])
                tposes([0, 1, 2])

                # ---- B-column half: hat_m cols 512:765, q classes 3-4 --------
                for k in range(KC):
                    nc.tensor.matmul(psB[:], mT_sb[:, k, :], W_sb[:, k, 512:768],
                                     start=(k == 0), stop=(k == KC - 1 and not with_bias))
                if with_bias:
                    nc.tensor.matmul(psB[:, 0:253], ones1[:], b_sb[:, 512:765],
                                     start=False, stop=True)
                q_cs(0, [3, 4])
                q_copy([3, 4])
                q_cs(1, [3, 4])
                q_add([3, 4])
                nc.vector.tensor_copy(hat_m_r[:, 512:765], psB[:, 0:253])
                tposes([3, 4])

                hm32 = hat_m_r[:, 0:765].bitcast(F32)

                # mTc2 row 32 = -mean_d(hat_m)[c,i] via ones-matmul over mTc
                # (split 512+128: PSUM banks hold 512 fp32/partition max;
                #  reuses the psA/psB banks, free after the hat_m copies)
                psMuA = ps1.tile([1, 512], F32, tag="psA")
                psMuB = ps1.tile([1, 128], F32, tag="psB")
                mTc1f = mTc1[:].rearrange("p c n -> p (c n)")
                mTc2f = mTc2[0:25].rearrange("p c n -> p (c n)")
                nc.tensor.matmul(psMuA[:], onesF[:], mTc1f[:, 0:512],
                                 start=True, stop=False)
                nc.tensor.matmul(psMuA[:], onesF[0:25], mTc2f[:, 0:512],
                                 start=False, stop=True)
                nc.tensor.matmul(psMuB[:], onesF[:], mTc1f[:, 512:640],
                                 start=True, stop=False)
                nc.tensor.matmul(psMuB[:], onesF[0:25], mTc2f[:, 512:640],
                                 start=False, stop=True)
                mTc2r32 = mTc2[32:33, :, :].rearrange("p c n -> p (c n)")
                nc.scalar.activation(mTc2r32[:, 0:512], psMuA[:], ACT.Copy, scale=-1.0 / D)
                nc.scalar.activation(mTc2r32[:, 512:640], psMuB[:], ACT.Copy, scale=-1.0 / D)

                # ------------- m stats part 1: sum_d hat_m^2 ----------------
                xn2r = sb.tile([128, C], F32, tag="xn2r")
                sqs = sb.tile([128, D], F32, tag="sqs")
                for c in range(C):
                    nc.vector.scalar_tensor_tensor(
                        sqs[:], hm32[:, D * c:D * (c + 1)], 1.0,
                        hm32[:, D * c:D * (c + 1)], op0=MUL, op1=MUL,
                        accum_out=xn2r[:, c:c + 1])

                # ------------- pearson #1, tq-only part (setup banks) --------
                sqA1 = sb2.tile([128, CQ], DT, tag="sqA")
                nc.gpsimd.tensor_tensor(sqA1[:], tqAf, tqAf, op=MUL)
                sqB1 = sb2.tile([25, CQ], DT, tag="sqB")
                nc.vector.tensor_tensor(sqB1[:], tqBf25, tqBf25, op=MUL)
                colsum1 = ps1.tile([1, CQ], F32, tag="psQA")
                nc.tensor.matmul(colsum1[:], onesF[:],
                                 tqA[:].rearrange("p c q -> p (c q)"),
                                 start=True, stop=False)
                nc.tensor.matmul(colsum1[:], onesF[0:25],
                                 tqB[0:25].rearrange("p c q -> p (c q)"),
                                 start=False, stop=True)
                tqB32w = tqB[32:33, :, :].rearrange("p c q -> p (c q)")
                nc.scalar.copy(tqB32w[:], colsum1[:])
                yn2r1 = ps1.tile([1, CQ], F32, tag="psQB")
                nc.tensor.matmul(yn2r1[:], onesF[:], sqA1[:], start=True, stop=False)
                nc.tensor.matmul(yn2r1[:], onesF[0:25], sqB1[:], start=False, stop=True)
                T2 = sb.tile([1, CQ], F32, tag="T2")
                nc.vector.tensor_copy(T2[:], yn2r1[:])
                tqB32f = tqB[32:33, :, :].bitcast(F32).rearrange("p c q -> p (c q)")
                csqv1 = sb2.tile([1, CQ], F32, tag="csqv")
                nc.vector.tensor_tensor(csqv1[:], tqB32f, tqB32f, op=MUL)
                yn21 = sb2.tile([1, CQ], F32, tag="yn2")
                nc.vector.scalar_tensor_tensor(yn21[:], csqv1[:], -1.0 / D, yn2r1[:],
                                               op0=MUL, op1=ADD)
                lyn1 = sb2.tile([1, CQ], F32, tag="lyn")
                nc.scalar.activation(lyn1[:], yn21[:], ACT.Ln)
                inv_yn1 = sb2.tile([1, CQ], DT, tag="invyn")
                nc.scalar.activation(inv_yn1[:], lyn1[:], ACT.Exp, scale=-0.5)

            if dbg:
                nc.sync.dma_start(dbg_d["hatm"][:], hm32)
                nc.sync.dma_start(dbg_d["tqA0"][:], tqAf)
                nc.sync.dma_start(dbg_d["tqB0"][:], tqB[:].bitcast(F32).rearrange("p c q -> p (c q)"))
                nc.sync.dma_start(dbg_d["mTc1d"][:], mTc1[:].bitcast(F32).rearrange("p c n -> p (c n)"))
                nc.sync.dma_start(dbg_d["mTc2d"][:], mTc2[:].bitcast(F32).rearrange("p c n -> p (c n)"))

            inv_xn = sb.tile([128, C], F32, tag="invxn")
            ixb = inv_xn[:].rearrange("p (c a) -> p c a", a=1).broadcast_to((128, C, QL))

            # ---------------- routing --------------------------------------
            with tc.tile_pool(name="ps2", bufs=1, space="PSUM") as ps2:

                def pearson_late(inv_yn, lhs1, extra=None):
                    iyb = ps2.tile([128, CQ], F32, tag="bcast")
                    nc.tensor.matmul(iyb[:], lhs1, inv_yn[:], start=True, stop=True)
                    if extra is not None:
                        extra()  # setup-only work that must precede the num matmuls
                    # num[i,(c,q)]: per-class A/B pairs back-to-back (groups share
                    # one bank; start=True clears the bank's has_written bits)
                    num = ps2.tile([128, C, QL], F32, tag="num")
                    for c in range(C):
                        nc.tensor.matmul(num[:, c, :], mTc1[:, c, :], tqA[:, c, :],
                                         start=True, stop=False)
                        nc.tensor.matmul(num[:, c, :], mTc2[:, c, :], tqB[:, c, :],
                                         start=False, stop=True)
                    # p = tanh(num * inv_xn * inv_yn); tanh(x) = 1 - 2/(1+exp(2x))
                    pp1 = sb2.tile([128, C, QL], F32, tag="pp1")
                    nc.vector.tensor_tensor(pp1[:], num[:], ixb, op=MUL)
                    pp = sb2.tile([128, CQ], F32, tag="pp")
                    nc.vector.tensor_tensor(pp[:], pp1[:].rearrange("p c q -> p (c q)"),
                                            iyb[:], op=MUL)
                    e2 = sb2.tile([128, CQ], F32, tag="e2")
                    nc.scalar.activation(e2[:], pp[:], ACT.Exp, scale=2.0)
                    den = sb2.tile([128, CQ], F32, tag="den")
                    nc.vector.tensor_scalar(den[:], e2[:], 1.0, None, op0=ADD)
                    rr = sb2.tile([128, CQ], F32, tag="rr")
                    nc.vector.reciprocal(rr[:], den[:])
                    return rr

                def pearson(extra=None):
                    """p = tanh(centered-corr(mT, tq)); returns rr tile [128, CQ]."""
                    # squares (row 32 of tqB excluded; centering via csq)
                    sqA = sb2.tile([128, CQ], DT, tag="sqA")
                    nc.gpsimd.tensor_tensor(sqA[:], tqAf, tqAf, op=MUL)
                    sqB = sb2.tile([25, CQ], DT, tag="sqB")
                    nc.vector.tensor_tensor(sqB[:], tqBf25, tqBf25, op=MUL)
                    # colsum(tq) -> tqB row 32 (feeds the centered num matmul)
                    colsum = ps2.tile([1, CQ], F32, tag="colsum")
                    nc.tensor.matmul(colsum[:], onesF[:],
                                     tqA[:].rearrange("p c q -> p (c q)"),
                                     start=True, stop=False)
                    nc.tensor.matmul(colsum[:], onesF[0:25],
                                     tqB[0:25].rearrange("p c q -> p (c q)"),
                                     start=False, stop=True)
                    nc.scalar.copy(tqB[32:33, :, :].rearrange("p c q -> p (c q)"), colsum[:])
                    # yn2 = sum(tq^2) - colsum^2/D, centered variance of tq
                    yn2r = ps2.tile([1, CQ], F32, tag="cs2")
                    nc.tensor.matmul(yn2r[:], onesF[:], sqA[:], start=True, stop=False)
                    nc.tensor.matmul(yn2r[:], onesF[0:25], sqB[:], start=False, stop=True)
                    csqv = sb2.tile([1, CQ], F32, tag="csqv")
                    nc.vector.tensor_tensor(csqv[:], tqB32f, tqB32f, op=MUL)
                    yn2 = sb2.tile([1, CQ], F32, tag="yn2")
                    nc.vector.scalar_tensor_tensor(yn2[:], csqv[:], -1.0 / D, yn2r[:],
                                                   op0=MUL, op1=ADD)
                    lyn = sb2.tile([1, CQ], F32, tag="lyn")
                    nc.scalar.activation(lyn[:], yn2[:], ACT.Ln)
                    inv_yn = sb2.tile([1, CQ], DT, tag="invyn")
                    nc.scalar.activation(inv_yn[:], lyn[:], ACT.Exp, scale=-0.5)
                    return pearson_late(inv_yn, ones1[:])

                def stats_tail():
                    # m stats part 2: -mean[i,c] from mTc2 row 32 via 5 tiny
                    # transposes; then inv_xn = 1/sqrt(sum hm^2 - D*mean^2).
                    t_nm = ps2.tile([128, C], F32, tag="nm")
                    for c in range(C):
                        nc.tensor.transpose(t_nm[:, c:c + 1],
                                            mTc2[32:33, c, :].bitcast(F32), eye[32:33, 32:33])
                    nmean = sb.tile([128, C], F32, tag="nmean")
                    nc.vector.tensor_copy(nmean[:], t_nm[:])
                    nm2 = sb.tile([128, C], F32, tag="nm2")
                    nc.vector.tensor_tensor(nm2[:], nmean[:], nmean[:], op=MUL)
                    xn2 = sb.tile([128, C], F32, tag="xn2")
                    nc.vector.scalar_tensor_tensor(xn2[:], nm2[:], -float(D), xn2r[:],
                                                   op0=MUL, op1=ADD)
                    lxn = sb.tile([128, C], F32, tag="lxn")
                    nc.scalar.activation(lxn[:], xn2[:], ACT.Ln)
                    nc.scalar.activation(inv_xn[:], lxn[:], ACT.Exp, scale=-0.5)

                def p_from_rr(rr):
                    p_new = sb2.tile([128, CQ], F32, tag="p")
                    nc.vector.tensor_scalar(p_new[:], rr[:], -2.0, 1.0, op0=MUL, op1=ADD)
                    return p_new

                rr = pearson_late(inv_yn1, ones1[:], extra=stats_tail)
                a_t = None
                p_t = None

                for it in range(2):
                    dsp = sb2.tile([128, C, QL], DT, tag="dsp")
                    if it == 0:
                        # softmax(0) = 1/C exactly; dsp = p + 1/C straight from rr
                        nc.vector.tensor_scalar(dsp[:].rearrange("p c q -> p (c q)"),
                                                rr[:], -2.0, 1.0 + 1.0 / C, op0=MUL, op1=ADD)
                        p_t = p_from_rr(rr)
                        if dbg:
                            nc.sync.dma_start(dbg_d["p1"][:], p_t[:])
                            nc.sync.dma_start(dbg_d["invxn"][:], inv_xn[:])
                    else:
                        p_t = p_from_rr(rr)
                        if dbg:
                            nc.sync.dma_start(dbg_d["p2"][:], p_t[:])
                        ea = sb2.tile([128, CQ], F32, tag="ea")
                        nc.scalar.activation(ea[:], a_t[:], ACT.Exp)
                        asum = sb2.tile([128, QL], F32, tag="asum")
                        nc.vector.tensor_reduce(asum[:], ea[:].rearrange("p (c q) -> p q c", c=C),
                                                axis=AX, op=ADD)
                        rs = sb2.tile([128, QL], F32, tag="rs")
                        nc.vector.reciprocal(rs[:], asum[:])
                        dd = sb2.tile([128, C, QL], F32, tag="dd")
                        nc.vector.tensor_tensor(
                            dd[:], ea[:].rearrange("p (c q) -> p c q", c=C),
                            rs[:].rearrange("p (a q) -> p a q", a=1).broadcast_to((128, C, QL)),
                            op=MUL)
                        nc.vector.tensor_tensor(dsp[:].rearrange("p c q -> p (c q)"),
                                                dd[:].rearrange("p c q -> p (c q)"), p_t[:], op=ADD)

                    # hv[d,(c,q)] in PSUM (consumed in place; never copied to SBUF)
                    hvA = ps2.tile([128, C, QL], F32, tag="hvA")
                    hvB = ps2.tile([26, C, QL], F32, tag="hvB")
                    for c in range(C):
                        nc.tensor.matmul(hvA[:, c, :], hat_m_r[:, D * c:D * c + 128], dsp[:, c, :],
                                         start=True, stop=True)
                        nc.tensor.matmul(hvB[:, c, :], hat_m_r[:, D * c + 128:D * c + 154], dsp[:, c, :],
                                         start=True, stop=True)
                    hvAf = hvA[:].rearrange("p c q -> p (c q)")
                    hvBf25 = hvB[0:25].rearrange("p c q -> p (c q)")
                    # n2 = sum_d hv^2 (raw); squash scale applied later
                    sqhA = sb2.tile([128, CQ], DT, tag="sqhA")
                    nc.scalar.activation(sqhA[:], hvAf, ACT.Square)
                    sqhB = sb2.tile([25, CQ], DT, tag="sqhB")
                    nc.scalar.activation(sqhB[:], hvBf25, ACT.Square)
                    # hv staged to SBUF so sv ops can read the sBh broadcast
                    # straight from PSUM (one-PSUM-input rule); 0.5*tqB is
                    # pre-scaled on Pool while DVE runs the squash chain
                    vA = sb2.tile([128, CQ], F32, tag="vA")
                    nc.scalar.copy(vA[:], hvAf)
                    vB = sb2.tile([25, CQ], F32, tag="vB")
                    nc.vector.tensor_copy(vB[:], hvBf25)
                    tqhB = sb2.tile([25, CQ], F32, tag="tqhB")
                    nc.gpsimd.tensor_scalar(tqhB[:], tqBf25, 0.5, None, op0=MUL)
                    n2 = ps2.tile([1, CQ], F32, tag="cs2")
                    nc.tensor.matmul(n2[:], onesF[:], sqhA[:], start=True, stop=False)
                    nc.tensor.matmul(n2[:], onesF[0:25], sqhB[:], start=False, stop=True)
                    # -s = (1/(1+n2) - 1) / sqrt(n2+eps)
                    n2p1 = sb2.tile([1, CQ], F32, tag="n2p1")
                    nc.vector.tensor_scalar(n2p1[:], n2[:], 1.0, None, op0=ADD)
                    r1 = sb2.tile([1, CQ], F32, tag="r1")
                    nc.vector.reciprocal(r1[:], n2p1[:])
                    ln2 = sb2.tile([1, CQ], F32, tag="ln2")
                    nc.scalar.activation(ln2[:], n2[:], ACT.Ln, bias=epsb[0:1, :])
                    r2 = sb2.tile([1, CQ], F32, tag="r2")
                    nc.scalar.activation(r2[:], ln2[:], ACT.Exp, scale=-0.5)
                    nsrow = sb2.tile([1, CQ], DT, tag="nsrow")
                    nc.vector.scalar_tensor_tensor(nsrow[:], r1[:], 1.0, r2[:],
                                                   op0=SUB, op1=MUL)
                    # 0.5*s broadcast to all partitions: (-0.5) x (-s)
                    sBh = ps2.tile([128, CQ], F32, tag="num")
                    nc.tensor.matmul(sBh[:], nhalf1[:], nsrow[:], start=True, stop=True)
                    # sv = (0.5*s)*hv; tq = 0.5*tq + sv
                    nc.vector.tensor_tensor(svA[:].rearrange("p c q -> p (c q)"),
                                            vA[:], sBh[:], op=MUL)
                    nc.vector.tensor_tensor(svB[0:25].rearrange("p c q -> p (c q)"),
                                            vB[:], sBh[0:25, :], op=MUL)
                    nc.vector.scalar_tensor_tensor(tqA[:].rearrange("p c q -> p (c q)"),
                                                   tqAf, 0.5,
                                                   svA[:].rearrange("p c q -> p (c q)").bitcast(F32),
                                                   op0=MUL, op1=ADD)
                    nc.gpsimd.tensor_tensor(tqB[0:25].rearrange("p c q -> p (c q)"),
                                            tqhB[:],
                                            svB[0:25].rearrange("p c q -> p (c q)").bitcast(F32),
                                            op=ADD)
                    # mdv' = mT . (0.5*s*v)  (rows 25..33 of svB stay 0 -> uncentered)
                    mdv = ps2.tile([128, C, QL], F32, tag="mdv")
                    for c in range(C):
                        nc.tensor.matmul(mdv[:, c, :], mTc1[:, c, :], svA[:, c, :],
                                         start=True, stop=False)
                        nc.tensor.matmul(mdv[:, c, :], mTc2[:, c, :], svB[:, c, :],
                                         start=False, stop=True)
                    # a += p * s * mdv_raw = p * (2*mdv')
                    if it == 0:
                        a_t = sb2.tile([128, CQ], F32, tag="a")
                        nc.vector.scalar_tensor_tensor(a_t[:], mdv[:].rearrange("p c q -> p (c q)"),
                                                       2.0, p_t[:], op0=MUL, op1=MUL)
                    else:
                        pm2 = sb2.tile([128, CQ], F32, tag="pm2")
                        nc.vector.scalar_tensor_tensor(pm2[:], mdv[:].rearrange("p c q -> p (c q)"),
                                                       2.0, p_t[:], op0=MUL, op1=MUL)
                        a_new = sb2.tile([128, CQ], F32, tag="a")
                        nc.vector.tensor_tensor(a_new[:], a_t[:], pm2[:], op=ADD)
                        a_t = a_new
                    if dbg:
                        nc.sync.dma_start(dbg_d["a1" if it == 0 else "a2"][:], a_t[:])

                    rr = pearson()

                # ---------------- final ------------------------------------
                p_t = p_from_rr(rr)
                if dbg:
                    nc.sync.dma_start(dbg_d["p3"][:], p_t[:])
                ea = sb2.tile([128, CQ], F32, tag="ea")
                nc.scalar.activation(ea[:], a_t[:], ACT.Exp)
                asum = sb2.tile([128, QL], F32, tag="asum")
                nc.vector.tensor_reduce(asum[:], ea[:].rearrange("p (c q) -> p q c", c=C),
                                        axis=AX, op=ADD)
                rs = sb2.tile([128, QL], F32, tag="rs")
                nc.vector.reciprocal(rs[:], asum[:])
                dd = sb2.tile([128, C, QL], F32, tag="dd")
                nc.vector.tensor_tensor(
                    dd[:], ea[:].rearrange("p (c q) -> p c q", c=C),
                    rs[:].rearrange("p (a q) -> p a q", a=1).broadcast_to((128, C, QL)), op=MUL)
                dspF = sb2.tile([128, C, QL], DT, tag="dsp")
                nc.vector.tensor_tensor(dspF[:].rearrange("p c q -> p (c q)"),
                                        dd[:].rearrange("p c q -> p (c q)"), p_t[:], op=ADD)

                # final hv: per-class matmul, N=256 window (cols 765+ are zero).
                # n2q via Act Square+accum straight from PSUM; hvF copies on DVE.
                hvF = sb.tile([QL, CD], F32, tag="hvF")
                n2q = sb2.tile([QL, C], F32, tag="n2q")
                for c in range(C):
                    fps = ps2.tile([QL, 256], F32, tag=("hvA" if c % 2 == 0 else "mdv"))
                    nc.tensor.matmul(fps[:], dspF[:, c, :], hat_m_r[:, D * c:D * c + 256],
                                     start=True, stop=True)
                    sqf = sb2.tile([QL, D], F32, tag="sqf")
                    nc.scalar.activation(sqf[:], fps[:, 0:D], ACT.Square,
                                         accum_out=n2q[:, c:c + 1])
                    nc.vector.tensor_copy(hvF[:, D * c:D * (c + 1)], fps[:, 0:D])
                if dbg:
                    nc.sync.dma_start(dbg_d["hvFd"][:], hvF[:])
                    nc.sync.dma_start(dbg_d["n2qd"][:], n2q[:])
                # fs = squash scale [QL, C] (positive)
                fp1 = sb2.tile([QL, C], F32, tag="fp1")
                nc.vector.tensor_scalar(fp1[:], n2q[:], 1.0, None, op0=ADD)
                fr1 = sb2.tile([QL, C], F32, tag="fr1")
                nc.vector.reciprocal(fr1[:], fp1[:])
                fln = sb2.tile([QL, C], F32, tag="fln")
                nc.scalar.activation(fln[:], n2q[:], ACT.Ln, bias=epsb[0:QL, :])
                fr2 = sb2.tile([QL, C], F32, tag="fr2")
                nc.scalar.activation(fr2[:], fln[:], ACT.Exp, scale=-0.5)
                omr = sb2.tile([QL, C], F32, tag="omr")
                nc.vector.tensor_scalar(omr[:], fr1[:], -1.0, 1.0, op0=MUL, op1=ADD)
                fs = sb2.tile([QL, C], F32, tag="fs")
                nc.vector.tensor_tensor(fs[:], omr[:], fr2[:], op=MUL)
                outT = sb.tile([QL, CD], F32, tag="outT")
                for c in range(C):
                    if c % 2 == 0:
                        nc.vector.tensor_scalar(outT[:, D * c:D * (c + 1)],
                                                hvF[:, D * c:D * (c + 1)],
                                                fs[:, c:c + 1], None, op0=MUL)
                    else:
                        nc.scalar.activation(outT[:, D * c:D * (c + 1)],
                                             hvF[:, D * c:D * (c + 1)], ACT.Copy,
                                             scale=fs[:, c:c + 1])
                nc.sync.dma_start(out_d[:], outT[:])

    # All activations use only {Ln, Exp, Copy, Square}, which live together in
    # act func set 6 (natural_log_exp_and_others). The default solver alternates
    # sets, inserting table reloads (~1.3us each); one load suffices.
    def _single_act_table_load():
        inst = mybir.InstLoadActFuncSet(
            name=nc.get_next_instruction_name(), ins=[], outs=[],
            act_func_set_id=6,
        )
        inst.engine = mybir.EngineType.Activation
        nc.register_instruction(inst)
        for blk in nc.main_func.blocks:
            for idx, bi in enumerate(blk.instructions):
                if isinstance(bi, mybir.InstActivation):
                    blk.instructions.insert(idx, inst)
                    return
        raise AssertionError("no activation found")

    nc.insert_act_table_loads = _single_act_table_load
    nc.compile()
    return nc


_CACHE = {}
LAST_EXEC_NS = None
LAST_RESULTS = None


def kernel(m, q, W, b):
    m = np.asarray(m, dtype=np.float32)
    q = np.asarray(q, dtype=np.float32)
    W = np.asarray(W, dtype=np.float32)
    b = np.asarray(b, dtype=np.float32)
    assert m.shape == (I, K) and q.shape == (NCORES * QL, K) and W.shape == (K, CD)

    with_bias = bool(np.any(b))
    dbg = bool(int(os.environ.get("KERNEL_DBG", "0")))
    key = ("v2", with_bias, str(DT), dbg)
    if key not in _CACHE:
        _CACHE[key] = build(with_bias, dbg)
    nc = _CACHE[key]

    Wp = np.zeros((K, NPAD), dtype=np.float32)
    Wp[:, :CD] = W
    mT = np.ascontiguousarray(m.T)
    eye = np.eye(128, dtype=np.float32)

    in_maps = []
    for i in range(NCORES):
        qT = np.ascontiguousarray(q[QL * i:QL * (i + 1)].T)
        im = {"mT": mT, "qT": qT, "Wp": Wp, "eye": eye}
        if with_bias:
            im["b"] = b.reshape(1, CD)
        in_maps.append(im)

    res = run_bass_kernel_spmd(nc, in_maps, list(range(NCORES)))
    global LAST_EXEC_NS, LAST_RESULTS
    LAST_EXEC_NS = res.exec_time_ns
    LAST_RESULTS = res.results
    out = np.concatenate([res.results[i]["out"] for i in range(NCORES)], axis=0)
    return out.astype(np.float32)


if __name__ == "__main__":
    rng = np.random.default_rng(0)
    m = rng.standard_normal((I, K)).astype(np.float32)
    q = rng.standard_normal((NCORES * QL, K)).astype(np.float32)
    W = (rng.standard_normal((K, CD)) * 0.02).astype(np.float32)
    b = np.zeros((CD,), dtype=np.float32)
    out = kernel(m=m, q=q, W=W, b=b)
    print("out", out.shape, out.dtype, np.abs(out).mean())


# revision 28
# speedup vs baseline: 1.4928x; 1.0005x over previous
"""DMR induction routing kernel for Trainium2 (Bass/Tile), 8-core data-parallel.

Problem: nn_DMRInduction. Full inputs:
  m [128, 768], q [256, 768], W [768, 765], b [765] -> out [256, 765] fp32.

Sharding: Q=256 split 8 ways (32 queries/core); m, W, b replicated.

v2 layout/dataflow (per core):
  - hat_m_r  [I=128, 1024] (I on partitions; cols 0..764 = m @ W, cols 765+ zero
      so the final per-class matmuls can stream N=256 on the fp32r fast path)
  - mTc1 [128, C, 128] / mTc2 [34, C, 128]: per-class transposes of hat_m
      (d on partitions). mTc2 row 32 = -mean_d(hat_m) per (c,i), computed by a
      ones-matmul over mTc + scaled copy, so the pearson numerator matmul is
      centered for free (row 32 of tqB carries colsum(tq)).
  - tq [d, (c,q)]: computed DIRECTLY transposed from qT/W with 60 small
      matmuls into PSUM (no PE eye-transposes for the q side at all).
  - routing state a, p, dsp: [I=128, C*Q=160].
  - squash/pearson scalars on [1, 160] rows; broadcasts via 1-row matmuls.
  - iteration v is consumed straight from PSUM (svA = hv_psum * 0.5s); the
    m-dot-v matmul runs on the scaled v, so a += p*(2*mdv') needs one fused op.
"""
import os
import sys

for _p in ("/opt/trn_rl_repo", "/root/.axon_site/_ro/trn_rl_repo"):
    if os.path.isdir(_p) and _p not in sys.path:
        sys.path.insert(0, _p)

import numpy as np
import concourse.bass as bass
import concourse.bacc as bacc
import concourse.mybir as mybir
import concourse.tile as tile
from concourse.bass_utils import run_bass_kernel_spmd

F32 = mybir.dt.float32
# float32r uses the fast PE path (1 cyc/row at N>=256 vs 4) at ~2.5e-4
# scale-relative output error (tolerance is 2e-2). KERNEL_MM_DT=float32
# restores exact matmuls.
DT = getattr(mybir.dt, os.environ.get("KERNEL_MM_DT", "float32r"))

NCORES = 8
I = 128         # memory capsules
C = 5           # capsule classes
D = 153         # dim per capsule
CD = C * D      # 765
K = 768         # input dim
KC = K // 128   # 6 contraction chunks
QL = 32         # queries per core
CQ = C * QL     # 160
NPAD = 768      # W padded to 768 cols so fp32r matmuls stream N>=256
HM_W = 1024     # hat_m_r padded width (final matmuls read 256-wide windows)
EPS = 1e-8
AX = mybir.AxisListType.X
MUL = mybir.AluOpType.mult
ADD = mybir.AluOpType.add
SUB = mybir.AluOpType.subtract
ACT = mybir.ActivationFunctionType


def build(with_bias: bool, dbg: bool = False):
    nc = bacc.Bacc("TRN2", target_bir_lowering=False, debug=False)

    mT_d = nc.dram_tensor("mT", [K, I], F32, kind="ExternalInput")
    qT_d = nc.dram_tensor("qT", [K, QL], F32, kind="ExternalInput")
    W_d = nc.dram_tensor("Wp", [K, NPAD], F32, kind="ExternalInput")
    if with_bias:
        b_d = nc.dram_tensor("b", [1, CD], F32, kind="ExternalInput")
    out_d = nc.dram_tensor("out", [QL, CD], F32, kind="ExternalOutput")
    dbg_d = {}
    if dbg:
        for nm, shp in [("hatm", [128, CD]), ("tqA0", [128, CQ]), ("tqB0", [34, CQ]),
                        ("mTc1d", [128, C * 128]), ("mTc2d", [34, C * 128]),
                        ("invxn", [128, C]), ("p1", [128, CQ]), ("a1", [128, CQ]),
                        ("p2", [128, CQ]), ("a2", [128, CQ]), ("p3", [128, CQ]),
                        ("hvFd", [QL, CD]), ("n2qd", [QL, C])]:
            dbg_d[nm] = nc.dram_tensor("dbg_" + nm, shp, F32, kind="ExternalOutput")

    with tile.TileContext(nc) as tc:
        with (
            tc.tile_pool(name="sb", bufs=1) as sb,
            tc.tile_pool(name="sb2", bufs=3) as sb2,
        ):
            # ---------------- input DMAs (order = HWDGE serial order) -------
            mT_sb = sb.tile([128, KC, I], DT, tag="mT")
            qT_sb = sb.tile([128, KC, QL], DT, tag="qT")
            W_sb = sb.tile([128, KC, NPAD], DT, tag="W")
            nc.sync.dma_start(mT_sb[:], mT_d[:].rearrange("(k p) n -> p k n", p=128).bitcast(DT))
            nc.sync.dma_start(qT_sb[:], qT_d[:].rearrange("(k p) n -> p k n", p=128).bitcast(DT))
            Wr = W_d[:].rearrange("(k p) n -> p k n", p=128).bitcast(DT)
            for k in range(KC):
                nc.sync.dma_start(W_sb[:, k, 0:512], Wr[:, k, 0:512])
            for k in range(KC):
                nc.sync.dma_start(W_sb[:, k, 512:768], Wr[:, k, 512:768])
            if with_bias:
                b_sb = sb.tile([1, CD], F32, tag="b")
                nc.sync.dma_start(b_sb[:], b_d[:])

            # ---------------- constants (no DMA) ----------------------------
            # float32r tiles cannot be memset directly; memset F32 staging and
            # copy through Act/DVE (engine writes perform the f32r rounding).
            zf = sb.tile([128, 640], F32, tag="zf")
            nc.vector.memset(zf[:], 0.0)
            of = sb.tile([128, 1], F32, tag="of")
            nc.vector.memset(of[:], 1.0)
            o1f = sb.tile([1, 128], F32, tag="o1f")
            nc.vector.memset(o1f[:], 1.0)
            nhf = sb.tile([1, 128], F32, tag="nhf")
            nc.vector.memset(nhf[:], -0.5)
            epsb = sb.tile([128, 1], F32, tag="epsb")
            nc.vector.memset(epsb[:], EPS)
            # identity for PE transposes, built on-chip (a DMA would arrive
            # ~12us in, after the W pieces, and gate the whole transpose block)
            eye = sb.tile([128, 128], F32, tag="eye")
            nc.vector.memset(eye[:], 1.0)
            nc.gpsimd.affine_select(eye[:], eye[:], pattern=[[-1, 128]],
                                    compare_op=mybir.AluOpType.is_equal,
                                    fill=0.0, base=0, channel_multiplier=1)

            ones1 = sb.tile([1, 128], DT, tag="ones1")
            nc.scalar.copy(ones1[:], o1f[:])
            twos1 = sb.tile([1, 128], DT, tag="twos1")
            nc.scalar.activation(twos1[:], o1f[:], ACT.Copy, scale=2.0)
            nhalf1 = sb.tile([1, 128], DT, tag="nhalf1")
            nc.vector.tensor_copy(nhalf1[:], nhf[:])
            onesF = sb.tile([128, 1], DT, tag="onesF")
            nc.scalar.copy(onesF[:], of[:])
            if with_bias:
                onesq = sb.tile([1, QL], DT, tag="onesq")
                nc.vector.tensor_copy(onesq[:], o1f[:, 0:QL])

            # persistent tiles that need zero rows
            hat_m_r = sb.tile([128, HM_W], DT, tag="hatmr")
            nc.vector.tensor_copy(hat_m_r[:, CD:HM_W], zf[:, 0:HM_W - CD])
            mTc1 = sb.tile([128, C, 128], DT, tag="mTc1")
            mTc2 = sb.tile([34, C, 128], DT, tag="mTc2")
            nc.scalar.copy(mTc2[:].rearrange("p c n -> p (c n)"), zf[0:34, 0:640])
            tqA = sb.tile([128, C, QL], DT, tag="tqA")
            tqB = sb.tile([34, C, QL], DT, tag="tqB")
            nc.vector.tensor_copy(tqB[:].rearrange("p c q -> p (c q)"), zf[0:34, 0:CQ])
            svA = sb.tile([128, C, QL], DT, tag="svA")
            svB = sb.tile([34, C, QL], DT, tag="svB")
            nc.scalar.copy(svB[:].rearrange("p c q -> p (c q)"), zf[0:34, 0:CQ])

            tqAf = tqA[:].bitcast(F32).rearrange("p c q -> p (c q)")
            tqBf25 = tqB[0:25].bitcast(F32).rearrange("p c q -> p (c q)")

            # ---------------- projections ----------------------------------
            with tc.tile_pool(name="ps1", bufs=1, space="PSUM") as ps1, \
                 tc.tile_pool(name="pstp", bufs=4, space="PSUM") as pstp:
                psA = ps1.tile([128, 512], F32, tag="psA")
                psB = ps1.tile([128, 256], F32, tag="psB")
                psQA = ps1.tile([128, C, QL], F32, tag="psQA")
                psQB = ps1.tile([34, C, QL], F32, tag="psQB")

                # hat_q (DIRECTLY transposed: out[d,(c,q)] = sum_k W[k,cD+d] q[q,k])
                # shares one bank across classes; start=True clears the whole
                # bank's has_written bits, so each (c, piece) group runs its
                # start..stop back-to-back. k is split in halves combined via
                # SBUF; classes are split c0-2 / c3-4 because c0-2 only read the
                # A columns (<512) of W, which stream in first.
                def q_cs(h, cs):
                    ks = range(3 * h, 3 * h + 3)
                    add_bias = with_bias and h == 1
                    for c in cs:
                        for j, k in enumerate(ks):
                            nc.tensor.matmul(psQA[:, c, :], W_sb[:, k, D * c:D * c + 128],
                                             qT_sb[:, k, :], start=(j == 0),
                                             stop=(j == 2 and not add_bias))
                        if add_bias:
                            nc.tensor.matmul(psQA[:, c, :], b_sb[:, D * c:D * c + 128],
                                             onesq[:], start=False, stop=True)
                        for j, k in enumerate(ks):
                            nc.tensor.matmul(psQB[0:25, c, :], W_sb[:, k, D * c + 128:D * (c + 1)],
                                             qT_sb[:, k, :], start=(j == 0),
                                             stop=(j == 2 and not add_bias))
                        if add_bias:
                            nc.tensor.matmul(psQB[0:25, c, :], b_sb[:, D * c + 128:D * (c + 1)],
                                             onesq[:], start=False, stop=True)

                def q_copy(cs):
                    c0, c1 = cs[0], cs[-1] + 1
                    nc.vector.tensor_copy(tqA[:, c0:c1, :].rearrange("p c q -> p (c q)"),
                                          psQA[:, c0:c1, :].rearrange("p c q -> p (c q)"))
                    nc.vector.tensor_copy(tqB[0:25, c0:c1, :].rearrange("p c q -> p (c q)"),
                                          psQB[0:25, c0:c1, :].rearrange("p c q -> p (c q)"))

                def q_add(cs):
                    c0, c1 = cs[0], cs[-1] + 1
                    nc.vector.tensor_tensor(
                        tqA[:, c0:c1, :].rearrange("p c q -> p (c q)"),
                        tqA[:, c0:c1, :].bitcast(F32).rearrange("p c q -> p (c q)"),
                        psQA[:, c0:c1, :].rearrange("p c q -> p (c q)"), op=ADD)
                    nc.vector.tensor_tensor(
                        tqB[0:25, c0:c1, :].rearrange("p c q -> p (c q)"),
                        tqB[0:25, c0:c1, :].bitcast(F32).rearrange("p c q -> p (c q)"),
                        psQB[0:25, c0:c1, :].rearrange("p c q -> p (c q)"), op=ADD)

                def tposes(cs):
                    for c in cs:
                        t1 = pstp.tile([128, 128], F32, tag="tp")
                        nc.tensor.transpose(t1[:], hat_m_r[:, D * c:D * c + 128].bitcast(F32), eye[:])
                        (nc.vector.tensor_copy if c % 2 else nc.scalar.copy)(mTc1[:, c, :], t1[:])
                        t2 = pstp.tile([25, 128], F32, tag="tp")
                        nc.tensor.transpose(t2[:], hat_m_r[:, D * c + 128:D * (c + 1)].bitcast(F32),
                                            eye[:])
                        (nc.scalar.copy if c % 2 else nc.vector.tensor_copy)(mTc2[0:25, c, :], t2[:])

                # ---- A-column half: hat_m cols 0:512, q classes 0-2 ----------
                for k in range(KC):
                    nc.tensor.matmul(psA[:], mT_sb[:, k, :], W_sb[:, k, 0:512],
                                     start=(k == 0), stop=(k == KC - 1 and not with_bias))
                if with_bias:
                    nc.tensor.matmul(psA[:], ones1[:], b_sb[:, 0:512], start=False, stop=True)
                q_cs(0, [0, 1, 2])
                q_copy([0, 1, 2])
                q_cs(1, [0, 1, 2])
                q_add([0, 1, 2])
                nc.scalar.copy(hat_m_r[:, 0:256], psA[:, 0:256])
                nc.vector.tensor_copy(hat_m_r[:, 256:512], psA[:, 256:512])
                tposes([0, 1, 2])

                # ---- B-column half: hat_m cols 512:765, q classes 3-4 --------
                for k in range(KC):
                    nc.tensor.matmul(psB[:], mT_sb[:, k, :], W_sb[:, k, 512:768],
                                     start=(k == 0), stop=(k == KC - 1 and not with_bias))
                if with_bias:
                    nc.tensor.matmul(psB[:, 0:253], ones1[:], b_sb[:, 512:765],
                                     start=False, stop=True)
                q_cs(0, [3, 4])
                q_copy([3, 4])
                q_cs(1, [3, 4])
                q_add([3, 4])
                nc.scalar.copy(hat_m_r[:, 512:640], psB[:, 0:128])
                nc.vector.tensor_copy(hat_m_r[:, 640:765], psB[:, 128:253])
                tposes([3, 4])

                hm32 = hat_m_r[:, 0:765].bitcast(F32)

                # mTc2 row 32 = -mean_d(hat_m)[c,i] via ones-matmul over mTc
                # (split 512+128: PSUM banks hold 512 fp32/partition max;
                #  reuses the psA/psB banks, free after the hat_m copies)
                psMuA = ps1.tile([1, 512], F32, tag="psA")
                psMuB = ps1.tile([1, 128], F32, tag="psB")
                mTc1f = mTc1[:].rearrange("p c n -> p (c n)")
                mTc2f = mTc2[0:25].rearrange("p c n -> p (c n)")
                nc.tensor.matmul(psMuA[:], onesF[:], mTc1f[:, 0:512],
                                 start=True, stop=False)
                nc.tensor.matmul(psMuA[:], onesF[0:25], mTc2f[:, 0:512],
                                 start=False, stop=True)
                nc.tensor.matmul(psMuB[:], onesF[:], mTc1f[:, 512:640],
                                 start=True, stop=False)
                nc.tensor.matmul(psMuB[:], onesF[0:25], mTc2f[:, 512:640],
                                 start=False, stop=True)
                mTc2r32 = mTc2[32:33, :, :].rearrange("p c n -> p (c n)")
                nc.scalar.activation(mTc2r32[:, 0:256], psMuA[:, 0:256], ACT.Copy, scale=-1.0 / D)
                nc.vector.tensor_scalar(mTc2r32[:, 256:512], psMuA[:, 256:512], -1.0 / D,
                                        None, op0=MUL)
                nc.scalar.activation(mTc2r32[:, 512:640], psMuB[:], ACT.Copy, scale=-1.0 / D)

                # ------------- m stats part 1: sum_d hat_m^2 ----------------
                xn2r = sb.tile([128, C], F32, tag="xn2r")
                sqs = sb.tile([128, D], F32, tag="sqs")
                for c in range(C):
                    nc.vector.scalar_tensor_tensor(
                        sqs[:], hm32[:, D * c:D * (c + 1)], 1.0,
                        hm32[:, D * c:D * (c + 1)], op0=MUL, op1=MUL,
                        accum_out=xn2r[:, c:c + 1])

                # ------------- pearson #1, tq-only part (setup banks) --------
                sqA1 = sb2.tile([128, CQ], DT, tag="sqA")
                nc.gpsimd.tensor_tensor(sqA1[:], tqAf, tqAf, op=MUL)
                sqB1 = sb2.tile([25, CQ], DT, tag="sqB")
                nc.vector.tensor_tensor(sqB1[:], tqBf25, tqBf25, op=MUL)
                colsum1 = ps1.tile([1, CQ], F32, tag="psQA")
                nc.tensor.matmul(colsum1[:], onesF[:],
                                 tqA[:].rearrange("p c q -> p (c q)"),
                                 start=True, stop=False)
                nc.tensor.matmul(colsum1[:], onesF[0:25],
                                 tqB[0:25].rearrange("p c q -> p (c q)"),
                                 start=False, stop=True)
                tqB32w = tqB[32:33, :, :].rearrange("p c q -> p (c q)")
                nc.scalar.copy(tqB32w[:], colsum1[:])
                yn2r1 = ps1.tile([1, CQ], F32, tag="psQB")
                nc.tensor.matmul(yn2r1[:], onesF[:], sqA1[:], start=True, stop=False)
                nc.tensor.matmul(yn2r1[:], onesF[0:25], sqB1[:], start=False, stop=True)
                T2 = sb.tile([1, CQ], F32, tag="T2")
                nc.vector.tensor_copy(T2[:], yn2r1[:])
                tqB32f = tqB[32:33, :, :].bitcast(F32).rearrange("p c q -> p (c q)")
                csqv1 = sb2.tile([1, CQ], F32, tag="csqv")
                nc.vector.tensor_tensor(csqv1[:], tqB32f, tqB32f, op=MUL)
                yn21 = sb2.tile([1, CQ], F32, tag="yn2")
                nc.vector.scalar_tensor_tensor(yn21[:], csqv1[:], -1.0 / D, yn2r1[:],
                                               op0=MUL, op1=ADD)
                lyn1 = sb2.tile([1, CQ], F32, tag="lyn")
                nc.scalar.activation(lyn1[:], yn21[:], ACT.Ln)
                inv_yn1 = sb2.tile([1, CQ], DT, tag="invyn")
                nc.scalar.activation(inv_yn1[:], lyn1[:], ACT.Exp, scale=-0.5)

            if dbg:
                nc.sync.dma_start(dbg_d["hatm"][:], hm32)
                nc.sync.dma_start(dbg_d["tqA0"][:], tqAf)
                nc.sync.dma_start(dbg_d["tqB0"][:], tqB[:].bitcast(F32).rearrange("p c q -> p (c q)"))
                nc.sync.dma_start(dbg_d["mTc1d"][:], mTc1[:].bitcast(F32).rearrange("p c n -> p (c n)"))
                nc.sync.dma_start(dbg_d["mTc2d"][:], mTc2[:].bitcast(F32).rearrange("p c n -> p (c n)"))

            inv_xn = sb.tile([128, C], F32, tag="invxn")
            ixb = inv_xn[:].rearrange("p (c a) -> p c a", a=1).broadcast_to((128, C, QL))

            # ---------------- routing --------------------------------------
            with tc.tile_pool(name="ps2", bufs=1, space="PSUM") as ps2:

                def pearson_late(inv_yn, lhs1, extra=None):
                    iyb = ps2.tile([128, CQ], F32, tag="bcast")
                    nc.tensor.matmul(iyb[:], lhs1, inv_yn[:], start=True, stop=True)
                    if extra is not None:
                        extra()  # setup-only work that must precede the num matmuls
                    # num[i,(c,q)]: per-class A/B pairs back-to-back (groups share
                    # one bank; start=True clears the bank's has_written bits)
                    num = ps2.tile([128, C, QL], F32, tag="num")
                    for c in range(C):
                        nc.tensor.matmul(num[:, c, :], mTc1[:, c, :], tqA[:, c, :],
                                         start=True, stop=False)
                        nc.tensor.matmul(num[:, c, :], mTc2[:, c, :], tqB[:, c, :],
                                         start=False, stop=True)
                    # p = tanh(num * inv_xn * inv_yn); tanh(x) = 1 - 2/(1+exp(2x))
                    pp1 = sb2.tile([128, C, QL], F32, tag="pp1")
                    nc.vector.tensor_tensor(pp1[:], num[:], ixb, op=MUL)
                    pp = sb2.tile([128, CQ], F32, tag="pp")
                    nc.vector.tensor_tensor(pp[:], pp1[:].rearrange("p c q -> p (c q)"),
                                            iyb[:], op=MUL)
                    e2 = sb2.tile([128, CQ], F32, tag="e2")
                    nc.scalar.activation(e2[:], pp[:], ACT.Exp, scale=2.0)
                    den = sb2.tile([128, CQ], F32, tag="den")
                    nc.vector.tensor_scalar(den[:], e2[:], 1.0, None, op0=ADD)
                    rr = sb2.tile([128, CQ], F32, tag="rr")
                    nc.vector.reciprocal(rr[:], den[:])
                    return rr

                def pearson(extra=None):
                    """p = tanh(centered-corr(mT, tq)); returns rr tile [128, CQ]."""
                    # squares (row 32 of tqB excluded; centering via csq)
                    sqA = sb2.tile([128, CQ], DT, tag="sqA")
                    nc.gpsimd.tensor_tensor(sqA[:], tqAf, tqAf, op=MUL)
                    sqB = sb2.tile([25, CQ], DT, tag="sqB")
                    nc.vector.tensor_tensor(sqB[:], tqBf25, tqBf25, op=MUL)
                    # colsum(tq) -> tqB row 32 (feeds the centered num matmul)
                    colsum = ps2.tile([1, CQ], F32, tag="colsum")
                    nc.tensor.matmul(colsum[:], onesF[:],
                                     tqA[:].rearrange("p c q -> p (c q)"),
                                     start=True, stop=False)
                    nc.tensor.matmul(colsum[:], onesF[0:25],
                                     tqB[0:25].rearrange("p c q -> p (c q)"),
                                     start=False, stop=True)
                    nc.scalar.copy(tqB[32:33, :, :].rearrange("p c q -> p (c q)"), colsum[:])
                    # yn2 = sum(tq^2) - colsum^2/D, centered variance of tq
                    yn2r = ps2.tile([1, CQ], F32, tag="cs2")
                    nc.tensor.matmul(yn2r[:], onesF[:], sqA[:], start=True, stop=False)
                    nc.tensor.matmul(yn2r[:], onesF[0:25], sqB[:], start=False, stop=True)
                    csqv = sb2.tile([1, CQ], F32, tag="csqv")
                    nc.vector.tensor_tensor(csqv[:], tqB32f, tqB32f, op=MUL)
                    yn2 = sb2.tile([1, CQ], F32, tag="yn2")
                    nc.vector.scalar_tensor_tensor(yn2[:], csqv[:], -1.0 / D, yn2r[:],
                                                   op0=MUL, op1=ADD)
                    lyn = sb2.tile([1, CQ], F32, tag="lyn")
                    nc.scalar.activation(lyn[:], yn2[:], ACT.Ln)
                    inv_yn = sb2.tile([1, CQ], DT, tag="invyn")
                    nc.scalar.activation(inv_yn[:], lyn[:], ACT.Exp, scale=-0.5)
                    return pearson_late(inv_yn, ones1[:])

                def stats_tail():
                    # m stats part 2: -mean[i,c] from mTc2 row 32 via 5 tiny
                    # transposes; then inv_xn = 1/sqrt(sum hm^2 - D*mean^2).
                    t_nm = ps2.tile([128, C], F32, tag="nm")
                    for c in range(C):
                        nc.tensor.transpose(t_nm[:, c:c + 1],
                                            mTc2[32:33, c, :].bitcast(F32), eye[32:33, 32:33])
                    nmean = sb.tile([128, C], F32, tag="nmean")
                    nc.vector.tensor_copy(nmean[:], t_nm[:])
                    nm2 = sb.tile([128, C], F32, tag="nm2")
                    nc.vector.tensor_tensor(nm2[:], nmean[:], nmean[:], op=MUL)
                    xn2 = sb.tile([128, C], F32, tag="xn2")
                    nc.vector.scalar_tensor_tensor(xn2[:], nm2[:], -float(D), xn2r[:],
                                                   op0=MUL, op1=ADD)
                    lxn = sb.tile([128, C], F32, tag="lxn")
                    nc.scalar.activation(lxn[:], xn2[:], ACT.Ln)
                    nc.scalar.activation(inv_xn[:], lxn[:], ACT.Exp, scale=-0.5)

                def p_from_rr(rr):
                    p_new = sb2.tile([128, CQ], F32, tag="p")
                    nc.vector.tensor_scalar(p_new[:], rr[:], -2.0, 1.0, op0=MUL, op1=ADD)
                    return p_new

                rr = pearson_late(inv_yn1, ones1[:], extra=stats_tail)
                a_t = None
                p_t = None

                for it in range(2):
                    dsp = sb2.tile([128, C, QL], DT, tag="dsp")
                    if it == 0:
                        # softmax(0) = 1/C exactly; dsp = p + 1/C straight from rr
                        nc.vector.tensor_scalar(dsp[:].rearrange("p c q -> p (c q)"),
                                                rr[:], -2.0, 1.0 + 1.0 / C, op0=MUL, op1=ADD)
                        p_t = p_from_rr(rr)
                        if dbg:
                            nc.sync.dma_start(dbg_d["p1"][:], p_t[:])
                            nc.sync.dma_start(dbg_d["invxn"][:], inv_xn[:])
                    else:
                        p_t = p_from_rr(rr)
                        if dbg:
                            nc.sync.dma_start(dbg_d["p2"][:], p_t[:])
                        ea = sb2.tile([128, CQ], F32, tag="ea")
                        nc.scalar.activation(ea[:], a_t[:], ACT.Exp)
                        asum = sb2.tile([128, QL], F32, tag="asum")
                        nc.vector.tensor_reduce(asum[:], ea[:].rearrange("p (c q) -> p q c", c=C),
                                                axis=AX, op=ADD)
                        rs = sb2.tile([128, QL], F32, tag="rs")
                        nc.vector.reciprocal(rs[:], asum[:])
                        dd = sb2.tile([128, C, QL], F32, tag="dd")
                        nc.vector.tensor_tensor(
                            dd[:], ea[:].rearrange("p (c q) -> p c q", c=C),
                            rs[:].rearrange("p (a q) -> p a q", a=1).broadcast_to((128, C, QL)),
                            op=MUL)
                        nc.vector.tensor_tensor(dsp[:].rearrange("p c q -> p (c q)"),
                                                dd[:].rearrange("p c q -> p (c q)"), p_t[:], op=ADD)

                    # hv[d,(c,q)] in PSUM (consumed in place; never copied to SBUF)
                    hvA = ps2.tile([128, C, QL], F32, tag="hvA")
                    hvB = ps2.tile([26, C, QL], F32, tag="hvB")
                    for c in range(C):
                        nc.tensor.matmul(hvA[:, c, :], hat_m_r[:, D * c:D * c + 128], dsp[:, c, :],
                                         start=True, stop=True)
                        nc.tensor.matmul(hvB[:, c, :], hat_m_r[:, D * c + 128:D * c + 154], dsp[:, c, :],
                                         start=True, stop=True)
                    hvAf = hvA[:].rearrange("p c q -> p (c q)")
                    hvBf25 = hvB[0:25].rearrange("p c q -> p (c q)")
                    # n2 = sum_d hv^2 (raw); squash scale applied later
                    sqhA = sb2.tile([128, CQ], DT, tag="sqhA")
                    nc.scalar.activation(sqhA[:], hvAf, ACT.Square)
                    sqhB = sb2.tile([25, CQ], DT, tag="sqhB")
                    nc.scalar.activation(sqhB[:], hvBf25, ACT.Square)
                    # hv staged to SBUF so sv ops can read the sBh broadcast
                    # straight from PSUM (one-PSUM-input rule); 0.5*tqB is
                    # pre-scaled on Pool while DVE runs the squash chain
                    vA = sb2.tile([128, CQ], F32, tag="vA")
                    nc.scalar.copy(vA[:], hvAf)
                    vB = sb2.tile([25, CQ], F32, tag="vB")
                    nc.vector.tensor_copy(vB[:], hvBf25)
                    tqhB = sb2.tile([25, CQ], F32, tag="tqhB")
                    nc.gpsimd.tensor_scalar(tqhB[:], tqBf25, 0.5, None, op0=MUL)
                    n2 = ps2.tile([1, CQ], F32, tag="cs2")
                    nc.tensor.matmul(n2[:], onesF[:], sqhA[:], start=True, stop=False)
                    nc.tensor.matmul(n2[:], onesF[0:25], sqhB[:], start=False, stop=True)
                    # -s = (1/(1+n2) - 1) / sqrt(n2+eps)
                    n2p1 = sb2.tile([1, CQ], F32, tag="n2p1")
                    nc.vector.tensor_scalar(n2p1[:], n2[:], 1.0, None, op0=ADD)
                    r1 = sb2.tile([1, CQ], F32, tag="r1")
                    nc.vector.reciprocal(r1[:], n2p1[:])
                    ln2 = sb2.tile([1, CQ], F32, tag="ln2")
                    nc.scalar.activation(ln2[:], n2[:], ACT.Ln, bias=epsb[0:1, :])
                    r2 = sb2.tile([1, CQ], F32, tag="r2")
                    nc.scalar.activation(r2[:], ln2[:], ACT.Exp, scale=-0.5)
                    nsrow = sb2.tile([1, CQ], DT, tag="nsrow")
                    nc.vector.scalar_tensor_tensor(nsrow[:], r1[:], 1.0, r2[:],
                                                   op0=SUB, op1=MUL)
                    # 0.5*s broadcast to all partitions: (-0.5) x (-s)
                    sBh = ps2.tile([128, CQ], F32, tag="num")
                    nc.tensor.matmul(sBh[:], nhalf1[:], nsrow[:], start=True, stop=True)
                    # sv = (0.5*s)*hv; tq = 0.5*tq + sv
                    nc.vector.tensor_tensor(svA[:].rearrange("p c q -> p (c q)"),
                                            vA[:], sBh[:], op=MUL)
                    nc.vector.tensor_tensor(svB[0:25].rearrange("p c q -> p (c q)"),
                                            vB[:], sBh[0:25, :], op=MUL)
                    nc.vector.scalar_tensor_tensor(tqA[:].rearrange("p c q -> p (c q)"),
                                                   tqAf, 0.5,
                                                   svA[:].rearrange("p c q -> p (c q)").bitcast(F32),
                                                   op0=MUL, op1=ADD)
                    nc.gpsimd.tensor_tensor(tqB[0:25].rearrange("p c q -> p (c q)"),
                                            tqhB[:],
                                            svB[0:25].rearrange("p c q -> p (c q)").bitcast(F32),
                                            op=ADD)
                    # mdv' = mT . (0.5*s*v)  (rows 25..33 of svB stay 0 -> uncentered)
                    mdv = ps2.tile([128, C, QL], F32, tag="mdv")
                    for c in range(C):
                        nc.tensor.matmul(mdv[:, c, :], mTc1[:, c, :], svA[:, c, :],
                                         start=True, stop=False)
                        nc.tensor.matmul(mdv[:, c, :], mTc2[:, c, :], svB[:, c, :],
                                         start=False, stop=True)
                    # a += p * s * mdv_raw = p * (2*mdv')
                    if it == 0:
                        a_t = sb2.tile([128, CQ], F32, tag="a")
                        nc.vector.scalar_tensor_tensor(a_t[:], mdv[:].rearrange("p c q -> p (c q)"),
                                                       2.0, p_t[:], op0=MUL, op1=MUL)
                    else:
                        pm2 = sb2.tile([128, CQ], F32, tag="pm2")
                        nc.vector.scalar_tensor_tensor(pm2[:], mdv[:].rearrange("p c q -> p (c q)"),
                                                       2.0, p_t[:], op0=MUL, op1=MUL)
                        a_new = sb2.tile([128, CQ], F32, tag="a")
                        nc.vector.tensor_tensor(a_new[:], a_t[:], pm2[:], op=ADD)
                        a_t = a_new
                    if dbg:
                        nc.sync.dma_start(dbg_d["a1" if it == 0 else "a2"][:], a_t[:])

                    rr = pearson()

                # ---------------- final ------------------------------------
                p_t = p_from_rr(rr)
                if dbg:
                    nc.sync.dma_start(dbg_d["p3"][:], p_t[:])
                ea = sb2.tile([128, CQ], F32, tag="ea")
                nc.scalar.activation(ea[:], a_t[:], ACT.Exp)
                asum = sb2.tile([128, QL], F32, tag="asum")
                nc.vector.tensor_reduce(asum[:], ea[:].rearrange("p (c q) -> p q c", c=C),
                                        axis=AX, op=ADD)
                rs = sb2.tile([128, QL], F32, tag="rs")
                nc.vector.reciprocal(rs[:], asum[:])
                dd = sb2.tile([128, C, QL], F32, tag="dd")
                nc.vector.tensor_tensor(
                    dd[:], ea[:].rearrange("p (c q) -> p c q", c=C),
                    rs[:].rearrange("p (a q) -> p a q", a=1).broadcast_to((128, C, QL)), op=MUL)
                dspF = sb2.tile([128, C, QL], DT, tag="dsp")
                nc.vector.tensor_tensor(dspF[:].rearrange("p c q -> p (c q)"),
                                        dd[:].rearrange("p c q -> p (c q)"), p_t[:], op=ADD)

                # final hv: per-class matmul, N=256 window (cols 765+ are zero).
                # n2q via Act Square+accum straight from PSUM; hvF copies on DVE.
                hvF = sb.tile([QL, CD], F32, tag="hvF")
                n2q = sb2.tile([QL, C], F32, tag="n2q")
                for c in range(C):
                    fps = ps2.tile([QL, 256], F32, tag=("hvA" if c % 2 == 0 else "mdv"))
                    nc.tensor.matmul(fps[:], dspF[:, c, :], hat_m_r[:, D * c:D * c + 256],
                                     start=True, stop=True)
                    sqf = sb2.tile([QL, D], F32, tag="sqf")
                    nc.scalar.activation(sqf[:], fps[:, 0:D], ACT.Square,
                                         accum_out=n2q[:, c:c + 1])
                    nc.vector.tensor_copy(hvF[:, D * c:D * (c + 1)], fps[:, 0:D])
                if dbg:
                    nc.sync.dma_start(dbg_d["hvFd"][:], hvF[:])
                    nc.sync.dma_start(dbg_d["n2qd"][:], n2q[:])
                # fs = squash scale [QL, C] (positive)
                fp1 = sb2.tile([QL, C], F32, tag="fp1")
                nc.vector.tensor_scalar(fp1[:], n2q[:], 1.0, None, op0=ADD)
                fr1 = sb2.tile([QL, C], F32, tag="fr1")
                nc.vector.reciprocal(fr1[:], fp1[:])
                fln = sb2.tile([QL, C], F32, tag="fln")
                nc.scalar.activation(fln[:], n2q[:], ACT.Ln, bias=epsb[0:QL, :])
                fr2 = sb2.tile([QL, C], F32, tag="fr2")
                nc.scalar.activation(fr2[:], fln[:], ACT.Exp, scale=-0.5)
                omr = sb2.tile([QL, C], F32, tag="omr")
                nc.vector.tensor_scalar(omr[:], fr1[:], -1.0, 1.0, op0=MUL, op1=ADD)
                fs = sb2.tile([QL, C], F32, tag="fs")
                nc.vector.tensor_tensor(fs[:], omr[:], fr2[:], op=MUL)
                outT = sb.tile([QL, CD], F32, tag="outT")
                for c in range(C):
                    if c % 2 == 0:
                        nc.vector.tensor_scalar(outT[:, D * c:D * (c + 1)],
                                                hvF[:, D * c:D * (c + 1)],
                                                fs[:, c:c + 1], None, op0=MUL)
                    else:
                        nc.scalar.activation(outT[:, D * c:D * (c + 1)],
                                             hvF[:, D * c:D * (c + 1)], ACT.Copy,
                                             scale=fs[:, c:c + 1])
                nc.sync.dma_start(out_d[:], outT[:])

    # All activations use only {Ln, Exp, Copy, Square}, which live together in
    # act func set 6 (natural_log_exp_and_others). The default solver alternates
    # sets, inserting table reloads (~1.3us each); one load suffices.
    def _single_act_table_load():
        inst = mybir.InstLoadActFuncSet(
            name=nc.get_next_instruction_name(), ins=[], outs=[],
            act_func_set_id=6,
        )
        inst.engine = mybir.EngineType.Activation
        nc.register_instruction(inst)
        for blk in nc.main_func.blocks:
            for idx, bi in enumerate(blk.instructions):
                if isinstance(bi, mybir.InstActivation):
                    blk.instructions.insert(idx, inst)
                    return
        raise AssertionError("no activation found")

    nc.insert_act_table_loads = _single_act_table_load
    nc.compile()
    return nc


_CACHE = {}
LAST_EXEC_NS = None
LAST_RESULTS = None


def kernel(m, q, W, b):
    m = np.asarray(m, dtype=np.float32)
    q = np.asarray(q, dtype=np.float32)
    W = np.asarray(W, dtype=np.float32)
    b = np.asarray(b, dtype=np.float32)
    assert m.shape == (I, K) and q.shape == (NCORES * QL, K) and W.shape == (K, CD)

    with_bias = bool(np.any(b))
    dbg = bool(int(os.environ.get("KERNEL_DBG", "0")))
    key = ("v2", with_bias, str(DT), dbg)
    if key not in _CACHE:
        _CACHE[key] = build(with_bias, dbg)
    nc = _CACHE[key]

    Wp = np.zeros((K, NPAD), dtype=np.float32)
    Wp[:, :CD] = W
    mT = np.ascontiguousarray(m.T)

    in_maps = []
    for i in range(NCORES):
        qT = np.ascontiguousarray(q[QL * i:QL * (i + 1)].T)
        im = {"mT": mT, "qT": qT, "Wp": Wp}
        if with_bias:
            im["b"] = b.reshape(1, CD)
        in_maps.append(im)

    res = run_bass_kernel_spmd(nc, in_maps, list(range(NCORES)))
    global LAST_EXEC_NS, LAST_RESULTS
    LAST_EXEC_NS = res.exec_time_ns
    LAST_RESULTS = res.results
    out = np.concatenate([res.results[i]["out"] for i in range(NCORES)], axis=0)
    return out.astype(np.float32)


if __name__ == "__main__":
    rng = np.random.default_rng(0)
    m = rng.standard_normal((I, K)).astype(np.float32)
    q = rng.standard_normal((NCORES * QL, K)).astype(np.float32)
    W = (rng.standard_normal((K, CD)) * 0.02).astype(np.float32)
    b = np.zeros((CD,), dtype=np.float32)
    out = kernel(m=m, q=q, W=W, b=b)
    print("out", out.shape, out.dtype, np.abs(out).mean())


# revision 29
# speedup vs baseline: 1.4968x; 1.0026x over previous
"""DMR induction routing kernel for Trainium2 (Bass/Tile), 8-core data-parallel.

Problem: nn_DMRInduction. Full inputs:
  m [128, 768], q [256, 768], W [768, 765], b [765] -> out [256, 765] fp32.

Sharding: Q=256 split 8 ways (32 queries/core); m, W, b replicated.

v2 layout/dataflow (per core):
  - hat_m_r  [I=128, 1024] (I on partitions; cols 0..764 = m @ W, cols 765+ zero
      so the final per-class matmuls can stream N=256 on the fp32r fast path)
  - mTc1 [128, C, 128] / mTc2 [34, C, 128]: per-class transposes of hat_m
      (d on partitions). mTc2 row 32 = -mean_d(hat_m) per (c,i), computed by a
      ones-matmul over mTc + scaled copy, so the pearson numerator matmul is
      centered for free (row 32 of tqB carries colsum(tq)).
  - tq [d, (c,q)]: computed DIRECTLY transposed from qT/W with 60 small
      matmuls into PSUM (no PE eye-transposes for the q side at all).
  - routing state a, p, dsp: [I=128, C*Q=160].
  - squash/pearson scalars on [1, 160] rows; broadcasts via 1-row matmuls.
  - iteration v is consumed straight from PSUM (svA = hv_psum * 0.5s); the
    m-dot-v matmul runs on the scaled v, so a += p*(2*mdv') needs one fused op.
"""
import os
import sys

for _p in ("/opt/trn_rl_repo", "/root/.axon_site/_ro/trn_rl_repo"):
    if os.path.isdir(_p) and _p not in sys.path:
        sys.path.insert(0, _p)

import numpy as np
import concourse.bass as bass
import concourse.bacc as bacc
import concourse.mybir as mybir
import concourse.tile as tile
from concourse.bass_utils import run_bass_kernel_spmd

F32 = mybir.dt.float32
# float32r uses the fast PE path (1 cyc/row at N>=256 vs 4) at ~2.5e-4
# scale-relative output error (tolerance is 2e-2). KERNEL_MM_DT=float32
# restores exact matmuls.
DT = getattr(mybir.dt, os.environ.get("KERNEL_MM_DT", "float32r"))

NCORES = 8
I = 128         # memory capsules
C = 5           # capsule classes
D = 153         # dim per capsule
CD = C * D      # 765
K = 768         # input dim
KC = K // 128   # 6 contraction chunks
QL = 32         # queries per core
CQ = C * QL     # 160
NPAD = 768      # W padded to 768 cols so fp32r matmuls stream N>=256
HM_W = 1024     # hat_m_r padded width (final matmuls read 256-wide windows)
EPS = 1e-8
AX = mybir.AxisListType.X
MUL = mybir.AluOpType.mult
ADD = mybir.AluOpType.add
SUB = mybir.AluOpType.subtract
ACT = mybir.ActivationFunctionType


def build(with_bias: bool, dbg: bool = False):
    nc = bacc.Bacc("TRN2", target_bir_lowering=False, debug=False)

    mT_d = nc.dram_tensor("mT", [K, I], F32, kind="ExternalInput")
    qT_d = nc.dram_tensor("qT", [K, QL], F32, kind="ExternalInput")
    W_d = nc.dram_tensor("Wp", [K, NPAD], F32, kind="ExternalInput")
    if with_bias:
        b_d = nc.dram_tensor("b", [1, CD], F32, kind="ExternalInput")
    out_d = nc.dram_tensor("out", [QL, CD], F32, kind="ExternalOutput")
    dbg_d = {}
    if dbg:
        for nm, shp in [("hatm", [128, CD]), ("tqA0", [128, CQ]), ("tqB0", [34, CQ]),
                        ("mTc1d", [128, C * 128]), ("mTc2d", [34, C * 128]),
                        ("invxn", [128, C]), ("p1", [128, CQ]), ("a1", [128, CQ]),
                        ("p2", [128, CQ]), ("a2", [128, CQ]), ("p3", [128, CQ]),
                        ("hvFd", [QL, CD]), ("n2qd", [QL, C])]:
            dbg_d[nm] = nc.dram_tensor("dbg_" + nm, shp, F32, kind="ExternalOutput")

    with tile.TileContext(nc) as tc:
        with (
            tc.tile_pool(name="sb", bufs=1) as sb,
            tc.tile_pool(name="sb2", bufs=3) as sb2,
        ):
            # ---------------- input DMAs (order = HWDGE serial order) -------
            mT_sb = sb.tile([128, KC, I], DT, tag="mT")
            qT_sb = sb.tile([128, KC, QL], DT, tag="qT")
            W_sb = sb.tile([128, KC, NPAD], DT, tag="W")
            nc.sync.dma_start(mT_sb[:], mT_d[:].rearrange("(k p) n -> p k n", p=128).bitcast(DT))
            nc.sync.dma_start(qT_sb[:], qT_d[:].rearrange("(k p) n -> p k n", p=128).bitcast(DT))
            Wr = W_d[:].rearrange("(k p) n -> p k n", p=128).bitcast(DT)
            for k in range(KC):
                nc.sync.dma_start(W_sb[:, k, 0:512], Wr[:, k, 0:512])
            for k in range(KC):
                nc.sync.dma_start(W_sb[:, k, 512:768], Wr[:, k, 512:768])
            if with_bias:
                b_sb = sb.tile([1, CD], F32, tag="b")
                nc.sync.dma_start(b_sb[:], b_d[:])

            # ---------------- constants (no DMA) ----------------------------
            # float32r tiles cannot be memset directly; memset F32 staging and
            # copy through Act/DVE (engine writes perform the f32r rounding).
            zf = sb.tile([128, 640], F32, tag="zf")
            nc.vector.memset(zf[:], 0.0)
            of = sb.tile([128, 1], F32, tag="of")
            nc.vector.memset(of[:], 1.0)
            o1f = sb.tile([1, 128], F32, tag="o1f")
            nc.vector.memset(o1f[:], 1.0)
            nhf = sb.tile([1, 128], F32, tag="nhf")
            nc.vector.memset(nhf[:], -0.5)
            epsb = sb.tile([128, 1], F32, tag="epsb")
            nc.vector.memset(epsb[:], EPS)
            # identity for PE transposes, built on-chip (a DMA would arrive
            # ~12us in, after the W pieces, and gate the whole transpose block)
            eye = sb.tile([128, 128], F32, tag="eye")
            nc.vector.memset(eye[:], 1.0)
            nc.gpsimd.affine_select(eye[:], eye[:], pattern=[[-1, 128]],
                                    compare_op=mybir.AluOpType.is_equal,
                                    fill=0.0, base=0, channel_multiplier=1)

            ones1 = sb.tile([1, 128], DT, tag="ones1")
            nc.scalar.copy(ones1[:], o1f[:])
            twos1 = sb.tile([1, 128], DT, tag="twos1")
            nc.scalar.activation(twos1[:], o1f[:], ACT.Copy, scale=2.0)
            nhalf1 = sb.tile([1, 128], DT, tag="nhalf1")
            nc.vector.tensor_copy(nhalf1[:], nhf[:])
            onesF = sb.tile([128, 1], DT, tag="onesF")
            nc.scalar.copy(onesF[:], of[:])
            if with_bias:
                onesq = sb.tile([1, QL], DT, tag="onesq")
                nc.vector.tensor_copy(onesq[:], o1f[:, 0:QL])

            # persistent tiles that need zero rows
            hat_m_r = sb.tile([128, HM_W], DT, tag="hatmr")
            nc.vector.tensor_copy(hat_m_r[:, CD:HM_W], zf[:, 0:HM_W - CD])
            mTc1 = sb.tile([128, C, 128], DT, tag="mTc1")
            mTc2 = sb.tile([34, C, 128], DT, tag="mTc2")
            nc.scalar.copy(mTc2[:].rearrange("p c n -> p (c n)"), zf[0:34, 0:640])
            tqA = sb.tile([128, C, QL], DT, tag="tqA")
            tqB = sb.tile([34, C, QL], DT, tag="tqB")
            nc.vector.tensor_copy(tqB[:].rearrange("p c q -> p (c q)"), zf[0:34, 0:CQ])
            svA = sb.tile([128, C, QL], DT, tag="svA")
            svB = sb.tile([34, C, QL], DT, tag="svB")
            nc.scalar.copy(svB[:].rearrange("p c q -> p (c q)"), zf[0:34, 0:CQ])

            tqAf = tqA[:].bitcast(F32).rearrange("p c q -> p (c q)")
            tqBf25 = tqB[0:25].bitcast(F32).rearrange("p c q -> p (c q)")

            # ---------------- projections ----------------------------------
            with tc.tile_pool(name="ps1", bufs=1, space="PSUM") as ps1, \
                 tc.tile_pool(name="pstp", bufs=4, space="PSUM") as pstp:
                psA = ps1.tile([128, 512], F32, tag="psA")
                psB = ps1.tile([128, 256], F32, tag="psB")
                psQA = ps1.tile([128, C, QL], F32, tag="psQA")
                psQB = ps1.tile([34, C, QL], F32, tag="psQB")

                # hat_q (DIRECTLY transposed: out[d,(c,q)] = sum_k W[k,cD+d] q[q,k])
                # shares one bank across classes; start=True clears the whole
                # bank's has_written bits, so each (c, piece) group runs its
                # start..stop back-to-back. k is split in halves combined via
                # SBUF; classes are split c0-2 / c3-4 because c0-2 only read the
                # A columns (<512) of W, which stream in first.
                def q_cs(h, cs):
                    ks = range(3 * h, 3 * h + 3)
                    add_bias = with_bias and h == 1
                    for c in cs:
                        for j, k in enumerate(ks):
                            nc.tensor.matmul(psQA[:, c, :], W_sb[:, k, D * c:D * c + 128],
                                             qT_sb[:, k, :], start=(j == 0),
                                             stop=(j == 2 and not add_bias))
                        if add_bias:
                            nc.tensor.matmul(psQA[:, c, :], b_sb[:, D * c:D * c + 128],
                                             onesq[:], start=False, stop=True)
                        for j, k in enumerate(ks):
                            nc.tensor.matmul(psQB[0:25, c, :], W_sb[:, k, D * c + 128:D * (c + 1)],
                                             qT_sb[:, k, :], start=(j == 0),
                                             stop=(j == 2 and not add_bias))
                        if add_bias:
                            nc.tensor.matmul(psQB[0:25, c, :], b_sb[:, D * c + 128:D * (c + 1)],
                                             onesq[:], start=False, stop=True)

                def q_copy(cs):
                    c0, c1 = cs[0], cs[-1] + 1
                    nc.vector.tensor_copy(tqA[:, c0:c1, :].rearrange("p c q -> p (c q)"),
                                          psQA[:, c0:c1, :].rearrange("p c q -> p (c q)"))
                    nc.vector.tensor_copy(tqB[0:25, c0:c1, :].rearrange("p c q -> p (c q)"),
                                          psQB[0:25, c0:c1, :].rearrange("p c q -> p (c q)"))

                def q_add(cs):
                    c0, c1 = cs[0], cs[-1] + 1
                    nc.vector.tensor_tensor(
                        tqA[:, c0:c1, :].rearrange("p c q -> p (c q)"),
                        tqA[:, c0:c1, :].bitcast(F32).rearrange("p c q -> p (c q)"),
                        psQA[:, c0:c1, :].rearrange("p c q -> p (c q)"), op=ADD)
                    nc.vector.tensor_tensor(
                        tqB[0:25, c0:c1, :].rearrange("p c q -> p (c q)"),
                        tqB[0:25, c0:c1, :].bitcast(F32).rearrange("p c q -> p (c q)"),
                        psQB[0:25, c0:c1, :].rearrange("p c q -> p (c q)"), op=ADD)

                def tposes(cs):
                    for c in cs:
                        t1 = pstp.tile([128, 128], F32, tag="tp")
                        nc.tensor.transpose(t1[:], hat_m_r[:, D * c:D * c + 128].bitcast(F32), eye[:])
                        (nc.vector.tensor_copy if c % 2 else nc.scalar.copy)(mTc1[:, c, :], t1[:])
                        t2 = pstp.tile([25, 128], F32, tag="tp")
                        nc.tensor.transpose(t2[:], hat_m_r[:, D * c + 128:D * (c + 1)].bitcast(F32),
                                            eye[:])
                        (nc.scalar.copy if c % 2 else nc.vector.tensor_copy)(mTc2[0:25, c, :], t2[:])

                # ---- A-column half: hat_m cols 0:512, q classes 0-2 ----------
                for k in range(KC):
                    nc.tensor.matmul(psA[:], mT_sb[:, k, :], W_sb[:, k, 0:512],
                                     start=(k == 0), stop=(k == KC - 1 and not with_bias))
                if with_bias:
                    nc.tensor.matmul(psA[:], ones1[:], b_sb[:, 0:512], start=False, stop=True)
                q_cs(0, [0, 1, 2])
                q_copy([0, 1, 2])
                q_cs(1, [0, 1, 2])
                q_add([0, 1, 2])
                nc.scalar.copy(hat_m_r[:, 0:256], psA[:, 0:256])
                nc.vector.tensor_copy(hat_m_r[:, 256:512], psA[:, 256:512])
                tposes([0, 1, 2])

                # ---- B-column half: hat_m cols 512:765, q classes 3-4 --------
                for k in range(KC):
                    nc.tensor.matmul(psB[:], mT_sb[:, k, :], W_sb[:, k, 512:768],
                                     start=(k == 0), stop=(k == KC - 1 and not with_bias))
                if with_bias:
                    nc.tensor.matmul(psB[:, 0:253], ones1[:], b_sb[:, 512:765],
                                     start=False, stop=True)
                q_cs(0, [3, 4])
                q_copy([3, 4])
                q_cs(1, [3, 4])
                q_add([3, 4])
                nc.scalar.copy(hat_m_r[:, 512:640], psB[:, 0:128])
                nc.vector.tensor_copy(hat_m_r[:, 640:765], psB[:, 128:253])

                hm32 = hat_m_r[:, 0:765].bitcast(F32)

                # ------------- pearson #1, tq-only part (setup banks) --------
                # emitted BEFORE the c3/c4 transposes + psMu so its matmuls can
                # run during the hat_m B-column tail (engines bypass at most 4
                # blocked instructions, so queue order matters)
                sqA1 = sb2.tile([128, CQ], DT, tag="sqA")
                nc.gpsimd.tensor_tensor(sqA1[:], tqAf, tqAf, op=MUL)
                sqB1 = sb2.tile([25, CQ], DT, tag="sqB")
                nc.vector.tensor_tensor(sqB1[:], tqBf25, tqBf25, op=MUL)
                colsum1 = ps1.tile([1, CQ], F32, tag="psQA")
                nc.tensor.matmul(colsum1[:], onesF[:],
                                 tqA[:].rearrange("p c q -> p (c q)"),
                                 start=True, stop=False)
                nc.tensor.matmul(colsum1[:], onesF[0:25],
                                 tqB[0:25].rearrange("p c q -> p (c q)"),
                                 start=False, stop=True)
                tqB32w = tqB[32:33, :, :].rearrange("p c q -> p (c q)")
                nc.scalar.copy(tqB32w[:], colsum1[:])
                yn2r1 = ps1.tile([1, CQ], F32, tag="psQB")
                nc.tensor.matmul(yn2r1[:], onesF[:], sqA1[:], start=True, stop=False)
                nc.tensor.matmul(yn2r1[:], onesF[0:25], sqB1[:], start=False, stop=True)
                T2 = sb.tile([1, CQ], F32, tag="T2")
                nc.vector.tensor_copy(T2[:], yn2r1[:])
                tqB32f = tqB[32:33, :, :].bitcast(F32).rearrange("p c q -> p (c q)")
                csqv1 = sb2.tile([1, CQ], F32, tag="csqv")
                nc.vector.tensor_tensor(csqv1[:], tqB32f, tqB32f, op=MUL)
                yn21 = sb2.tile([1, CQ], F32, tag="yn2")
                nc.vector.scalar_tensor_tensor(yn21[:], csqv1[:], -1.0 / D, yn2r1[:],
                                               op0=MUL, op1=ADD)
                lyn1 = sb2.tile([1, CQ], F32, tag="lyn")
                nc.scalar.activation(lyn1[:], yn21[:], ACT.Ln)
                inv_yn1 = sb2.tile([1, CQ], DT, tag="invyn")
                nc.scalar.activation(inv_yn1[:], lyn1[:], ACT.Exp, scale=-0.5)

                tposes([3, 4])

                # mTc2 row 32 = -mean_d(hat_m)[c,i] via ones-matmul over mTc
                # (split 512+128: PSUM banks hold 512 fp32/partition max;
                #  reuses the psA/psB banks, free after the hat_m copies)
                psMuA = ps1.tile([1, 512], F32, tag="psA")
                psMuB = ps1.tile([1, 128], F32, tag="psB")
                mTc1f = mTc1[:].rearrange("p c n -> p (c n)")
                mTc2f = mTc2[0:25].rearrange("p c n -> p (c n)")
                nc.tensor.matmul(psMuA[:], onesF[:], mTc1f[:, 0:512],
                                 start=True, stop=False)
                nc.tensor.matmul(psMuA[:], onesF[0:25], mTc2f[:, 0:512],
                                 start=False, stop=True)
                nc.tensor.matmul(psMuB[:], onesF[:], mTc1f[:, 512:640],
                                 start=True, stop=False)
                nc.tensor.matmul(psMuB[:], onesF[0:25], mTc2f[:, 512:640],
                                 start=False, stop=True)
                mTc2r32 = mTc2[32:33, :, :].rearrange("p c n -> p (c n)")
                nc.scalar.activation(mTc2r32[:, 0:256], psMuA[:, 0:256], ACT.Copy, scale=-1.0 / D)
                nc.vector.tensor_scalar(mTc2r32[:, 256:512], psMuA[:, 256:512], -1.0 / D,
                                        None, op0=MUL)
                nc.scalar.activation(mTc2r32[:, 512:640], psMuB[:], ACT.Copy, scale=-1.0 / D)

                # ------------- m stats part 1: sum_d hat_m^2 ----------------
                xn2r = sb.tile([128, C], F32, tag="xn2r")
                sqs = sb.tile([128, D], F32, tag="sqs")
                for c in range(C):
                    nc.vector.scalar_tensor_tensor(
                        sqs[:], hm32[:, D * c:D * (c + 1)], 1.0,
                        hm32[:, D * c:D * (c + 1)], op0=MUL, op1=MUL,
                        accum_out=xn2r[:, c:c + 1])


            if dbg:
                nc.sync.dma_start(dbg_d["hatm"][:], hm32)
                nc.sync.dma_start(dbg_d["tqA0"][:], tqAf)
                nc.sync.dma_start(dbg_d["tqB0"][:], tqB[:].bitcast(F32).rearrange("p c q -> p (c q)"))
                nc.sync.dma_start(dbg_d["mTc1d"][:], mTc1[:].bitcast(F32).rearrange("p c n -> p (c n)"))
                nc.sync.dma_start(dbg_d["mTc2d"][:], mTc2[:].bitcast(F32).rearrange("p c n -> p (c n)"))

            inv_xn = sb.tile([128, C], F32, tag="invxn")
            ixb = inv_xn[:].rearrange("p (c a) -> p c a", a=1).broadcast_to((128, C, QL))

            # ---------------- routing --------------------------------------
            with tc.tile_pool(name="ps2", bufs=1, space="PSUM") as ps2:

                def pearson_late(inv_yn, lhs1, extra=None):
                    iyb = ps2.tile([128, CQ], F32, tag="bcast")
                    nc.tensor.matmul(iyb[:], lhs1, inv_yn[:], start=True, stop=True)
                    if extra is not None:
                        extra()  # setup-only work that must precede the num matmuls
                    # num[i,(c,q)]: per-class A/B pairs back-to-back (groups share
                    # one bank; start=True clears the bank's has_written bits)
                    num = ps2.tile([128, C, QL], F32, tag="num")
                    for c in range(C):
                        nc.tensor.matmul(num[:, c, :], mTc1[:, c, :], tqA[:, c, :],
                                         start=True, stop=False)
                        nc.tensor.matmul(num[:, c, :], mTc2[:, c, :], tqB[:, c, :],
                                         start=False, stop=True)
                    # p = tanh(num * inv_xn * inv_yn); tanh(x) = 1 - 2/(1+exp(2x))
                    pp1 = sb2.tile([128, C, QL], F32, tag="pp1")
                    nc.vector.tensor_tensor(pp1[:], num[:], ixb, op=MUL)
                    pp = sb2.tile([128, CQ], F32, tag="pp")
                    nc.vector.tensor_tensor(pp[:], pp1[:].rearrange("p c q -> p (c q)"),
                                            iyb[:], op=MUL)
                    e2 = sb2.tile([128, CQ], F32, tag="e2")
                    nc.scalar.activation(e2[:], pp[:], ACT.Exp, scale=2.0)
                    den = sb2.tile([128, CQ], F32, tag="den")
                    nc.vector.tensor_scalar(den[:], e2[:], 1.0, None, op0=ADD)
                    rr = sb2.tile([128, CQ], F32, tag="rr")
                    nc.vector.reciprocal(rr[:], den[:])
                    return rr

                def pearson(extra=None):
                    """p = tanh(centered-corr(mT, tq)); returns rr tile [128, CQ]."""
                    # squares (row 32 of tqB excluded; centering via csq)
                    sqA = sb2.tile([128, CQ], DT, tag="sqA")
                    nc.gpsimd.tensor_tensor(sqA[:], tqAf, tqAf, op=MUL)
                    sqB = sb2.tile([25, CQ], DT, tag="sqB")
                    nc.vector.tensor_tensor(sqB[:], tqBf25, tqBf25, op=MUL)
                    # colsum(tq) -> tqB row 32 (feeds the centered num matmul)
                    colsum = ps2.tile([1, CQ], F32, tag="colsum")
                    nc.tensor.matmul(colsum[:], onesF[:],
                                     tqA[:].rearrange("p c q -> p (c q)"),
                                     start=True, stop=False)
                    nc.tensor.matmul(colsum[:], onesF[0:25],
                                     tqB[0:25].rearrange("p c q -> p (c q)"),
                                     start=False, stop=True)
                    nc.scalar.copy(tqB[32:33, :, :].rearrange("p c q -> p (c q)"), colsum[:])
                    # yn2 = sum(tq^2) - colsum^2/D, centered variance of tq
                    yn2r = ps2.tile([1, CQ], F32, tag="cs2")
                    nc.tensor.matmul(yn2r[:], onesF[:], sqA[:], start=True, stop=False)
                    nc.tensor.matmul(yn2r[:], onesF[0:25], sqB[:], start=False, stop=True)
                    csqv = sb2.tile([1, CQ], F32, tag="csqv")
                    nc.vector.tensor_tensor(csqv[:], tqB32f, tqB32f, op=MUL)
                    yn2 = sb2.tile([1, CQ], F32, tag="yn2")
                    nc.vector.scalar_tensor_tensor(yn2[:], csqv[:], -1.0 / D, yn2r[:],
                                                   op0=MUL, op1=ADD)
                    lyn = sb2.tile([1, CQ], F32, tag="lyn")
                    nc.scalar.activation(lyn[:], yn2[:], ACT.Ln)
                    inv_yn = sb2.tile([1, CQ], DT, tag="invyn")
                    nc.scalar.activation(inv_yn[:], lyn[:], ACT.Exp, scale=-0.5)
                    return pearson_late(inv_yn, ones1[:])

                def stats_tail():
                    # m stats part 2: -mean[i,c] from mTc2 row 32 via 5 tiny
                    # transposes; then inv_xn = 1/sqrt(sum hm^2 - D*mean^2).
                    t_nm = ps2.tile([128, C], F32, tag="nm")
                    for c in range(C):
                        nc.tensor.transpose(t_nm[:, c:c + 1],
                                            mTc2[32:33, c, :].bitcast(F32), eye[32:33, 32:33])
                    nmean = sb.tile([128, C], F32, tag="nmean")
                    nc.vector.tensor_copy(nmean[:], t_nm[:])
                    nm2 = sb.tile([128, C], F32, tag="nm2")
                    nc.vector.tensor_tensor(nm2[:], nmean[:], nmean[:], op=MUL)
                    xn2 = sb.tile([128, C], F32, tag="xn2")
                    nc.vector.scalar_tensor_tensor(xn2[:], nm2[:], -float(D), xn2r[:],
                                                   op0=MUL, op1=ADD)
                    lxn = sb.tile([128, C], F32, tag="lxn")
                    nc.scalar.activation(lxn[:], xn2[:], ACT.Ln)
                    nc.scalar.activation(inv_xn[:], lxn[:], ACT.Exp, scale=-0.5)

                def p_from_rr(rr):
                    p_new = sb2.tile([128, CQ], F32, tag="p")
                    nc.vector.tensor_scalar(p_new[:], rr[:], -2.0, 1.0, op0=MUL, op1=ADD)
                    return p_new

                rr = pearson_late(inv_yn1, ones1[:], extra=stats_tail)
                a_t = None
                p_t = None

                for it in range(2):
                    dsp = sb2.tile([128, C, QL], DT, tag="dsp")
                    if it == 0:
                        # softmax(0) = 1/C exactly; dsp = p + 1/C straight from rr
                        nc.vector.tensor_scalar(dsp[:].rearrange("p c q -> p (c q)"),
                                                rr[:], -2.0, 1.0 + 1.0 / C, op0=MUL, op1=ADD)
                        p_t = p_from_rr(rr)
                        if dbg:
                            nc.sync.dma_start(dbg_d["p1"][:], p_t[:])
                            nc.sync.dma_start(dbg_d["invxn"][:], inv_xn[:])
                    else:
                        p_t = p_from_rr(rr)
                        if dbg:
                            nc.sync.dma_start(dbg_d["p2"][:], p_t[:])
                        ea = sb2.tile([128, CQ], F32, tag="ea")
                        nc.scalar.activation(ea[:], a_t[:], ACT.Exp)
                        asum = sb2.tile([128, QL], F32, tag="asum")
                        nc.vector.tensor_reduce(asum[:], ea[:].rearrange("p (c q) -> p q c", c=C),
                                                axis=AX, op=ADD)
                        rs = sb2.tile([128, QL], F32, tag="rs")
                        nc.vector.reciprocal(rs[:], asum[:])
                        dd = sb2.tile([128, C, QL], F32, tag="dd")
                        nc.vector.tensor_tensor(
                            dd[:], ea[:].rearrange("p (c q) -> p c q", c=C),
                            rs[:].rearrange("p (a q) -> p a q", a=1).broadcast_to((128, C, QL)),
                            op=MUL)
                        dd1 = sb2.tile([128, CQ], F32, tag="dd1")
                        nc.vector.tensor_scalar(dd1[:], dd[:].rearrange("p c q -> p (c q)"),
                                                1.0, None, op0=ADD)
                        nc.vector.scalar_tensor_tensor(dsp[:].rearrange("p c q -> p (c q)"),
                                                       rr[:], -2.0, dd1[:], op0=MUL, op1=ADD)

                    # hv[d,(c,q)] in PSUM (consumed in place; never copied to SBUF)
                    hvA = ps2.tile([128, C, QL], F32, tag="hvA")
                    hvB = ps2.tile([26, C, QL], F32, tag="hvB")
                    for c in range(C):
                        nc.tensor.matmul(hvA[:, c, :], hat_m_r[:, D * c:D * c + 128], dsp[:, c, :],
                                         start=True, stop=True)
                        nc.tensor.matmul(hvB[:, c, :], hat_m_r[:, D * c + 128:D * c + 154], dsp[:, c, :],
                                         start=True, stop=True)
                    hvAf = hvA[:].rearrange("p c q -> p (c q)")
                    hvBf25 = hvB[0:25].rearrange("p c q -> p (c q)")
                    # n2 = sum_d hv^2 (raw); squash scale applied later
                    sqhA = sb2.tile([128, CQ], DT, tag="sqhA")
                    nc.scalar.activation(sqhA[:], hvAf, ACT.Square)
                    sqhB = sb2.tile([25, CQ], DT, tag="sqhB")
                    nc.scalar.activation(sqhB[:], hvBf25, ACT.Square)
                    # hv staged to SBUF so sv ops can read the sBh broadcast
                    # straight from PSUM (one-PSUM-input rule); 0.5*tqB is
                    # pre-scaled on Pool while DVE runs the squash chain
                    vA = sb2.tile([128, CQ], F32, tag="vA")
                    nc.scalar.copy(vA[:], hvAf)
                    vB = sb2.tile([25, CQ], F32, tag="vB")
                    nc.vector.tensor_copy(vB[:], hvBf25)
                    tqhB = sb2.tile([25, CQ], F32, tag="tqhB")
                    nc.gpsimd.tensor_scalar(tqhB[:], tqBf25, 0.5, None, op0=MUL)
                    n2 = ps2.tile([1, CQ], F32, tag="cs2")
                    nc.tensor.matmul(n2[:], onesF[:], sqhA[:], start=True, stop=False)
                    nc.tensor.matmul(n2[:], onesF[0:25], sqhB[:], start=False, stop=True)
                    # -s = (1/(1+n2) - 1) / sqrt(n2+eps)
                    n2p1 = sb2.tile([1, CQ], F32, tag="n2p1")
                    nc.vector.tensor_scalar(n2p1[:], n2[:], 1.0, None, op0=ADD)
                    r1 = sb2.tile([1, CQ], F32, tag="r1")
                    nc.vector.reciprocal(r1[:], n2p1[:])
                    ln2 = sb2.tile([1, CQ], F32, tag="ln2")
                    nc.scalar.activation(ln2[:], n2[:], ACT.Ln, bias=epsb[0:1, :])
                    r2 = sb2.tile([1, CQ], F32, tag="r2")
                    nc.scalar.activation(r2[:], ln2[:], ACT.Exp, scale=-0.5)
                    nsrow = sb2.tile([1, CQ], DT, tag="nsrow")
                    nc.vector.scalar_tensor_tensor(nsrow[:], r1[:], 1.0, r2[:],
                                                   op0=SUB, op1=MUL)
                    # 0.5*s broadcast to all partitions: (-0.5) x (-s)
                    sBh = ps2.tile([128, CQ], F32, tag="num")
                    nc.tensor.matmul(sBh[:], nhalf1[:], nsrow[:], start=True, stop=True)
                    # sv = (0.5*s)*hv; tq = 0.5*tq + sv
                    nc.vector.tensor_tensor(svA[:].rearrange("p c q -> p (c q)"),
                                            vA[:], sBh[:], op=MUL)
                    nc.vector.tensor_tensor(svB[0:25].rearrange("p c q -> p (c q)"),
                                            vB[:], sBh[0:25, :], op=MUL)
                    nc.vector.scalar_tensor_tensor(tqA[:].rearrange("p c q -> p (c q)"),
                                                   tqAf, 0.5,
                                                   svA[:].rearrange("p c q -> p (c q)").bitcast(F32),
                                                   op0=MUL, op1=ADD)
                    nc.gpsimd.tensor_tensor(tqB[0:25].rearrange("p c q -> p (c q)"),
                                            tqhB[:],
                                            svB[0:25].rearrange("p c q -> p (c q)").bitcast(F32),
                                            op=ADD)
                    # mdv' = mT . (0.5*s*v)  (rows 25..33 of svB stay 0 -> uncentered)
                    mdv = ps2.tile([128, C, QL], F32, tag="mdv")
                    for c in range(C):
                        nc.tensor.matmul(mdv[:, c, :], mTc1[:, c, :], svA[:, c, :],
                                         start=True, stop=False)
                        nc.tensor.matmul(mdv[:, c, :], mTc2[:, c, :], svB[:, c, :],
                                         start=False, stop=True)
                    # a += p * s * mdv_raw = p * (2*mdv')
                    if it == 0:
                        a_t = sb2.tile([128, CQ], F32, tag="a")
                        nc.vector.scalar_tensor_tensor(a_t[:], mdv[:].rearrange("p c q -> p (c q)"),
                                                       2.0, p_t[:], op0=MUL, op1=MUL)
                    else:
                        pm2 = sb2.tile([128, CQ], F32, tag="pm2")
                        nc.vector.scalar_tensor_tensor(pm2[:], mdv[:].rearrange("p c q -> p (c q)"),
                                                       2.0, p_t[:], op0=MUL, op1=MUL)
                        a_new = sb2.tile([128, CQ], F32, tag="a")
                        nc.vector.tensor_tensor(a_new[:], a_t[:], pm2[:], op=ADD)
                        a_t = a_new
                    if dbg:
                        nc.sync.dma_start(dbg_d["a1" if it == 0 else "a2"][:], a_t[:])

                    rr = pearson()

                # ---------------- final ------------------------------------
                p_t = p_from_rr(rr)
                if dbg:
                    nc.sync.dma_start(dbg_d["p3"][:], p_t[:])
                ea = sb2.tile([128, CQ], F32, tag="ea")
                nc.scalar.activation(ea[:], a_t[:], ACT.Exp)
                asum = sb2.tile([128, QL], F32, tag="asum")
                nc.vector.tensor_reduce(asum[:], ea[:].rearrange("p (c q) -> p q c", c=C),
                                        axis=AX, op=ADD)
                rs = sb2.tile([128, QL], F32, tag="rs")
                nc.vector.reciprocal(rs[:], asum[:])
                dd = sb2.tile([128, C, QL], F32, tag="dd")
                nc.vector.tensor_tensor(
                    dd[:], ea[:].rearrange("p (c q) -> p c q", c=C),
                    rs[:].rearrange("p (a q) -> p a q", a=1).broadcast_to((128, C, QL)), op=MUL)
                dd1 = sb2.tile([128, CQ], F32, tag="dd1")
                nc.vector.tensor_scalar(dd1[:], dd[:].rearrange("p c q -> p (c q)"),
                                        1.0, None, op0=ADD)
                dspF = sb2.tile([128, C, QL], DT, tag="dsp")
                nc.vector.scalar_tensor_tensor(dspF[:].rearrange("p c q -> p (c q)"),
                                               rr[:], -2.0, dd1[:], op0=MUL, op1=ADD)

                # final hv: per-class matmul, N=256 window (cols 765+ are zero).
                # n2q via Act Square+accum straight from PSUM; hvF copies on DVE.
                hvF = sb.tile([QL, CD], F32, tag="hvF")
                n2q = sb2.tile([QL, C], F32, tag="n2q")
                for c in range(C):
                    fps = ps2.tile([QL, 256], F32, tag=("hvA" if c % 2 == 0 else "mdv"))
                    nc.tensor.matmul(fps[:], dspF[:, c, :], hat_m_r[:, D * c:D * c + 256],
                                     start=True, stop=True)
                    sqf = sb2.tile([QL, D], F32, tag="sqf")
                    nc.scalar.activation(sqf[:], fps[:, 0:D], ACT.Square,
                                         accum_out=n2q[:, c:c + 1])
                    nc.vector.tensor_copy(hvF[:, D * c:D * (c + 1)], fps[:, 0:D])
                if dbg:
                    nc.sync.dma_start(dbg_d["hvFd"][:], hvF[:])
                    nc.sync.dma_start(dbg_d["n2qd"][:], n2q[:])
                # fs = squash scale [QL, C] (positive)
                fp1 = sb2.tile([QL, C], F32, tag="fp1")
                nc.vector.tensor_scalar(fp1[:], n2q[:], 1.0, None, op0=ADD)
                fr1 = sb2.tile([QL, C], F32, tag="fr1")
                nc.vector.reciprocal(fr1[:], fp1[:])
                fln = sb2.tile([QL, C], F32, tag="fln")
                nc.scalar.activation(fln[:], n2q[:], ACT.Ln, bias=epsb[0:QL, :])
                fr2 = sb2.tile([QL, C], F32, tag="fr2")
                nc.scalar.activation(fr2[:], fln[:], ACT.Exp, scale=-0.5)
                omr = sb2.tile([QL, C], F32, tag="omr")
                nc.vector.tensor_scalar(omr[:], fr1[:], -1.0, 1.0, op0=MUL, op1=ADD)
                fs = sb2.tile([QL, C], F32, tag="fs")
                nc.vector.tensor_tensor(fs[:], omr[:], fr2[:], op=MUL)
                outT = sb.tile([QL, CD], F32, tag="outT")
                for c in range(C):
                    if c % 2 == 0:
                        nc.vector.tensor_scalar(outT[:, D * c:D * (c + 1)],
                                                hvF[:, D * c:D * (c + 1)],
                                                fs[:, c:c + 1], None, op0=MUL)
                    else:
                        nc.scalar.activation(outT[:, D * c:D * (c + 1)],
                                             hvF[:, D * c:D * (c + 1)], ACT.Copy,
                                             scale=fs[:, c:c + 1])
                nc.sync.dma_start(out_d[:], outT[:])

    # All activations use only {Ln, Exp, Copy, Square}, which live together in
    # act func set 6 (natural_log_exp_and_others). The default solver alternates
    # sets, inserting table reloads (~1.3us each); one load suffices.
    def _single_act_table_load():
        inst = mybir.InstLoadActFuncSet(
            name=nc.get_next_instruction_name(), ins=[], outs=[],
            act_func_set_id=6,
        )
        inst.engine = mybir.EngineType.Activation
        nc.register_instruction(inst)
        for blk in nc.main_func.blocks:
            for idx, bi in enumerate(blk.instructions):
                if isinstance(bi, mybir.InstActivation):
                    blk.instructions.insert(idx, inst)
                    return
        raise AssertionError("no activation found")

    nc.insert_act_table_loads = _single_act_table_load
    nc.compile()
    return nc


_CACHE = {}
LAST_EXEC_NS = None
LAST_RESULTS = None


def kernel(m, q, W, b):
    m = np.asarray(m, dtype=np.float32)
    q = np.asarray(q, dtype=np.float32)
    W = np.asarray(W, dtype=np.float32)
    b = np.asarray(b, dtype=np.float32)
    assert m.shape == (I, K) and q.shape == (NCORES * QL, K) and W.shape == (K, CD)

    with_bias = bool(np.any(b))
    dbg = bool(int(os.environ.get("KERNEL_DBG", "0")))
    key = ("v2", with_bias, str(DT), dbg)
    if key not in _CACHE:
        _CACHE[key] = build(with_bias, dbg)
    nc = _CACHE[key]

    Wp = np.zeros((K, NPAD), dtype=np.float32)
    Wp[:, :CD] = W
    mT = np.ascontiguousarray(m.T)

    in_maps = []
    for i in range(NCORES):
        qT = np.ascontiguousarray(q[QL * i:QL * (i + 1)].T)
        im = {"mT": mT, "qT": qT, "Wp": Wp}
        if with_bias:
            im["b"] = b.reshape(1, CD)
        in_maps.append(im)

    res = run_bass_kernel_spmd(nc, in_maps, list(range(NCORES)))
    global LAST_EXEC_NS, LAST_RESULTS
    LAST_EXEC_NS = res.exec_time_ns
    LAST_RESULTS = res.results
    out = np.concatenate([res.results[i]["out"] for i in range(NCORES)], axis=0)
    return out.astype(np.float32)


if __name__ == "__main__":
    rng = np.random.default_rng(0)
    m = rng.standard_normal((I, K)).astype(np.float32)
    q = rng.standard_normal((NCORES * QL, K)).astype(np.float32)
    W = (rng.standard_normal((K, CD)) * 0.02).astype(np.float32)
    b = np.zeros((CD,), dtype=np.float32)
    out = kernel(m=m, q=q, W=W, b=b)
    print("out", out.shape, out.dtype, np.abs(out).mean())


# revision 30
# speedup vs baseline: 1.4989x; 1.0014x over previous
"""DMR induction routing kernel for Trainium2 (Bass/Tile), 8-core data-parallel.

Problem: nn_DMRInduction. Full inputs:
  m [128, 768], q [256, 768], W [768, 765], b [765] -> out [256, 765] fp32.

Sharding: Q=256 split 8 ways (32 queries/core); m, W, b replicated.

v2 layout/dataflow (per core):
  - hat_m_r  [I=128, 1024] (I on partitions; cols 0..764 = m @ W, cols 765+ zero
      so the final per-class matmuls can stream N=256 on the fp32r fast path)
  - mTc1 [128, C, 128] / mTc2 [34, C, 128]: per-class transposes of hat_m
      (d on partitions). mTc2 row 32 = -mean_d(hat_m) per (c,i), computed by a
      ones-matmul over mTc + scaled copy, so the pearson numerator matmul is
      centered for free (row 32 of tqB carries colsum(tq)).
  - tq [d, (c,q)]: computed DIRECTLY transposed from qT/W with 60 small
      matmuls into PSUM (no PE eye-transposes for the q side at all).
  - routing state a, p, dsp: [I=128, C*Q=160].
  - squash/pearson scalars on [1, 160] rows; broadcasts via 1-row matmuls.
  - iteration v is consumed straight from PSUM (svA = hv_psum * 0.5s); the
    m-dot-v matmul runs on the scaled v, so a += p*(2*mdv') needs one fused op.
"""
import os
import sys

for _p in ("/opt/trn_rl_repo", "/root/.axon_site/_ro/trn_rl_repo"):
    if os.path.isdir(_p) and _p not in sys.path:
        sys.path.insert(0, _p)

import numpy as np
import concourse.bass as bass
import concourse.bacc as bacc
import concourse.mybir as mybir
import concourse.tile as tile
from concourse.bass_utils import run_bass_kernel_spmd

F32 = mybir.dt.float32
# float32r uses the fast PE path (1 cyc/row at N>=256 vs 4) at ~2.5e-4
# scale-relative output error (tolerance is 2e-2). KERNEL_MM_DT=float32
# restores exact matmuls.
DT = getattr(mybir.dt, os.environ.get("KERNEL_MM_DT", "float32r"))

NCORES = 8
I = 128         # memory capsules
C = 5           # capsule classes
D = 153         # dim per capsule
CD = C * D      # 765
K = 768         # input dim
KC = K // 128   # 6 contraction chunks
QL = 32         # queries per core
CQ = C * QL     # 160
NPAD = 768      # W padded to 768 cols so fp32r matmuls stream N>=256
HM_W = 1024     # hat_m_r padded width (final matmuls read 256-wide windows)
EPS = 1e-8
AX = mybir.AxisListType.X
MUL = mybir.AluOpType.mult
ADD = mybir.AluOpType.add
SUB = mybir.AluOpType.subtract
ACT = mybir.ActivationFunctionType


def build(with_bias: bool, dbg: bool = False):
    nc = bacc.Bacc("TRN2", target_bir_lowering=False, debug=False)

    mT_d = nc.dram_tensor("mT", [K, I], F32, kind="ExternalInput")
    qT_d = nc.dram_tensor("qT", [K, QL], F32, kind="ExternalInput")
    W_d = nc.dram_tensor("Wp", [K, NPAD], F32, kind="ExternalInput")
    if with_bias:
        b_d = nc.dram_tensor("b", [1, CD], F32, kind="ExternalInput")
    out_d = nc.dram_tensor("out", [QL, CD], F32, kind="ExternalOutput")
    dbg_d = {}
    if dbg:
        for nm, shp in [("hatm", [128, CD]), ("tqA0", [128, CQ]), ("tqB0", [34, CQ]),
                        ("mTc1d", [128, C * 128]), ("mTc2d", [34, C * 128]),
                        ("invxn", [128, C]), ("p1", [128, CQ]), ("a1", [128, CQ]),
                        ("p2", [128, CQ]), ("a2", [128, CQ]), ("p3", [128, CQ]),
                        ("hvFd", [QL, CD]), ("n2qd", [QL, C])]:
            dbg_d[nm] = nc.dram_tensor("dbg_" + nm, shp, F32, kind="ExternalOutput")

    with tile.TileContext(nc) as tc:
        with (
            tc.tile_pool(name="sb", bufs=1) as sb,
            tc.tile_pool(name="sb2", bufs=3) as sb2,
        ):
            # ---------------- input DMAs (order = HWDGE serial order) -------
            mT_sb = sb.tile([128, KC, I], DT, tag="mT")
            qT_sb = sb.tile([128, KC, QL], DT, tag="qT")
            W_sb = sb.tile([128, KC, NPAD], DT, tag="W")
            nc.sync.dma_start(mT_sb[:], mT_d[:].rearrange("(k p) n -> p k n", p=128).bitcast(DT))
            nc.sync.dma_start(qT_sb[:], qT_d[:].rearrange("(k p) n -> p k n", p=128).bitcast(DT))
            Wr = W_d[:].rearrange("(k p) n -> p k n", p=128).bitcast(DT)
            for k in range(KC):
                nc.sync.dma_start(W_sb[:, k, 0:512], Wr[:, k, 0:512])
            for k in range(KC):
                nc.sync.dma_start(W_sb[:, k, 512:768], Wr[:, k, 512:768])
            if with_bias:
                b_sb = sb.tile([1, CD], F32, tag="b")
                nc.sync.dma_start(b_sb[:], b_d[:])

            # ---------------- constants (no DMA) ----------------------------
            # float32r tiles cannot be memset directly; memset F32 staging and
            # copy through Act/DVE (engine writes perform the f32r rounding).
            zf = sb.tile([128, 640], F32, tag="zf")
            nc.vector.memset(zf[:], 0.0)
            of = sb.tile([128, 1], F32, tag="of")
            nc.vector.memset(of[:], 1.0)
            o1f = sb.tile([1, 128], F32, tag="o1f")
            nc.vector.memset(o1f[:], 1.0)
            nhf = sb.tile([1, 128], F32, tag="nhf")
            nc.vector.memset(nhf[:], -0.5)
            epsb = sb.tile([128, 1], F32, tag="epsb")
            nc.vector.memset(epsb[:], EPS)
            # identity for PE transposes, built on-chip (a DMA would arrive
            # ~12us in, after the W pieces, and gate the whole transpose block)
            eye = sb.tile([128, 128], F32, tag="eye")
            nc.vector.memset(eye[:], 1.0)
            nc.gpsimd.affine_select(eye[:], eye[:], pattern=[[-1, 128]],
                                    compare_op=mybir.AluOpType.is_equal,
                                    fill=0.0, base=0, channel_multiplier=1)

            ones1 = sb.tile([1, 128], DT, tag="ones1")
            nc.scalar.copy(ones1[:], o1f[:])
            twos1 = sb.tile([1, 128], DT, tag="twos1")
            nc.scalar.activation(twos1[:], o1f[:], ACT.Copy, scale=2.0)
            nhalf1 = sb.tile([1, 128], DT, tag="nhalf1")
            nc.vector.tensor_copy(nhalf1[:], nhf[:])
            onesF = sb.tile([128, 1], DT, tag="onesF")
            nc.scalar.copy(onesF[:], of[:])
            if with_bias:
                onesq = sb.tile([1, QL], DT, tag="onesq")
                nc.vector.tensor_copy(onesq[:], o1f[:, 0:QL])

            # persistent tiles that need zero rows
            hat_m_r = sb.tile([128, HM_W], DT, tag="hatmr")
            nc.vector.tensor_copy(hat_m_r[:, CD:HM_W], zf[:, 0:HM_W - CD])
            mTc1 = sb.tile([128, C, 128], DT, tag="mTc1")
            mTc2 = sb.tile([34, C, 128], DT, tag="mTc2")
            nc.scalar.copy(mTc2[:].rearrange("p c n -> p (c n)"), zf[0:34, 0:640])
            tqA = sb.tile([128, C, QL], DT, tag="tqA")
            tqB = sb.tile([34, C, QL], DT, tag="tqB")
            nc.vector.tensor_copy(tqB[:].rearrange("p c q -> p (c q)"), zf[0:34, 0:CQ])
            svA = sb.tile([128, C, QL], DT, tag="svA")
            svB = sb.tile([34, C, QL], DT, tag="svB")
            nc.scalar.copy(svB[:].rearrange("p c q -> p (c q)"), zf[0:34, 0:CQ])

            tqAf = tqA[:].bitcast(F32).rearrange("p c q -> p (c q)")
            tqBf25 = tqB[0:25].bitcast(F32).rearrange("p c q -> p (c q)")

            # ---------------- projections ----------------------------------
            with tc.tile_pool(name="ps1", bufs=1, space="PSUM") as ps1, \
                 tc.tile_pool(name="pstp", bufs=4, space="PSUM") as pstp:
                psA = ps1.tile([128, 512], F32, tag="psA")
                psB = ps1.tile([128, 256], F32, tag="psB")
                psQA = ps1.tile([128, C, QL], F32, tag="psQA")
                psQB = ps1.tile([34, C, QL], F32, tag="psQB")

                # hat_q (DIRECTLY transposed: out[d,(c,q)] = sum_k W[k,cD+d] q[q,k])
                # shares one bank across classes; start=True clears the whole
                # bank's has_written bits, so each (c, piece) group runs its
                # start..stop back-to-back. k is split in halves combined via
                # SBUF; classes are split c0-2 / c3-4 because c0-2 only read the
                # A columns (<512) of W, which stream in first.
                def q_cs(h, cs):
                    ks = range(3 * h, 3 * h + 3)
                    add_bias = with_bias and h == 1
                    for c in cs:
                        for j, k in enumerate(ks):
                            nc.tensor.matmul(psQA[:, c, :], W_sb[:, k, D * c:D * c + 128],
                                             qT_sb[:, k, :], start=(j == 0),
                                             stop=(j == 2 and not add_bias))
                        if add_bias:
                            nc.tensor.matmul(psQA[:, c, :], b_sb[:, D * c:D * c + 128],
                                             onesq[:], start=False, stop=True)
                        for j, k in enumerate(ks):
                            nc.tensor.matmul(psQB[0:25, c, :], W_sb[:, k, D * c + 128:D * (c + 1)],
                                             qT_sb[:, k, :], start=(j == 0),
                                             stop=(j == 2 and not add_bias))
                        if add_bias:
                            nc.tensor.matmul(psQB[0:25, c, :], b_sb[:, D * c + 128:D * (c + 1)],
                                             onesq[:], start=False, stop=True)

                def q_copy(cs):
                    c0, c1 = cs[0], cs[-1] + 1
                    nc.vector.tensor_copy(tqA[:, c0:c1, :].rearrange("p c q -> p (c q)"),
                                          psQA[:, c0:c1, :].rearrange("p c q -> p (c q)"))
                    nc.vector.tensor_copy(tqB[0:25, c0:c1, :].rearrange("p c q -> p (c q)"),
                                          psQB[0:25, c0:c1, :].rearrange("p c q -> p (c q)"))

                def q_add(cs):
                    c0, c1 = cs[0], cs[-1] + 1
                    nc.vector.tensor_tensor(
                        tqA[:, c0:c1, :].rearrange("p c q -> p (c q)"),
                        tqA[:, c0:c1, :].bitcast(F32).rearrange("p c q -> p (c q)"),
                        psQA[:, c0:c1, :].rearrange("p c q -> p (c q)"), op=ADD)
                    nc.vector.tensor_tensor(
                        tqB[0:25, c0:c1, :].rearrange("p c q -> p (c q)"),
                        tqB[0:25, c0:c1, :].bitcast(F32).rearrange("p c q -> p (c q)"),
                        psQB[0:25, c0:c1, :].rearrange("p c q -> p (c q)"), op=ADD)

                def tposes(cs):
                    for c in cs:
                        t1 = pstp.tile([128, 128], F32, tag="tp")
                        nc.tensor.transpose(t1[:], hat_m_r[:, D * c:D * c + 128].bitcast(F32), eye[:])
                        (nc.vector.tensor_copy if c % 2 else nc.scalar.copy)(mTc1[:, c, :], t1[:])
                        t2 = pstp.tile([25, 128], F32, tag="tp")
                        nc.tensor.transpose(t2[:], hat_m_r[:, D * c + 128:D * (c + 1)].bitcast(F32),
                                            eye[:])
                        (nc.scalar.copy if c % 2 else nc.vector.tensor_copy)(mTc2[0:25, c, :], t2[:])

                # ---- A-column half: hat_m cols 0:512, q classes 0-2 ----------
                for k in range(KC):
                    nc.tensor.matmul(psA[:], mT_sb[:, k, :], W_sb[:, k, 0:512],
                                     start=(k == 0), stop=(k == KC - 1 and not with_bias))
                if with_bias:
                    nc.tensor.matmul(psA[:], ones1[:], b_sb[:, 0:512], start=False, stop=True)
                q_cs(0, [0, 1, 2])
                q_copy([0, 1, 2])
                q_cs(1, [0, 1, 2])
                q_add([0, 1, 2])
                nc.scalar.copy(hat_m_r[:, 0:256], psA[:, 0:256])
                nc.vector.tensor_copy(hat_m_r[:, 256:512], psA[:, 256:512])
                tposes([0, 1, 2])

                # ---- B-column half: hat_m cols 512:765, q classes 3-4 --------
                q_cs(0, [3, 4])
                q_copy([3, 4])
                for k in range(KC):
                    nc.tensor.matmul(psB[:], mT_sb[:, k, :], W_sb[:, k, 512:768],
                                     start=(k == 0), stop=(k == KC - 1 and not with_bias))
                if with_bias:
                    nc.tensor.matmul(psB[:, 0:253], ones1[:], b_sb[:, 512:765],
                                     start=False, stop=True)
                q_cs(1, [3, 4])
                q_add([3, 4])
                nc.scalar.copy(hat_m_r[:, 512:640], psB[:, 0:128])
                nc.vector.tensor_copy(hat_m_r[:, 640:765], psB[:, 128:253])

                hm32 = hat_m_r[:, 0:765].bitcast(F32)

                tposes([3, 4])

                # mTc2 row 32 = -mean_d(hat_m)[c,i] via ones-matmul over mTc
                # (split 512+128: PSUM banks hold 512 fp32/partition max;
                #  reuses the psA/psB banks, free after the hat_m copies)
                psMuA = ps1.tile([1, 512], F32, tag="psA")
                psMuB = ps1.tile([1, 128], F32, tag="psB")
                mTc1f = mTc1[:].rearrange("p c n -> p (c n)")
                mTc2f = mTc2[0:25].rearrange("p c n -> p (c n)")
                nc.tensor.matmul(psMuA[:], onesF[:], mTc1f[:, 0:512],
                                 start=True, stop=False)
                nc.tensor.matmul(psMuA[:], onesF[0:25], mTc2f[:, 0:512],
                                 start=False, stop=True)
                nc.tensor.matmul(psMuB[:], onesF[:], mTc1f[:, 512:640],
                                 start=True, stop=False)
                nc.tensor.matmul(psMuB[:], onesF[0:25], mTc2f[:, 512:640],
                                 start=False, stop=True)
                mTc2r32 = mTc2[32:33, :, :].rearrange("p c n -> p (c n)")
                nc.scalar.activation(mTc2r32[:, 0:256], psMuA[:, 0:256], ACT.Copy, scale=-1.0 / D)
                nc.vector.tensor_scalar(mTc2r32[:, 256:512], psMuA[:, 256:512], -1.0 / D,
                                        None, op0=MUL)
                nc.scalar.activation(mTc2r32[:, 512:640], psMuB[:], ACT.Copy, scale=-1.0 / D)

                # ------------- pearson #1, tq-only part (setup banks) --------
                # (colsum/yn2r run in PE gaps while psMu waits on the transpose
                # copies; the wait queue parks at most 4 blocked instructions)
                sqA1 = sb2.tile([128, CQ], DT, tag="sqA")
                nc.gpsimd.tensor_tensor(sqA1[:], tqAf, tqAf, op=MUL)
                sqB1 = sb2.tile([25, CQ], DT, tag="sqB")
                nc.vector.tensor_tensor(sqB1[:], tqBf25, tqBf25, op=MUL)
                colsum1 = ps1.tile([1, CQ], F32, tag="psQA")
                nc.tensor.matmul(colsum1[:], onesF[:],
                                 tqA[:].rearrange("p c q -> p (c q)"),
                                 start=True, stop=False)
                nc.tensor.matmul(colsum1[:], onesF[0:25],
                                 tqB[0:25].rearrange("p c q -> p (c q)"),
                                 start=False, stop=True)
                tqB32w = tqB[32:33, :, :].rearrange("p c q -> p (c q)")
                nc.scalar.copy(tqB32w[:], colsum1[:])
                yn2r1 = ps1.tile([1, CQ], F32, tag="psQB")
                nc.tensor.matmul(yn2r1[:], onesF[:], sqA1[:], start=True, stop=False)
                nc.tensor.matmul(yn2r1[:], onesF[0:25], sqB1[:], start=False, stop=True)
                T2 = sb.tile([1, CQ], F32, tag="T2")
                nc.vector.tensor_copy(T2[:], yn2r1[:])
                tqB32f = tqB[32:33, :, :].bitcast(F32).rearrange("p c q -> p (c q)")
                csqv1 = sb2.tile([1, CQ], F32, tag="csqv")
                nc.vector.tensor_tensor(csqv1[:], tqB32f, tqB32f, op=MUL)
                yn21 = sb2.tile([1, CQ], F32, tag="yn2")
                nc.vector.scalar_tensor_tensor(yn21[:], csqv1[:], -1.0 / D, yn2r1[:],
                                               op0=MUL, op1=ADD)
                lyn1 = sb2.tile([1, CQ], F32, tag="lyn")
                nc.scalar.activation(lyn1[:], yn21[:], ACT.Ln)
                inv_yn1 = sb2.tile([1, CQ], DT, tag="invyn")
                nc.scalar.activation(inv_yn1[:], lyn1[:], ACT.Exp, scale=-0.5)

                # ------------- m stats part 1: sum_d hat_m^2 ----------------
                xn2r = sb.tile([128, C], F32, tag="xn2r")
                sqs = sb.tile([128, D], F32, tag="sqs")
                for c in range(C):
                    nc.vector.scalar_tensor_tensor(
                        sqs[:], hm32[:, D * c:D * (c + 1)], 1.0,
                        hm32[:, D * c:D * (c + 1)], op0=MUL, op1=MUL,
                        accum_out=xn2r[:, c:c + 1])

            if dbg:
                nc.sync.dma_start(dbg_d["hatm"][:], hm32)
                nc.sync.dma_start(dbg_d["tqA0"][:], tqAf)
                nc.sync.dma_start(dbg_d["tqB0"][:], tqB[:].bitcast(F32).rearrange("p c q -> p (c q)"))
                nc.sync.dma_start(dbg_d["mTc1d"][:], mTc1[:].bitcast(F32).rearrange("p c n -> p (c n)"))
                nc.sync.dma_start(dbg_d["mTc2d"][:], mTc2[:].bitcast(F32).rearrange("p c n -> p (c n)"))

            inv_xn = sb.tile([128, C], F32, tag="invxn")
            ixb = inv_xn[:].rearrange("p (c a) -> p c a", a=1).broadcast_to((128, C, QL))

            # ---------------- routing --------------------------------------
            with tc.tile_pool(name="ps2", bufs=1, space="PSUM") as ps2:

                def pearson_late(inv_yn, lhs1, extra=None):
                    iyb = ps2.tile([128, CQ], F32, tag="bcast")
                    nc.tensor.matmul(iyb[:], lhs1, inv_yn[:], start=True, stop=True)
                    if extra is not None:
                        extra()  # setup-only work that must precede the num matmuls
                    # num[i,(c,q)]: per-class A/B pairs back-to-back (groups share
                    # one bank; start=True clears the bank's has_written bits)
                    num = ps2.tile([128, C, QL], F32, tag="num")
                    for c in range(C):
                        nc.tensor.matmul(num[:, c, :], mTc1[:, c, :], tqA[:, c, :],
                                         start=True, stop=False)
                        nc.tensor.matmul(num[:, c, :], mTc2[:, c, :], tqB[:, c, :],
                                         start=False, stop=True)
                    # p = tanh(num * inv_xn * inv_yn); tanh(x) = 1 - 2/(1+exp(2x))
                    pp1 = sb2.tile([128, C, QL], F32, tag="pp1")
                    nc.vector.tensor_tensor(pp1[:], num[:], ixb, op=MUL)
                    pp = sb2.tile([128, CQ], F32, tag="pp")
                    nc.vector.tensor_tensor(pp[:], pp1[:].rearrange("p c q -> p (c q)"),
                                            iyb[:], op=MUL)
                    e2 = sb2.tile([128, CQ], F32, tag="e2")
                    nc.scalar.activation(e2[:], pp[:], ACT.Exp, scale=2.0)
                    den = sb2.tile([128, CQ], F32, tag="den")
                    nc.vector.tensor_scalar(den[:], e2[:], 1.0, None, op0=ADD)
                    rr = sb2.tile([128, CQ], F32, tag="rr")
                    nc.vector.reciprocal(rr[:], den[:])
                    return rr

                def pearson(extra=None):
                    """p = tanh(centered-corr(mT, tq)); returns rr tile [128, CQ]."""
                    # squares (row 32 of tqB excluded; centering via csq)
                    sqA = sb2.tile([128, CQ], DT, tag="sqA")
                    nc.gpsimd.tensor_tensor(sqA[:], tqAf, tqAf, op=MUL)
                    sqB = sb2.tile([25, CQ], DT, tag="sqB")
                    nc.vector.tensor_tensor(sqB[:], tqBf25, tqBf25, op=MUL)
                    # colsum(tq) -> tqB row 32 (feeds the centered num matmul)
                    colsum = ps2.tile([1, CQ], F32, tag="colsum")
                    nc.tensor.matmul(colsum[:], onesF[:],
                                     tqA[:].rearrange("p c q -> p (c q)"),
                                     start=True, stop=False)
                    nc.tensor.matmul(colsum[:], onesF[0:25],
                                     tqB[0:25].rearrange("p c q -> p (c q)"),
                                     start=False, stop=True)
                    nc.scalar.copy(tqB[32:33, :, :].rearrange("p c q -> p (c q)"), colsum[:])
                    # yn2 = sum(tq^2) - colsum^2/D, centered variance of tq
                    yn2r = ps2.tile([1, CQ], F32, tag="cs2")
                    nc.tensor.matmul(yn2r[:], onesF[:], sqA[:], start=True, stop=False)
                    nc.tensor.matmul(yn2r[:], onesF[0:25], sqB[:], start=False, stop=True)
                    csqv = sb2.tile([1, CQ], F32, tag="csqv")
                    nc.vector.tensor_tensor(csqv[:], tqB32f, tqB32f, op=MUL)
                    yn2 = sb2.tile([1, CQ], F32, tag="yn2")
                    nc.vector.scalar_tensor_tensor(yn2[:], csqv[:], -1.0 / D, yn2r[:],
                                                   op0=MUL, op1=ADD)
                    lyn = sb2.tile([1, CQ], F32, tag="lyn")
                    nc.scalar.activation(lyn[:], yn2[:], ACT.Ln)
                    inv_yn = sb2.tile([1, CQ], DT, tag="invyn")
                    nc.scalar.activation(inv_yn[:], lyn[:], ACT.Exp, scale=-0.5)
                    return pearson_late(inv_yn, ones1[:])

                def stats_tail():
                    # m stats part 2: -mean[i,c] from mTc2 row 32 via 5 tiny
                    # transposes; then inv_xn = 1/sqrt(sum hm^2 - D*mean^2).
                    t_nm = ps2.tile([128, C], F32, tag="nm")
                    for c in range(C):
                        nc.tensor.transpose(t_nm[:, c:c + 1],
                                            mTc2[32:33, c, :].bitcast(F32), eye[32:33, 32:33])
                    nmean = sb.tile([128, C], F32, tag="nmean")
                    nc.vector.tensor_copy(nmean[:], t_nm[:])
                    nm2 = sb.tile([128, C], F32, tag="nm2")
                    nc.vector.tensor_tensor(nm2[:], nmean[:], nmean[:], op=MUL)
                    xn2 = sb.tile([128, C], F32, tag="xn2")
                    nc.vector.scalar_tensor_tensor(xn2[:], nm2[:], -float(D), xn2r[:],
                                                   op0=MUL, op1=ADD)
                    lxn = sb.tile([128, C], F32, tag="lxn")
                    nc.scalar.activation(lxn[:], xn2[:], ACT.Ln)
                    nc.scalar.activation(inv_xn[:], lxn[:], ACT.Exp, scale=-0.5)

                def p_from_rr(rr):
                    p_new = sb2.tile([128, CQ], F32, tag="p")
                    nc.vector.tensor_scalar(p_new[:], rr[:], -2.0, 1.0, op0=MUL, op1=ADD)
                    return p_new

                rr = pearson_late(inv_yn1, ones1[:], extra=stats_tail)
                a_t = None
                p_t = None

                for it in range(2):
                    dsp = sb2.tile([128, C, QL], DT, tag="dsp")
                    if it == 0:
                        # softmax(0) = 1/C exactly; dsp = p + 1/C straight from rr
                        nc.vector.tensor_scalar(dsp[:].rearrange("p c q -> p (c q)"),
                                                rr[:], -2.0, 1.0 + 1.0 / C, op0=MUL, op1=ADD)
                        p_t = p_from_rr(rr)
                        if dbg:
                            nc.sync.dma_start(dbg_d["p1"][:], p_t[:])
                            nc.sync.dma_start(dbg_d["invxn"][:], inv_xn[:])
                    else:
                        p_t = p_from_rr(rr)
                        if dbg:
                            nc.sync.dma_start(dbg_d["p2"][:], p_t[:])
                        ea = sb2.tile([128, CQ], F32, tag="ea")
                        nc.scalar.activation(ea[:], a_t[:], ACT.Exp)
                        asum = sb2.tile([128, QL], F32, tag="asum")
                        nc.vector.tensor_reduce(asum[:], ea[:].rearrange("p (c q) -> p q c", c=C),
                                                axis=AX, op=ADD)
                        rs = sb2.tile([128, QL], F32, tag="rs")
                        nc.vector.reciprocal(rs[:], asum[:])
                        dd = sb2.tile([128, C, QL], F32, tag="dd")
                        nc.vector.tensor_tensor(
                            dd[:], ea[:].rearrange("p (c q) -> p c q", c=C),
                            rs[:].rearrange("p (a q) -> p a q", a=1).broadcast_to((128, C, QL)),
                            op=MUL)
                        dd1 = sb2.tile([128, CQ], F32, tag="dd1")
                        nc.vector.tensor_scalar(dd1[:], dd[:].rearrange("p c q -> p (c q)"),
                                                1.0, None, op0=ADD)
                        nc.vector.scalar_tensor_tensor(dsp[:].rearrange("p c q -> p (c q)"),
                                                       rr[:], -2.0, dd1[:], op0=MUL, op1=ADD)

                    # hv[d,(c,q)] in PSUM (consumed in place; never copied to SBUF)
                    hvA = ps2.tile([128, C, QL], F32, tag="hvA")
                    hvB = ps2.tile([26, C, QL], F32, tag="hvB")
                    for c in range(C):
                        nc.tensor.matmul(hvA[:, c, :], hat_m_r[:, D * c:D * c + 128], dsp[:, c, :],
                                         start=True, stop=True)
                        nc.tensor.matmul(hvB[:, c, :], hat_m_r[:, D * c + 128:D * c + 154], dsp[:, c, :],
                                         start=True, stop=True)
                    hvAf = hvA[:].rearrange("p c q -> p (c q)")
                    hvBf25 = hvB[0:25].rearrange("p c q -> p (c q)")
                    # n2 = sum_d hv^2 (raw); squash scale applied later
                    sqhA = sb2.tile([128, CQ], DT, tag="sqhA")
                    nc.scalar.activation(sqhA[:], hvAf, ACT.Square)
                    sqhB = sb2.tile([25, CQ], DT, tag="sqhB")
                    nc.scalar.activation(sqhB[:], hvBf25, ACT.Square)
                    # hv staged to SBUF so sv ops can read the sBh broadcast
                    # straight from PSUM (one-PSUM-input rule); 0.5*tqB is
                    # pre-scaled on Pool while DVE runs the squash chain
                    vA = sb2.tile([128, CQ], F32, tag="vA")
                    nc.scalar.copy(vA[:], hvAf)
                    vB = sb2.tile([25, CQ], F32, tag="vB")
                    nc.vector.tensor_copy(vB[:], hvBf25)
                    tqhB = sb2.tile([25, CQ], F32, tag="tqhB")
                    nc.gpsimd.tensor_scalar(tqhB[:], tqBf25, 0.5, None, op0=MUL)
                    n2 = ps2.tile([1, CQ], F32, tag="cs2")
                    nc.tensor.matmul(n2[:], onesF[:], sqhA[:], start=True, stop=False)
                    nc.tensor.matmul(n2[:], onesF[0:25], sqhB[:], start=False, stop=True)
                    # -s = (1/(1+n2) - 1) / sqrt(n2+eps)
                    n2p1 = sb2.tile([1, CQ], F32, tag="n2p1")
                    nc.vector.tensor_scalar(n2p1[:], n2[:], 1.0, None, op0=ADD)
                    r1 = sb2.tile([1, CQ], F32, tag="r1")
                    nc.vector.reciprocal(r1[:], n2p1[:])
                    ln2 = sb2.tile([1, CQ], F32, tag="ln2")
                    nc.scalar.activation(ln2[:], n2[:], ACT.Ln, bias=epsb[0:1, :])
                    r2 = sb2.tile([1, CQ], F32, tag="r2")
                    nc.scalar.activation(r2[:], ln2[:], ACT.Exp, scale=-0.5)
                    nsrow = sb2.tile([1, CQ], DT, tag="nsrow")
                    nc.vector.scalar_tensor_tensor(nsrow[:], r1[:], 1.0, r2[:],
                                                   op0=SUB, op1=MUL)
                    # 0.5*s broadcast to all partitions: (-0.5) x (-s)
                    sBh = ps2.tile([128, CQ], F32, tag="num")
                    nc.tensor.matmul(sBh[:], nhalf1[:], nsrow[:], start=True, stop=True)
                    # sv = (0.5*s)*hv; tq = 0.5*tq + sv
                    nc.vector.tensor_tensor(svA[:].rearrange("p c q -> p (c q)"),
                                            vA[:], sBh[:], op=MUL)
                    nc.vector.tensor_tensor(svB[0:25].rearrange("p c q -> p (c q)"),
                                            vB[:], sBh[0:25, :], op=MUL)
                    nc.vector.scalar_tensor_tensor(tqA[:].rearrange("p c q -> p (c q)"),
                                                   tqAf, 0.5,
                                                   svA[:].rearrange("p c q -> p (c q)").bitcast(F32),
                                                   op0=MUL, op1=ADD)
                    nc.gpsimd.tensor_tensor(tqB[0:25].rearrange("p c q -> p (c q)"),
                                            tqhB[:],
                                            svB[0:25].rearrange("p c q -> p (c q)").bitcast(F32),
                                            op=ADD)
                    # mdv' = mT . (0.5*s*v)  (rows 25..33 of svB stay 0 -> uncentered)
                    mdv = ps2.tile([128, C, QL], F32, tag="mdv")
                    for c in range(C):
                        nc.tensor.matmul(mdv[:, c, :], mTc1[:, c, :], svA[:, c, :],
                                         start=True, stop=False)
                        nc.tensor.matmul(mdv[:, c, :], mTc2[:, c, :], svB[:, c, :],
                                         start=False, stop=True)
                    # a += p * s * mdv_raw = p * (2*mdv')
                    if it == 0:
                        a_t = sb2.tile([128, CQ], F32, tag="a")
                        nc.vector.scalar_tensor_tensor(a_t[:], mdv[:].rearrange("p c q -> p (c q)"),
                                                       2.0, p_t[:], op0=MUL, op1=MUL)
                    else:
                        pm2 = sb2.tile([128, CQ], F32, tag="pm2")
                        nc.vector.scalar_tensor_tensor(pm2[:], mdv[:].rearrange("p c q -> p (c q)"),
                                                       2.0, p_t[:], op0=MUL, op1=MUL)
                        a_new = sb2.tile([128, CQ], F32, tag="a")
                        nc.vector.tensor_tensor(a_new[:], a_t[:], pm2[:], op=ADD)
                        a_t = a_new
                    if dbg:
                        nc.sync.dma_start(dbg_d["a1" if it == 0 else "a2"][:], a_t[:])

                    rr = pearson()

                # ---------------- final ------------------------------------
                p_t = p_from_rr(rr)
                if dbg:
                    nc.sync.dma_start(dbg_d["p3"][:], p_t[:])
                ea = sb2.tile([128, CQ], F32, tag="ea")
                nc.scalar.activation(ea[:], a_t[:], ACT.Exp)
                asum = sb2.tile([128, QL], F32, tag="asum")
                nc.vector.tensor_reduce(asum[:], ea[:].rearrange("p (c q) -> p q c", c=C),
                                        axis=AX, op=ADD)
                rs = sb2.tile([128, QL], F32, tag="rs")
                nc.vector.reciprocal(rs[:], asum[:])
                dd = sb2.tile([128, C, QL], F32, tag="dd")
                nc.vector.tensor_tensor(
                    dd[:], ea[:].rearrange("p (c q) -> p c q", c=C),
                    rs[:].rearrange("p (a q) -> p a q", a=1).broadcast_to((128, C, QL)), op=MUL)
                dd1 = sb2.tile([128, CQ], F32, tag="dd1")
                nc.vector.tensor_scalar(dd1[:], dd[:].rearrange("p c q -> p (c q)"),
                                        1.0, None, op0=ADD)
                dspF = sb2.tile([128, C, QL], DT, tag="dsp")
                nc.vector.scalar_tensor_tensor(dspF[:].rearrange("p c q -> p (c q)"),
                                               rr[:], -2.0, dd1[:], op0=MUL, op1=ADD)

                # final hv: per-class matmul, N=256 window (cols 765+ are zero).
                # n2q via Act Square+accum straight from PSUM; hvF copies on DVE.
                hvF = sb.tile([QL, CD], F32, tag="hvF")
                n2q = sb2.tile([QL, C], F32, tag="n2q")
                for c in range(C):
                    fps = ps2.tile([QL, 256], F32, tag=("hvA" if c % 2 == 0 else "mdv"))
                    nc.tensor.matmul(fps[:], dspF[:, c, :], hat_m_r[:, D * c:D * c + 256],
                                     start=True, stop=True)
                    sqf = sb2.tile([QL, D], F32, tag="sqf")
                    nc.scalar.activation(sqf[:], fps[:, 0:D], ACT.Square,
                                         accum_out=n2q[:, c:c + 1])
                    nc.vector.tensor_copy(hvF[:, D * c:D * (c + 1)], fps[:, 0:D])
                if dbg:
                    nc.sync.dma_start(dbg_d["hvFd"][:], hvF[:])
                    nc.sync.dma_start(dbg_d["n2qd"][:], n2q[:])
                # fs = squash scale [QL, C] (positive)
                fp1 = sb2.tile([QL, C], F32, tag="fp1")
                nc.vector.tensor_scalar(fp1[:], n2q[:], 1.0, None, op0=ADD)
                fr1 = sb2.tile([QL, C], F32, tag="fr1")
                nc.vector.reciprocal(fr1[:], fp1[:])
                fln = sb2.tile([QL, C], F32, tag="fln")
                nc.scalar.activation(fln[:], n2q[:], ACT.Ln, bias=epsb[0:QL, :])
                fr2 = sb2.tile([QL, C], F32, tag="fr2")
                nc.scalar.activation(fr2[:], fln[:], ACT.Exp, scale=-0.5)
                omr = sb2.tile([QL, C], F32, tag="omr")
                nc.vector.tensor_scalar(omr[:], fr1[:], -1.0, 1.0, op0=MUL, op1=ADD)
                fs = sb2.tile([QL, C], F32, tag="fs")
                nc.vector.tensor_tensor(fs[:], omr[:], fr2[:], op=MUL)
                outT = sb.tile([QL, CD], F32, tag="outT")
                for c in range(C):
                    if c % 2 == 0:
                        nc.vector.tensor_scalar(outT[:, D * c:D * (c + 1)],
                                                hvF[:, D * c:D * (c + 1)],
                                                fs[:, c:c + 1], None, op0=MUL)
                    else:
                        nc.scalar.activation(outT[:, D * c:D * (c + 1)],
                                             hvF[:, D * c:D * (c + 1)], ACT.Copy,
                                             scale=fs[:, c:c + 1])
                nc.sync.dma_start(out_d[:], outT[:])

    # All activations use only {Ln, Exp, Copy, Square}, which live together in
    # act func set 6 (natural_log_exp_and_others). The default solver alternates
    # sets, inserting table reloads (~1.3us each); one load suffices.
    def _single_act_table_load():
        inst = mybir.InstLoadActFuncSet(
            name=nc.get_next_instruction_name(), ins=[], outs=[],
            act_func_set_id=6,
        )
        inst.engine = mybir.EngineType.Activation
        nc.register_instruction(inst)
        for blk in nc.main_func.blocks:
            for idx, bi in enumerate(blk.instructions):
                if isinstance(bi, mybir.InstActivation):
                    blk.instructions.insert(idx, inst)
                    return
        raise AssertionError("no activation found")

    nc.insert_act_table_loads = _single_act_table_load
    nc.compile()
    return nc


_CACHE = {}
LAST_EXEC_NS = None
LAST_RESULTS = None


def kernel(m, q, W, b):
    m = np.asarray(m, dtype=np.float32)
    q = np.asarray(q, dtype=np.float32)
    W = np.asarray(W, dtype=np.float32)
    b = np.asarray(b, dtype=np.float32)
    assert m.shape == (I, K) and q.shape == (NCORES * QL, K) and W.shape == (K, CD)

    with_bias = bool(np.any(b))
    dbg = bool(int(os.environ.get("KERNEL_DBG", "0")))
    key = ("v2", with_bias, str(DT), dbg)
    if key not in _CACHE:
        _CACHE[key] = build(with_bias, dbg)
    nc = _CACHE[key]

    Wp = np.zeros((K, NPAD), dtype=np.float32)
    Wp[:, :CD] = W
    mT = np.ascontiguousarray(m.T)

    in_maps = []
    for i in range(NCORES):
        qT = np.ascontiguousarray(q[QL * i:QL * (i + 1)].T)
        im = {"mT": mT, "qT": qT, "Wp": Wp}
        if with_bias:
            im["b"] = b.reshape(1, CD)
        in_maps.append(im)

    res = run_bass_kernel_spmd(nc, in_maps, list(range(NCORES)))
    global LAST_EXEC_NS, LAST_RESULTS
    LAST_EXEC_NS = res.exec_time_ns
    LAST_RESULTS = res.results
    out = np.concatenate([res.results[i]["out"] for i in range(NCORES)], axis=0)
    return out.astype(np.float32)


if __name__ == "__main__":
    rng = np.random.default_rng(0)
    m = rng.standard_normal((I, K)).astype(np.float32)
    q = rng.standard_normal((NCORES * QL, K)).astype(np.float32)
    W = (rng.standard_normal((K, CD)) * 0.02).astype(np.float32)
    b = np.zeros((CD,), dtype=np.float32)
    out = kernel(m=m, q=q, W=W, b=b)
    print("out", out.shape, out.dtype, np.abs(out).mean())


# revision 32
# speedup vs baseline: 1.5346x; 1.0239x over previous
"""DMR induction routing kernel for Trainium2 (Bass/Tile), 8-core data-parallel.

Problem: nn_DMRInduction. Full inputs:
  m [128, 768], q [256, 768], W [768, 765], b [765] -> out [256, 765] fp32.

Sharding: Q=256 split 8 ways (32 queries/core); m, W, b replicated.

v2 layout/dataflow (per core):
  - hat_m_r  [I=128, 1024] (I on partitions; cols 0..764 = m @ W, cols 765+ zero
      so the final per-class matmuls can stream N=256 on the fp32r fast path)
  - mTc1 [128, C, 128] / mTc2 [34, C, 128]: per-class transposes of hat_m
      (d on partitions). mTc2 row 32 = -mean_d(hat_m) per (c,i), computed by a
      ones-matmul over mTc + scaled copy, so the pearson numerator matmul is
      centered for free (row 32 of tqB carries colsum(tq)).
  - tq [d, (c,q)]: computed DIRECTLY transposed from qT/W with 60 small
      matmuls into PSUM (no PE eye-transposes for the q side at all).
  - routing state a, p, dsp: [I=128, C*Q=160].
  - squash/pearson scalars on [1, 160] rows; broadcasts via 1-row matmuls.
  - iteration v is consumed straight from PSUM (svA = hv_psum * 0.5s); the
    m-dot-v matmul runs on the scaled v, so a += p*(2*mdv') needs one fused op.
"""
import os
import sys

for _p in ("/opt/trn_rl_repo", "/root/.axon_site/_ro/trn_rl_repo"):
    if os.path.isdir(_p) and _p not in sys.path:
        sys.path.insert(0, _p)

import numpy as np
import concourse.bass as bass
import concourse.bacc as bacc
import concourse.mybir as mybir
import concourse.tile as tile
from concourse.bass_utils import run_bass_kernel_spmd

F32 = mybir.dt.float32
# float32r uses the fast PE path (1 cyc/row at N>=256 vs 4) at ~2.5e-4
# scale-relative output error (tolerance is 2e-2). KERNEL_MM_DT=float32
# restores exact matmuls.
DT = getattr(mybir.dt, os.environ.get("KERNEL_MM_DT", "float32r"))

NCORES = 8
I = 128         # memory capsules
C = 5           # capsule classes
D = 153         # dim per capsule
CD = C * D      # 765
K = 768         # input dim
KC = K // 128   # 6 contraction chunks
QL = 32         # queries per core
CQ = C * QL     # 160
NPAD = 768      # W padded to 768 cols so fp32r matmuls stream N>=256
HM_W = 1024     # hat_m_r padded width (final matmuls read 256-wide windows)
EPS = 1e-8
AX = mybir.AxisListType.X
MUL = mybir.AluOpType.mult
ADD = mybir.AluOpType.add
SUB = mybir.AluOpType.subtract
ACT = mybir.ActivationFunctionType


def build(with_bias: bool, dbg: bool = False):
    nc = bacc.Bacc("TRN2", target_bir_lowering=False, debug=False)

    mT_d = nc.dram_tensor("mT", [K, I], F32, kind="ExternalInput")
    qT_d = nc.dram_tensor("qT", [K, QL], F32, kind="ExternalInput")
    W_d = nc.dram_tensor("Wp", [K, NPAD], F32, kind="ExternalInput")
    if with_bias:
        b_d = nc.dram_tensor("b", [1, CD], F32, kind="ExternalInput")
    out_d = nc.dram_tensor("out", [QL, CD], F32, kind="ExternalOutput")
    dbg_d = {}
    if dbg:
        for nm, shp in [("hatm", [128, CD]), ("tqA0", [128, CQ]), ("tqB0", [34, CQ]),
                        ("mTc1d", [128, C * 128]), ("mTc2d", [34, C * 128]),
                        ("invxn", [128, C]), ("p1", [128, CQ]), ("a1", [128, CQ]),
                        ("p2", [128, CQ]), ("a2", [128, CQ]), ("p3", [128, CQ]),
                        ("n2qd", [QL, C])]:
            dbg_d[nm] = nc.dram_tensor("dbg_" + nm, shp, F32, kind="ExternalOutput")

    with tile.TileContext(nc) as tc:
        with (
            tc.tile_pool(name="sb", bufs=1) as sb,
            tc.tile_pool(name="sb2", bufs=3) as sb2,
        ):
            # ---------------- input DMAs (order = HWDGE serial order) -------
            mT_sb = sb.tile([128, KC, I], DT, tag="mT")
            qT_sb = sb.tile([128, KC, QL], DT, tag="qT")
            W_sb = sb.tile([128, KC, NPAD], DT, tag="W")
            nc.sync.dma_start(mT_sb[:], mT_d[:].rearrange("(k p) n -> p k n", p=128).bitcast(DT))
            nc.sync.dma_start(qT_sb[:], qT_d[:].rearrange("(k p) n -> p k n", p=128).bitcast(DT))
            Wr = W_d[:].rearrange("(k p) n -> p k n", p=128).bitcast(DT)
            for k in range(KC):
                nc.sync.dma_start(W_sb[:, k, 0:512], Wr[:, k, 0:512])
            for k in range(KC):
                nc.sync.dma_start(W_sb[:, k, 512:768], Wr[:, k, 512:768])
            if with_bias:
                b_sb = sb.tile([1, CD], F32, tag="b")
                nc.sync.dma_start(b_sb[:], b_d[:])

            # ---------------- constants (no DMA) ----------------------------
            # float32r tiles cannot be memset directly; memset F32 staging and
            # copy through Act/DVE (engine writes perform the f32r rounding).
            zf = sb.tile([128, 640], F32, tag="zf")
            nc.vector.memset(zf[:], 0.0)
            of = sb.tile([128, 1], F32, tag="of")
            nc.vector.memset(of[:], 1.0)
            o1f = sb.tile([1, 128], F32, tag="o1f")
            nc.vector.memset(o1f[:], 1.0)
            nhf = sb.tile([1, 128], F32, tag="nhf")
            nc.vector.memset(nhf[:], -0.5)
            epsb = sb.tile([128, 1], F32, tag="epsb")
            nc.vector.memset(epsb[:], EPS)
            # identity for PE transposes, built on-chip (a DMA would arrive
            # ~12us in, after the W pieces, and gate the whole transpose block)
            eye = sb.tile([128, 128], F32, tag="eye")
            nc.vector.memset(eye[:], 1.0)
            nc.gpsimd.affine_select(eye[:], eye[:], pattern=[[-1, 128]],
                                    compare_op=mybir.AluOpType.is_equal,
                                    fill=0.0, base=0, channel_multiplier=1)

            ones1 = sb.tile([1, 128], DT, tag="ones1")
            nc.scalar.copy(ones1[:], o1f[:])
            twos1 = sb.tile([1, 128], DT, tag="twos1")
            nc.scalar.activation(twos1[:], o1f[:], ACT.Copy, scale=2.0)
            nhalf1 = sb.tile([1, 128], DT, tag="nhalf1")
            nc.vector.tensor_copy(nhalf1[:], nhf[:])
            onesF = sb.tile([128, 1], DT, tag="onesF")
            nc.scalar.copy(onesF[:], of[:])
            if with_bias:
                onesq = sb.tile([1, QL], DT, tag="onesq")
                nc.vector.tensor_copy(onesq[:], o1f[:, 0:QL])

            # persistent tiles that need zero rows
            hat_m_r = sb.tile([128, HM_W], DT, tag="hatmr")
            nc.vector.tensor_copy(hat_m_r[:, CD:HM_W], zf[:, 0:HM_W - CD])
            mTc1 = sb.tile([128, C, 128], DT, tag="mTc1")
            mTc2 = sb.tile([34, C, 128], DT, tag="mTc2")
            nc.scalar.copy(mTc2[:].rearrange("p c n -> p (c n)"), zf[0:34, 0:640])
            tqA = sb.tile([128, C, QL], DT, tag="tqA")
            tqB = sb.tile([34, C, QL], DT, tag="tqB")
            nc.vector.tensor_copy(tqB[:].rearrange("p c q -> p (c q)"), zf[0:34, 0:CQ])
            svA = sb.tile([128, C, QL], DT, tag="svA")
            svB = sb.tile([34, C, QL], DT, tag="svB")
            nc.scalar.copy(svB[:].rearrange("p c q -> p (c q)"), zf[0:34, 0:CQ])

            tqAf = tqA[:].bitcast(F32).rearrange("p c q -> p (c q)")
            tqBf25 = tqB[0:25].bitcast(F32).rearrange("p c q -> p (c q)")

            # ---------------- projections ----------------------------------
            with tc.tile_pool(name="ps1", bufs=1, space="PSUM") as ps1, \
                 tc.tile_pool(name="pstp", bufs=4, space="PSUM") as pstp:
                psA = ps1.tile([128, 512], F32, tag="psA")
                psB = ps1.tile([128, 256], F32, tag="psB")
                psQA = ps1.tile([128, C, QL], F32, tag="psQA")
                psQB = ps1.tile([34, C, QL], F32, tag="psQB")

                # hat_q (DIRECTLY transposed: out[d,(c,q)] = sum_k W[k,cD+d] q[q,k])
                # shares one bank across classes; start=True clears the whole
                # bank's has_written bits, so each (c, piece) group runs its
                # start..stop back-to-back. k is split in halves combined via
                # SBUF; classes are split c0-2 / c3-4 because c0-2 only read the
                # A columns (<512) of W, which stream in first.
                def q_cs(h, cs):
                    ks = range(3 * h, 3 * h + 3)
                    add_bias = with_bias and h == 1
                    for c in cs:
                        for j, k in enumerate(ks):
                            nc.tensor.matmul(psQA[:, c, :], W_sb[:, k, D * c:D * c + 128],
                                             qT_sb[:, k, :], start=(j == 0),
                                             stop=(j == 2 and not add_bias))
                        if add_bias:
                            nc.tensor.matmul(psQA[:, c, :], b_sb[:, D * c:D * c + 128],
                                             onesq[:], start=False, stop=True)
                        for j, k in enumerate(ks):
                            nc.tensor.matmul(psQB[0:25, c, :], W_sb[:, k, D * c + 128:D * (c + 1)],
                                             qT_sb[:, k, :], start=(j == 0),
                                             stop=(j == 2 and not add_bias))
                        if add_bias:
                            nc.tensor.matmul(psQB[0:25, c, :], b_sb[:, D * c + 128:D * (c + 1)],
                                             onesq[:], start=False, stop=True)

                def q_copy(cs):
                    c0, c1 = cs[0], cs[-1] + 1
                    nc.vector.tensor_copy(tqA[:, c0:c1, :].rearrange("p c q -> p (c q)"),
                                          psQA[:, c0:c1, :].rearrange("p c q -> p (c q)"))
                    nc.vector.tensor_copy(tqB[0:25, c0:c1, :].rearrange("p c q -> p (c q)"),
                                          psQB[0:25, c0:c1, :].rearrange("p c q -> p (c q)"))

                def q_add(cs):
                    c0, c1 = cs[0], cs[-1] + 1
                    nc.vector.tensor_tensor(
                        tqA[:, c0:c1, :].rearrange("p c q -> p (c q)"),
                        tqA[:, c0:c1, :].bitcast(F32).rearrange("p c q -> p (c q)"),
                        psQA[:, c0:c1, :].rearrange("p c q -> p (c q)"), op=ADD)
                    nc.vector.tensor_tensor(
                        tqB[0:25, c0:c1, :].rearrange("p c q -> p (c q)"),
                        tqB[0:25, c0:c1, :].bitcast(F32).rearrange("p c q -> p (c q)"),
                        psQB[0:25, c0:c1, :].rearrange("p c q -> p (c q)"), op=ADD)

                def tposes(cs):
                    for c in cs:
                        t1 = pstp.tile([128, 128], F32, tag="tp")
                        nc.tensor.transpose(t1[:], hat_m_r[:, D * c:D * c + 128].bitcast(F32), eye[:])
                        (nc.vector.tensor_copy if c % 2 else nc.scalar.copy)(mTc1[:, c, :], t1[:])
                        t2 = pstp.tile([25, 128], F32, tag="tp")
                        nc.tensor.transpose(t2[:], hat_m_r[:, D * c + 128:D * (c + 1)].bitcast(F32),
                                            eye[:])
                        (nc.scalar.copy if c % 2 else nc.vector.tensor_copy)(mTc2[0:25, c, :], t2[:])

                # ---- A-column half: hat_m cols 0:512, q classes 0-2 ----------
                for k in range(KC):
                    nc.tensor.matmul(psA[:], mT_sb[:, k, :], W_sb[:, k, 0:512],
                                     start=(k == 0), stop=(k == KC - 1 and not with_bias))
                if with_bias:
                    nc.tensor.matmul(psA[:], ones1[:], b_sb[:, 0:512], start=False, stop=True)
                q_cs(0, [0, 1, 2])
                q_copy([0, 1, 2])
                q_cs(1, [0, 1, 2])
                q_add([0, 1, 2])
                nc.scalar.copy(hat_m_r[:, 0:256], psA[:, 0:256])
                nc.vector.tensor_copy(hat_m_r[:, 256:512], psA[:, 256:512])
                tposes([0, 1, 2])

                # ---- B-column half: hat_m cols 512:765, q classes 3-4 --------
                q_cs(0, [3, 4])
                q_copy([3, 4])
                for k in range(KC):
                    nc.tensor.matmul(psB[:], mT_sb[:, k, :], W_sb[:, k, 512:768],
                                     start=(k == 0), stop=(k == KC - 1 and not with_bias))
                if with_bias:
                    nc.tensor.matmul(psB[:, 0:253], ones1[:], b_sb[:, 512:765],
                                     start=False, stop=True)
                nc.scalar.copy(hat_m_r[:, 512:640], psB[:, 0:128])
                nc.vector.tensor_copy(hat_m_r[:, 640:765], psB[:, 128:253])
                q_cs(1, [3, 4])
                q_add([3, 4])

                hm32 = hat_m_r[:, 0:765].bitcast(F32)

                tposes([3, 4])

                # mTc2 row 32 = -mean_d(hat_m)[c,i] via ones-matmul over mTc
                # (split 512+128: PSUM banks hold 512 fp32/partition max;
                #  reuses the psA/psB banks, free after the hat_m copies)
                psMuA = ps1.tile([1, 512], F32, tag="psA")
                psMuB = ps1.tile([1, 128], F32, tag="psB")
                mTc1f = mTc1[:].rearrange("p c n -> p (c n)")
                mTc2f = mTc2[0:25].rearrange("p c n -> p (c n)")
                nc.tensor.matmul(psMuA[:], onesF[:], mTc1f[:, 0:512],
                                 start=True, stop=False)
                nc.tensor.matmul(psMuA[:], onesF[0:25], mTc2f[:, 0:512],
                                 start=False, stop=True)
                nc.tensor.matmul(psMuB[:], onesF[:], mTc1f[:, 512:640],
                                 start=True, stop=False)
                nc.tensor.matmul(psMuB[:], onesF[0:25], mTc2f[:, 512:640],
                                 start=False, stop=True)
                mTc2r32 = mTc2[32:33, :, :].rearrange("p c n -> p (c n)")
                nc.scalar.activation(mTc2r32[:, 0:256], psMuA[:, 0:256], ACT.Copy, scale=-1.0 / D)
                nc.vector.tensor_scalar(mTc2r32[:, 256:512], psMuA[:, 256:512], -1.0 / D,
                                        None, op0=MUL)
                nc.scalar.activation(mTc2r32[:, 512:640], psMuB[:], ACT.Copy, scale=-1.0 / D)

                # ------------- pearson #1, tq-only part (setup banks) --------
                # (colsum/yn2r run in PE gaps while psMu waits on the transpose
                # copies; the wait queue parks at most 4 blocked instructions)
                sqA1 = sb2.tile([128, CQ], DT, tag="sqA")
                nc.gpsimd.tensor_tensor(sqA1[:], tqAf, tqAf, op=MUL)
                sqB1 = sb2.tile([25, CQ], DT, tag="sqB")
                nc.vector.tensor_tensor(sqB1[:], tqBf25, tqBf25, op=MUL)
                colsum1 = ps1.tile([1, CQ], F32, tag="psQA")
                nc.tensor.matmul(colsum1[:], onesF[:],
                                 tqA[:].rearrange("p c q -> p (c q)"),
                                 start=True, stop=False)
                nc.tensor.matmul(colsum1[:], onesF[0:25],
                                 tqB[0:25].rearrange("p c q -> p (c q)"),
                                 start=False, stop=True)
                tqB32w = tqB[32:33, :, :].rearrange("p c q -> p (c q)")
                nc.scalar.copy(tqB32w[:], colsum1[:])
                yn2r1 = ps1.tile([1, CQ], F32, tag="psQB")
                nc.tensor.matmul(yn2r1[:], onesF[:], sqA1[:], start=True, stop=False)
                nc.tensor.matmul(yn2r1[:], onesF[0:25], sqB1[:], start=False, stop=True)
                tqB32f = tqB[32:33, :, :].bitcast(F32).rearrange("p c q -> p (c q)")
                csqv1 = sb2.tile([1, CQ], F32, tag="csqv")
                nc.vector.tensor_tensor(csqv1[:], tqB32f, tqB32f, op=MUL)
                yn21 = sb2.tile([1, CQ], F32, tag="yn2")
                nc.vector.scalar_tensor_tensor(yn21[:], csqv1[:], -1.0 / D, yn2r1[:],
                                               op0=MUL, op1=ADD)
                lyn1 = sb2.tile([1, CQ], F32, tag="lyn")
                nc.scalar.activation(lyn1[:], yn21[:], ACT.Ln)
                inv_yn1 = sb2.tile([1, CQ], DT, tag="invyn")
                nc.scalar.activation(inv_yn1[:], lyn1[:], ACT.Exp, scale=-0.5)

                # ------------- m stats part 1: sum_d hat_m^2 ----------------
                xn2r = sb.tile([128, C], F32, tag="xn2r")
                sqs = sb.tile([128, D], F32, tag="sqs")
                for c in range(C):
                    nc.vector.scalar_tensor_tensor(
                        sqs[:], hm32[:, D * c:D * (c + 1)], 1.0,
                        hm32[:, D * c:D * (c + 1)], op0=MUL, op1=MUL,
                        accum_out=xn2r[:, c:c + 1])

            if dbg:
                nc.sync.dma_start(dbg_d["hatm"][:], hm32)
                nc.sync.dma_start(dbg_d["tqA0"][:], tqAf)
                nc.sync.dma_start(dbg_d["tqB0"][:], tqB[:].bitcast(F32).rearrange("p c q -> p (c q)"))
                nc.sync.dma_start(dbg_d["mTc1d"][:], mTc1[:].bitcast(F32).rearrange("p c n -> p (c n)"))
                nc.sync.dma_start(dbg_d["mTc2d"][:], mTc2[:].bitcast(F32).rearrange("p c n -> p (c n)"))

            inv_xn = sb.tile([128, C], F32, tag="invxn")
            ixb = inv_xn[:].rearrange("p (c a) -> p c a", a=1).broadcast_to((128, C, QL))

            # ---------------- routing --------------------------------------
            with tc.tile_pool(name="ps2", bufs=1, space="PSUM") as ps2:

                def pearson_late(inv_yn, lhs1, extra=None):
                    iyb = ps2.tile([128, CQ], F32, tag="bcast")
                    nc.tensor.matmul(iyb[:], lhs1, inv_yn[:], start=True, stop=True)
                    if extra is not None:
                        extra()  # setup-only work that must precede the num matmuls
                    # num[i,(c,q)]: per-class A/B pairs back-to-back (groups share
                    # one bank; start=True clears the bank's has_written bits)
                    num = ps2.tile([128, C, QL], F32, tag="num")
                    for c in range(C):
                        nc.tensor.matmul(num[:, c, :], mTc1[:, c, :], tqA[:, c, :],
                                         start=True, stop=False)
                        nc.tensor.matmul(num[:, c, :], mTc2[:, c, :], tqB[:, c, :],
                                         start=False, stop=True)
                    # p = tanh(num * inv_xn * inv_yn); tanh(x) = 1 - 2/(1+exp(2x))
                    pp1 = sb2.tile([128, C, QL], F32, tag="pp1")
                    nc.vector.tensor_tensor(pp1[:], num[:], ixb, op=MUL)
                    pp = sb2.tile([128, CQ], F32, tag="pp")
                    nc.vector.tensor_tensor(pp[:], pp1[:].rearrange("p c q -> p (c q)"),
                                            iyb[:], op=MUL)
                    e2 = sb2.tile([128, CQ], F32, tag="e2")
                    nc.scalar.activation(e2[:], pp[:], ACT.Exp, scale=2.0)
                    den = sb2.tile([128, CQ], F32, tag="den")
                    nc.vector.tensor_scalar(den[:], e2[:], 1.0, None, op0=ADD)
                    rr = sb2.tile([128, CQ], F32, tag="rr")
                    nc.vector.reciprocal(rr[:], den[:])
                    return rr

                def pearson(extra=None):
                    """p = tanh(centered-corr(mT, tq)); returns rr tile [128, CQ]."""
                    # squares (row 32 of tqB excluded; centering via csq)
                    sqA = sb2.tile([128, CQ], DT, tag="sqA")
                    nc.gpsimd.tensor_tensor(sqA[:], tqAf, tqAf, op=MUL)
                    sqB = sb2.tile([25, CQ], DT, tag="sqB")
                    nc.vector.tensor_tensor(sqB[:], tqBf25, tqBf25, op=MUL)
                    # colsum(tq) -> tqB row 32 (feeds the centered num matmul)
                    colsum = ps2.tile([1, CQ], F32, tag="colsum")
                    nc.tensor.matmul(colsum[:], onesF[:],
                                     tqA[:].rearrange("p c q -> p (c q)"),
                                     start=True, stop=False)
                    nc.tensor.matmul(colsum[:], onesF[0:25],
                                     tqB[0:25].rearrange("p c q -> p (c q)"),
                                     start=False, stop=True)
                    nc.scalar.copy(tqB[32:33, :, :].rearrange("p c q -> p (c q)"), colsum[:])
                    # yn2 = sum(tq^2) - colsum^2/D, centered variance of tq
                    yn2r = ps2.tile([1, CQ], F32, tag="cs2")
                    nc.tensor.matmul(yn2r[:], onesF[:], sqA[:], start=True, stop=False)
                    nc.tensor.matmul(yn2r[:], onesF[0:25], sqB[:], start=False, stop=True)
                    csqv = sb2.tile([1, CQ], F32, tag="csqv")
                    nc.vector.tensor_tensor(csqv[:], tqB32f, tqB32f, op=MUL)
                    yn2 = sb2.tile([1, CQ], F32, tag="yn2")
                    nc.vector.scalar_tensor_tensor(yn2[:], csqv[:], -1.0 / D, yn2r[:],
                                                   op0=MUL, op1=ADD)
                    lyn = sb2.tile([1, CQ], F32, tag="lyn")
                    nc.scalar.activation(lyn[:], yn2[:], ACT.Ln)
                    inv_yn = sb2.tile([1, CQ], DT, tag="invyn")
                    nc.scalar.activation(inv_yn[:], lyn[:], ACT.Exp, scale=-0.5)
                    return pearson_late(inv_yn, ones1[:])

                def stats_tail():
                    # m stats part 2: -mean[i,c] from mTc2 row 32 via 5 tiny
                    # transposes; then inv_xn = 1/sqrt(sum hm^2 - D*mean^2).
                    t_nm = ps2.tile([128, C], F32, tag="nm")
                    for c in range(C):
                        nc.tensor.transpose(t_nm[:, c:c + 1],
                                            mTc2[32:33, c, :].bitcast(F32), eye[32:33, 32:33])
                    nmean = sb.tile([128, C], F32, tag="nmean")
                    nc.vector.tensor_copy(nmean[:], t_nm[:])
                    nm2 = sb.tile([128, C], F32, tag="nm2")
                    nc.vector.tensor_tensor(nm2[:], nmean[:], nmean[:], op=MUL)
                    xn2 = sb.tile([128, C], F32, tag="xn2")
                    nc.vector.scalar_tensor_tensor(xn2[:], nm2[:], -float(D), xn2r[:],
                                                   op0=MUL, op1=ADD)
                    lxn = sb.tile([128, C], F32, tag="lxn")
                    nc.scalar.activation(lxn[:], xn2[:], ACT.Ln)
                    nc.scalar.activation(inv_xn[:], lxn[:], ACT.Exp, scale=-0.5)

                def p_from_rr(rr):
                    p_new = sb2.tile([128, CQ], F32, tag="p")
                    nc.vector.tensor_scalar(p_new[:], rr[:], -2.0, 1.0, op0=MUL, op1=ADD)
                    return p_new

                rr = pearson_late(inv_yn1, ones1[:], extra=stats_tail)
                a_t = None
                p_t = None

                for it in range(2):
                    dsp = sb2.tile([128, C, QL], DT, tag="dsp")
                    if it == 0:
                        # softmax(0) = 1/C exactly; dsp = p + 1/C straight from rr
                        nc.vector.tensor_scalar(dsp[:].rearrange("p c q -> p (c q)"),
                                                rr[:], -2.0, 1.0 + 1.0 / C, op0=MUL, op1=ADD)
                        p_t = p_from_rr(rr)
                        if dbg:
                            nc.sync.dma_start(dbg_d["p1"][:], p_t[:])
                            nc.sync.dma_start(dbg_d["invxn"][:], inv_xn[:])
                    else:
                        p_t = p_from_rr(rr)
                        if dbg:
                            nc.sync.dma_start(dbg_d["p2"][:], p_t[:])
                        ea = sb2.tile([128, CQ], F32, tag="ea")
                        nc.scalar.activation(ea[:], a_t[:], ACT.Exp)
                        asum = sb2.tile([128, QL], F32, tag="asum")
                        nc.vector.tensor_reduce(asum[:], ea[:].rearrange("p (c q) -> p q c", c=C),
                                                axis=AX, op=ADD)
                        rs = sb2.tile([128, QL], F32, tag="rs")
                        nc.vector.reciprocal(rs[:], asum[:])
                        dd = sb2.tile([128, C, QL], F32, tag="dd")
                        nc.vector.tensor_tensor(
                            dd[:], ea[:].rearrange("p (c q) -> p c q", c=C),
                            rs[:].rearrange("p (a q) -> p a q", a=1).broadcast_to((128, C, QL)),
                            op=MUL)
                        dd1 = sb2.tile([128, CQ], F32, tag="dd1")
                        nc.vector.tensor_scalar(dd1[:], dd[:].rearrange("p c q -> p (c q)"),
                                                1.0, None, op0=ADD)
                        nc.vector.scalar_tensor_tensor(dsp[:].rearrange("p c q -> p (c q)"),
                                                       rr[:], -2.0, dd1[:], op0=MUL, op1=ADD)

                    # hv[d,(c,q)] in PSUM (consumed in place; never copied to SBUF)
                    hvA = ps2.tile([128, C, QL], F32, tag="hvA")
                    hvB = ps2.tile([26, C, QL], F32, tag="hvB")
                    for c in range(C):
                        nc.tensor.matmul(hvA[:, c, :], hat_m_r[:, D * c:D * c + 128], dsp[:, c, :],
                                         start=True, stop=True)
                        nc.tensor.matmul(hvB[:, c, :], hat_m_r[:, D * c + 128:D * c + 154], dsp[:, c, :],
                                         start=True, stop=True)
                    hvAf = hvA[:].rearrange("p c q -> p (c q)")
                    hvBf25 = hvB[0:25].rearrange("p c q -> p (c q)")
                    # n2 = sum_d hv^2 (raw); squash scale applied later
                    sqhA = sb2.tile([128, CQ], DT, tag="sqhA")
                    nc.scalar.activation(sqhA[:], hvAf, ACT.Square)
                    sqhB = sb2.tile([25, CQ], DT, tag="sqhB")
                    nc.scalar.activation(sqhB[:], hvBf25, ACT.Square)
                    # hv staged to SBUF so sv ops can read the sBh broadcast
                    # straight from PSUM (one-PSUM-input rule); 0.5*tqB is
                    # pre-scaled on Pool while DVE runs the squash chain
                    vA = sb2.tile([128, CQ], F32, tag="vA")
                    nc.scalar.copy(vA[:], hvAf)
                    vB = sb2.tile([25, CQ], F32, tag="vB")
                    nc.vector.tensor_copy(vB[:], hvBf25)
                    tqhB = sb2.tile([25, CQ], F32, tag="tqhB")
                    nc.gpsimd.tensor_scalar(tqhB[:], tqBf25, 0.5, None, op0=MUL)
                    n2 = ps2.tile([1, CQ], F32, tag="cs2")
                    nc.tensor.matmul(n2[:], onesF[:], sqhA[:], start=True, stop=False)
                    nc.tensor.matmul(n2[:], onesF[0:25], sqhB[:], start=False, stop=True)
                    # -s = (1/(1+n2) - 1) / sqrt(n2+eps)
                    n2p1 = sb2.tile([1, CQ], F32, tag="n2p1")
                    nc.vector.tensor_scalar(n2p1[:], n2[:], 1.0, None, op0=ADD)
                    r1 = sb2.tile([1, CQ], F32, tag="r1")
                    nc.vector.reciprocal(r1[:], n2p1[:])
                    ln2 = sb2.tile([1, CQ], F32, tag="ln2")
                    nc.scalar.activation(ln2[:], n2[:], ACT.Ln, bias=epsb[0:1, :])
                    r2 = sb2.tile([1, CQ], F32, tag="r2")
                    nc.scalar.activation(r2[:], ln2[:], ACT.Exp, scale=-0.5)
                    nsrow = sb2.tile([1, CQ], DT, tag="nsrow")
                    nc.vector.scalar_tensor_tensor(nsrow[:], r1[:], 1.0, r2[:],
                                                   op0=SUB, op1=MUL)
                    # 0.5*s broadcast to all partitions: (-0.5) x (-s)
                    sBh = ps2.tile([128, CQ], F32, tag="num")
                    nc.tensor.matmul(sBh[:], nhalf1[:], nsrow[:], start=True, stop=True)
                    # sv = (0.5*s)*hv; tq = 0.5*tq + sv
                    nc.vector.tensor_tensor(svA[:].rearrange("p c q -> p (c q)"),
                                            vA[:], sBh[:], op=MUL)
                    nc.vector.tensor_tensor(svB[0:25].rearrange("p c q -> p (c q)"),
                                            vB[:], sBh[0:25, :], op=MUL)
                    nc.vector.scalar_tensor_tensor(tqA[:].rearrange("p c q -> p (c q)"),
                                                   tqAf, 0.5,
                                                   svA[:].rearrange("p c q -> p (c q)").bitcast(F32),
                                                   op0=MUL, op1=ADD)
                    nc.gpsimd.tensor_tensor(tqB[0:25].rearrange("p c q -> p (c q)"),
                                            tqhB[:],
                                            svB[0:25].rearrange("p c q -> p (c q)").bitcast(F32),
                                            op=ADD)
                    # mdv' = mT . (0.5*s*v)  (rows 25..33 of svB stay 0 -> uncentered)
                    mdv = ps2.tile([128, C, QL], F32, tag="mdv")
                    for c in range(C):
                        nc.tensor.matmul(mdv[:, c, :], mTc1[:, c, :], svA[:, c, :],
                                         start=True, stop=False)
                        nc.tensor.matmul(mdv[:, c, :], mTc2[:, c, :], svB[:, c, :],
                                         start=False, stop=True)
                    # a += p * s * mdv_raw = p * (2*mdv')
                    if it == 0:
                        a_t = sb2.tile([128, CQ], F32, tag="a")
                        nc.vector.scalar_tensor_tensor(a_t[:], mdv[:].rearrange("p c q -> p (c q)"),
                                                       2.0, p_t[:], op0=MUL, op1=MUL)
                    else:
                        pm2 = sb2.tile([128, CQ], F32, tag="pm2")
                        nc.vector.scalar_tensor_tensor(pm2[:], mdv[:].rearrange("p c q -> p (c q)"),
                                                       2.0, p_t[:], op0=MUL, op1=MUL)
                        a_new = sb2.tile([128, CQ], F32, tag="a")
                        nc.vector.tensor_tensor(a_new[:], a_t[:], pm2[:], op=ADD)
                        a_t = a_new
                    if dbg:
                        nc.sync.dma_start(dbg_d["a1" if it == 0 else "a2"][:], a_t[:])

                    rr = pearson()

                # ---------------- final ------------------------------------
                p_t = p_from_rr(rr)
                if dbg:
                    nc.sync.dma_start(dbg_d["p3"][:], p_t[:])
                ea = sb2.tile([128, CQ], F32, tag="ea")
                nc.scalar.activation(ea[:], a_t[:], ACT.Exp)
                asum = sb2.tile([128, QL], F32, tag="asum")
                nc.vector.tensor_reduce(asum[:], ea[:].rearrange("p (c q) -> p q c", c=C),
                                        axis=AX, op=ADD)
                rs = sb2.tile([128, QL], F32, tag="rs")
                nc.vector.reciprocal(rs[:], asum[:])
                dd = sb2.tile([128, C, QL], F32, tag="dd")
                nc.vector.tensor_tensor(
                    dd[:], ea[:].rearrange("p (c q) -> p c q", c=C),
                    rs[:].rearrange("p (a q) -> p a q", a=1).broadcast_to((128, C, QL)), op=MUL)
                dd1 = sb2.tile([128, CQ], F32, tag="dd1")
                nc.vector.tensor_scalar(dd1[:], dd[:].rearrange("p c q -> p (c q)"),
                                        1.0, None, op0=ADD)
                dspF = sb2.tile([128, C, QL], DT, tag="dsp")
                nc.vector.scalar_tensor_tensor(dspF[:].rearrange("p c q -> p (c q)"),
                                               rr[:], -2.0, dd1[:], op0=MUL, op1=ADD)

                # final hv: per-class matmul, N=256 window (cols 765+ zero).
                # Each class gets its own PSUM bank (all free by now) so the
                # matmuls are not serialized by buffer rotation; n2q = Act
                # Square -> DVE accum; outT scales straight from PSUM.
                n2q = sb2.tile([QL, C], F32, tag="n2q")
                fps_l = []
                for c, tg in zip(range(C), ("hvA", "mdv", "num", "bcast", "colsum")):
                    fps = ps2.tile([QL, 256], F32, tag=tg)
                    nc.tensor.matmul(fps[:], dspF[:, c, :], hat_m_r[:, D * c:D * c + 256],
                                     start=True, stop=True)
                    fps_l.append(fps)
                    sqf = sb2.tile([QL, D], F32, tag="sqf")
                    nc.scalar.activation(sqf[:], fps[:, 0:D], ACT.Square)
                    jnk = sb2.tile([QL, D], F32, tag="jnk")
                    nc.vector.tensor_scalar(jnk[:], sqf[:], 1.0, 0.0, op0=MUL, op1=ADD,
                                            accum_out=n2q[:, c:c + 1])
                if dbg:
                    nc.sync.dma_start(dbg_d["n2qd"][:], n2q[:])
                # fs = squash scale [QL, C] (positive)
                fp1 = sb2.tile([QL, C], F32, tag="fp1")
                nc.vector.tensor_scalar(fp1[:], n2q[:], 1.0, None, op0=ADD)
                fr1 = sb2.tile([QL, C], F32, tag="fr1")
                nc.vector.reciprocal(fr1[:], fp1[:])
                fln = sb2.tile([QL, C], F32, tag="fln")
                nc.scalar.activation(fln[:], n2q[:], ACT.Ln, bias=epsb[0:QL, :])
                fr2 = sb2.tile([QL, C], F32, tag="fr2")
                nc.scalar.activation(fr2[:], fln[:], ACT.Exp, scale=-0.5)
                omr = sb2.tile([QL, C], F32, tag="omr")
                nc.vector.tensor_scalar(omr[:], fr1[:], -1.0, 1.0, op0=MUL, op1=ADD)
                fs = sb2.tile([QL, C], F32, tag="fs")
                nc.vector.tensor_tensor(fs[:], omr[:], fr2[:], op=MUL)
                outT = sb.tile([QL, CD], F32, tag="outT")
                for c in range(C):
                    if c % 2 == 0:
                        nc.vector.tensor_scalar(outT[:, D * c:D * (c + 1)],
                                                fps_l[c][:, 0:D],
                                                fs[:, c:c + 1], None, op0=MUL)
                    else:
                        nc.scalar.activation(outT[:, D * c:D * (c + 1)],
                                             fps_l[c][:, 0:D], ACT.Copy,
                                             scale=fs[:, c:c + 1])
                nc.sync.dma_start(out_d[:], outT[:])

    # All activations use only {Ln, Exp, Copy, Square}, which live together in
    # act func set 6 (natural_log_exp_and_others). The default solver alternates
    # sets, inserting table reloads (~1.3us each); one load suffices.
    def _single_act_table_load():
        inst = mybir.InstLoadActFuncSet(
            name=nc.get_next_instruction_name(), ins=[], outs=[],
            act_func_set_id=6,
        )
        inst.engine = mybir.EngineType.Activation
        nc.register_instruction(inst)
        for blk in nc.main_func.blocks:
            for idx, bi in enumerate(blk.instructions):
                if isinstance(bi, mybir.InstActivation):
                    blk.instructions.insert(idx, inst)
                    return
        raise AssertionError("no activation found")

    nc.insert_act_table_loads = _single_act_table_load
    nc.compile()
    return nc


_CACHE = {}
LAST_EXEC_NS = None
LAST_RESULTS = None


def kernel(m, q, W, b):
    m = np.asarray(m, dtype=np.float32)
    q = np.asarray(q, dtype=np.float32)
    W = np.asarray(W, dtype=np.float32)
    b = np.asarray(b, dtype=np.float32)
    assert m.shape == (I, K) and q.shape == (NCORES * QL, K) and W.shape == (K, CD)

    with_bias = bool(np.any(b))
    dbg = bool(int(os.environ.get("KERNEL_DBG", "0")))
    key = ("v2", with_bias, str(DT), dbg)
    if key not in _CACHE:
        _CACHE[key] = build(with_bias, dbg)
    nc = _CACHE[key]

    Wp = np.zeros((K, NPAD), dtype=np.float32)
    Wp[:, :CD] = W
    mT = np.ascontiguousarray(m.T)

    in_maps = []
    for i in range(NCORES):
        qT = np.ascontiguousarray(q[QL * i:QL * (i + 1)].T)
        im = {"mT": mT, "qT": qT, "Wp": Wp}
        if with_bias:
            im["b"] = b.reshape(1, CD)
        in_maps.append(im)

    res = run_bass_kernel_spmd(nc, in_maps, list(range(NCORES)))
    global LAST_EXEC_NS, LAST_RESULTS
    LAST_EXEC_NS = res.exec_time_ns
    LAST_RESULTS = res.results
    out = np.concatenate([res.results[i]["out"] for i in range(NCORES)], axis=0)
    return out.astype(np.float32)


if __name__ == "__main__":
    rng = np.random.default_rng(0)
    m = rng.standard_normal((I, K)).astype(np.float32)
    q = rng.standard_normal((NCORES * QL, K)).astype(np.float32)
    W = (rng.standard_normal((K, CD)) * 0.02).astype(np.float32)
    b = np.zeros((CD,), dtype=np.float32)
    out = kernel(m=m, q=q, W=W, b=b)
    print("out", out.shape, out.dtype, np.abs(out).mean())


# revision 33
# speedup vs baseline: 1.5363x; 1.0011x over previous
"""DMR induction routing kernel for Trainium2 (Bass/Tile), 8-core data-parallel.

Problem: nn_DMRInduction. Full inputs:
  m [128, 768], q [256, 768], W [768, 765], b [765] -> out [256, 765] fp32.

Sharding: Q=256 split 8 ways (32 queries/core); m, W, b replicated.

v2 layout/dataflow (per core):
  - hat_m_r  [I=128, 1024] (I on partitions; cols 0..764 = m @ W, cols 765+ zero
      so the final per-class matmuls can stream N=256 on the fp32r fast path)
  - mTc1 [128, C, 128] / mTc2 [34, C, 128]: per-class transposes of hat_m
      (d on partitions). mTc2 row 32 = -mean_d(hat_m) per (c,i), computed by a
      ones-matmul over mTc + scaled copy, so the pearson numerator matmul is
      centered for free (row 32 of tqB carries colsum(tq)).
  - tq [d, (c,q)]: computed DIRECTLY transposed from qT/W with 60 small
      matmuls into PSUM (no PE eye-transposes for the q side at all).
  - routing state a, p, dsp: [I=128, C*Q=160].
  - squash/pearson scalars on [1, 160] rows; broadcasts via 1-row matmuls.
  - iteration v is consumed straight from PSUM (svA = hv_psum * 0.5s); the
    m-dot-v matmul runs on the scaled v, so a += p*(2*mdv') needs one fused op.
"""
import os
import sys

for _p in ("/opt/trn_rl_repo", "/root/.axon_site/_ro/trn_rl_repo"):
    if os.path.isdir(_p) and _p not in sys.path:
        sys.path.insert(0, _p)

import numpy as np
import concourse.bass as bass
import concourse.bacc as bacc
import concourse.mybir as mybir
import concourse.tile as tile
from concourse.bass_utils import run_bass_kernel_spmd

F32 = mybir.dt.float32
# float32r uses the fast PE path (1 cyc/row at N>=256 vs 4) at ~2.5e-4
# scale-relative output error (tolerance is 2e-2). KERNEL_MM_DT=float32
# restores exact matmuls.
DT = getattr(mybir.dt, os.environ.get("KERNEL_MM_DT", "float32r"))

NCORES = 8
I = 128         # memory capsules
C = 5           # capsule classes
D = 153         # dim per capsule
CD = C * D      # 765
K = 768         # input dim
KC = K // 128   # 6 contraction chunks
QL = 32         # queries per core
CQ = C * QL     # 160
NPAD = 768      # W padded to 768 cols so fp32r matmuls stream N>=256
HM_W = 1024     # hat_m_r padded width (final matmuls read 256-wide windows)
EPS = 1e-8
AX = mybir.AxisListType.X
MUL = mybir.AluOpType.mult
ADD = mybir.AluOpType.add
SUB = mybir.AluOpType.subtract
ACT = mybir.ActivationFunctionType


def build(with_bias: bool, dbg: bool = False):
    nc = bacc.Bacc("TRN2", target_bir_lowering=False, debug=False)

    mT_d = nc.dram_tensor("mT", [K, I], F32, kind="ExternalInput")
    qT_d = nc.dram_tensor("qT", [K, QL], F32, kind="ExternalInput")
    W_d = nc.dram_tensor("Wp", [K, NPAD], F32, kind="ExternalInput")
    if with_bias:
        b_d = nc.dram_tensor("b", [1, CD], F32, kind="ExternalInput")
    out_d = nc.dram_tensor("out", [QL, CD], F32, kind="ExternalOutput")
    dbg_d = {}
    if dbg:
        for nm, shp in [("hatm", [128, CD]), ("tqA0", [128, CQ]), ("tqB0", [34, CQ]),
                        ("mTc1d", [128, C * 128]), ("mTc2d", [34, C * 128]),
                        ("invxn", [128, C]), ("p1", [128, CQ]), ("a1", [128, CQ]),
                        ("p2", [128, CQ]), ("a2", [128, CQ]), ("p3", [128, CQ]),
                        ("n2qd", [QL, C])]:
            dbg_d[nm] = nc.dram_tensor("dbg_" + nm, shp, F32, kind="ExternalOutput")

    with tile.TileContext(nc) as tc:
        with (
            tc.tile_pool(name="sb", bufs=1) as sb,
            tc.tile_pool(name="sb2", bufs=3) as sb2,
        ):
            # ---------------- input DMAs (order = HWDGE serial order) -------
            mT_sb = sb.tile([128, KC, I], DT, tag="mT")
            qT_sb = sb.tile([128, KC, QL], DT, tag="qT")
            W_sb = sb.tile([128, KC, NPAD], DT, tag="W")
            nc.sync.dma_start(mT_sb[:], mT_d[:].rearrange("(k p) n -> p k n", p=128).bitcast(DT))
            nc.sync.dma_start(qT_sb[:], qT_d[:].rearrange("(k p) n -> p k n", p=128).bitcast(DT))
            Wr = W_d[:].rearrange("(k p) n -> p k n", p=128).bitcast(DT)
            for k in range(KC):
                nc.sync.dma_start(W_sb[:, k, 0:512], Wr[:, k, 0:512])
            for k in range(KC):
                nc.sync.dma_start(W_sb[:, k, 512:768], Wr[:, k, 512:768])
            if with_bias:
                b_sb = sb.tile([1, CD], F32, tag="b")
                nc.sync.dma_start(b_sb[:], b_d[:])

            # ---------------- constants (no DMA) ----------------------------
            # float32r tiles cannot be memset directly; memset F32 staging and
            # copy through Act/DVE (engine writes perform the f32r rounding).
            zf = sb.tile([128, 640], F32, tag="zf")
            nc.vector.memset(zf[:], 0.0)
            of = sb.tile([128, 1], F32, tag="of")
            nc.vector.memset(of[:], 1.0)
            o1f = sb.tile([1, 128], F32, tag="o1f")
            nc.vector.memset(o1f[:], 1.0)
            nhf = sb.tile([1, 128], F32, tag="nhf")
            nc.vector.memset(nhf[:], -0.5)
            epsb = sb.tile([128, 1], F32, tag="epsb")
            nc.vector.memset(epsb[:], EPS)
            # identity for PE transposes, built on-chip (a DMA would arrive
            # ~12us in, after the W pieces, and gate the whole transpose block)
            eye = sb.tile([128, 128], F32, tag="eye")
            nc.vector.memset(eye[:], 1.0)
            nc.gpsimd.affine_select(eye[:], eye[:], pattern=[[-1, 128]],
                                    compare_op=mybir.AluOpType.is_equal,
                                    fill=0.0, base=0, channel_multiplier=1)

            ones1 = sb.tile([1, 128], DT, tag="ones1")
            nc.scalar.copy(ones1[:], o1f[:])
            twos1 = sb.tile([1, 128], DT, tag="twos1")
            nc.scalar.activation(twos1[:], o1f[:], ACT.Copy, scale=2.0)
            nhalf1 = sb.tile([1, 128], DT, tag="nhalf1")
            nc.vector.tensor_copy(nhalf1[:], nhf[:])
            onesF = sb.tile([128, 1], DT, tag="onesF")
            nc.scalar.copy(onesF[:], of[:])
            if with_bias:
                onesq = sb.tile([1, QL], DT, tag="onesq")
                nc.vector.tensor_copy(onesq[:], o1f[:, 0:QL])

            # persistent tiles that need zero rows
            hat_m_r = sb.tile([128, HM_W], DT, tag="hatmr")
            nc.vector.tensor_copy(hat_m_r[:, CD:HM_W], zf[:, 0:HM_W - CD])
            mTc1 = sb.tile([128, C, 128], DT, tag="mTc1")
            mTc2 = sb.tile([34, C, 128], DT, tag="mTc2")
            nc.scalar.copy(mTc2[:].rearrange("p c n -> p (c n)"), zf[0:34, 0:640])
            tqA = sb.tile([128, C, QL], DT, tag="tqA")
            tqB = sb.tile([34, C, QL], DT, tag="tqB")
            nc.vector.tensor_copy(tqB[:].rearrange("p c q -> p (c q)"), zf[0:34, 0:CQ])
            svA = sb.tile([128, C, QL], DT, tag="svA")
            svB = sb.tile([34, C, QL], DT, tag="svB")
            nc.scalar.copy(svB[:].rearrange("p c q -> p (c q)"), zf[0:34, 0:CQ])

            tqAf = tqA[:].bitcast(F32).rearrange("p c q -> p (c q)")
            tqBf25 = tqB[0:25].bitcast(F32).rearrange("p c q -> p (c q)")

            # ---------------- projections ----------------------------------
            with tc.tile_pool(name="ps1", bufs=1, space="PSUM") as ps1, \
                 tc.tile_pool(name="pstp", bufs=4, space="PSUM") as pstp:
                psA = ps1.tile([128, 512], F32, tag="psA")
                psB = ps1.tile([128, 256], F32, tag="psB")
                psQA = ps1.tile([128, C, QL], F32, tag="psQA")
                psQB = ps1.tile([34, C, QL], F32, tag="psQB")

                # hat_q (DIRECTLY transposed: out[d,(c,q)] = sum_k W[k,cD+d] q[q,k])
                # shares one bank across classes; start=True clears the whole
                # bank's has_written bits, so each (c, piece) group runs its
                # start..stop back-to-back. k is split in halves combined via
                # SBUF; classes are split c0-2 / c3-4 because c0-2 only read the
                # A columns (<512) of W, which stream in first.
                def q_cs(h, cs):
                    ks = range(3 * h, 3 * h + 3)
                    add_bias = with_bias and h == 1
                    for c in cs:
                        for j, k in enumerate(ks):
                            nc.tensor.matmul(psQA[:, c, :], W_sb[:, k, D * c:D * c + 128],
                                             qT_sb[:, k, :], start=(j == 0),
                                             stop=(j == 2 and not add_bias))
                        if add_bias:
                            nc.tensor.matmul(psQA[:, c, :], b_sb[:, D * c:D * c + 128],
                                             onesq[:], start=False, stop=True)
                        for j, k in enumerate(ks):
                            nc.tensor.matmul(psQB[0:25, c, :], W_sb[:, k, D * c + 128:D * (c + 1)],
                                             qT_sb[:, k, :], start=(j == 0),
                                             stop=(j == 2 and not add_bias))
                        if add_bias:
                            nc.tensor.matmul(psQB[0:25, c, :], b_sb[:, D * c + 128:D * (c + 1)],
                                             onesq[:], start=False, stop=True)

                def q_copy(cs):
                    c0, c1 = cs[0], cs[-1] + 1
                    nc.vector.tensor_copy(tqA[:, c0:c1, :].rearrange("p c q -> p (c q)"),
                                          psQA[:, c0:c1, :].rearrange("p c q -> p (c q)"))
                    nc.vector.tensor_copy(tqB[0:25, c0:c1, :].rearrange("p c q -> p (c q)"),
                                          psQB[0:25, c0:c1, :].rearrange("p c q -> p (c q)"))

                def q_add(cs):
                    c0, c1 = cs[0], cs[-1] + 1
                    nc.vector.tensor_tensor(
                        tqA[:, c0:c1, :].rearrange("p c q -> p (c q)"),
                        tqA[:, c0:c1, :].bitcast(F32).rearrange("p c q -> p (c q)"),
                        psQA[:, c0:c1, :].rearrange("p c q -> p (c q)"), op=ADD)
                    nc.vector.tensor_tensor(
                        tqB[0:25, c0:c1, :].rearrange("p c q -> p (c q)"),
                        tqB[0:25, c0:c1, :].bitcast(F32).rearrange("p c q -> p (c q)"),
                        psQB[0:25, c0:c1, :].rearrange("p c q -> p (c q)"), op=ADD)

                def tposes(cs):
                    for c in cs:
                        t1 = pstp.tile([128, 128], F32, tag="tp")
                        nc.tensor.transpose(t1[:], hat_m_r[:, D * c:D * c + 128].bitcast(F32), eye[:])
                        (nc.vector.tensor_copy if c % 2 else nc.scalar.copy)(mTc1[:, c, :], t1[:])
                        t2 = pstp.tile([25, 128], F32, tag="tp")
                        nc.tensor.transpose(t2[:], hat_m_r[:, D * c + 128:D * (c + 1)].bitcast(F32),
                                            eye[:])
                        (nc.scalar.copy if c % 2 else nc.vector.tensor_copy)(mTc2[0:25, c, :], t2[:])

                # ---- A-column half: hat_m cols 0:512, q classes 0-2 ----------
                for k in range(KC):
                    nc.tensor.matmul(psA[:], mT_sb[:, k, :], W_sb[:, k, 0:512],
                                     start=(k == 0), stop=(k == KC - 1 and not with_bias))
                if with_bias:
                    nc.tensor.matmul(psA[:], ones1[:], b_sb[:, 0:512], start=False, stop=True)
                q_cs(0, [0, 1, 2])
                q_copy([0, 1, 2])
                q_cs(1, [0, 1, 2])
                q_add([0, 1, 2])
                nc.scalar.copy(hat_m_r[:, 0:256], psA[:, 0:256])
                nc.vector.tensor_copy(hat_m_r[:, 256:512], psA[:, 256:512])
                tposes([0, 1, 2])

                # ---- B-column half: hat_m cols 512:765, q classes 3-4 --------
                q_cs(0, [3, 4])
                q_copy([3, 4])
                for k in range(KC):
                    nc.tensor.matmul(psB[:], mT_sb[:, k, :], W_sb[:, k, 512:768],
                                     start=(k == 0), stop=(k == KC - 1 and not with_bias))
                if with_bias:
                    nc.tensor.matmul(psB[:, 0:253], ones1[:], b_sb[:, 512:765],
                                     start=False, stop=True)
                nc.scalar.copy(hat_m_r[:, 512:640], psB[:, 0:128])
                nc.vector.tensor_copy(hat_m_r[:, 640:765], psB[:, 128:253])
                q_cs(1, [3, 4])
                q_add([3, 4])

                hm32 = hat_m_r[:, 0:765].bitcast(F32)

                tposes([3, 4])

                # mTc2 row 32 = -mean_d(hat_m)[c,i] via ones-matmul over mTc
                # (split 512+128: PSUM banks hold 512 fp32/partition max;
                #  reuses the psA/psB banks, free after the hat_m copies)
                psMuA = ps1.tile([1, 512], F32, tag="psA")
                psMuB = ps1.tile([1, 128], F32, tag="psB")
                mTc1f = mTc1[:].rearrange("p c n -> p (c n)")
                mTc2f = mTc2[0:25].rearrange("p c n -> p (c n)")
                nc.tensor.matmul(psMuA[:], onesF[:], mTc1f[:, 0:512],
                                 start=True, stop=False)
                nc.tensor.matmul(psMuA[:], onesF[0:25], mTc2f[:, 0:512],
                                 start=False, stop=True)
                nc.tensor.matmul(psMuB[:], onesF[:], mTc1f[:, 512:640],
                                 start=True, stop=False)
                nc.tensor.matmul(psMuB[:], onesF[0:25], mTc2f[:, 512:640],
                                 start=False, stop=True)
                mTc2r32 = mTc2[32:33, :, :].rearrange("p c n -> p (c n)")
                nc.scalar.activation(mTc2r32[:, 0:256], psMuA[:, 0:256], ACT.Copy, scale=-1.0 / D)
                nc.vector.tensor_scalar(mTc2r32[:, 256:512], psMuA[:, 256:512], -1.0 / D,
                                        None, op0=MUL)
                nc.scalar.activation(mTc2r32[:, 512:640], psMuB[:], ACT.Copy, scale=-1.0 / D)

                # ------------- pearson #1, tq-only part (setup banks) --------
                # (colsum/yn2r run in PE gaps while psMu waits on the transpose
                # copies; the wait queue parks at most 4 blocked instructions)
                sqA1 = sb2.tile([128, CQ], DT, tag="sqA")
                nc.gpsimd.tensor_tensor(sqA1[:], tqAf, tqAf, op=MUL)
                sqB1 = sb2.tile([25, CQ], DT, tag="sqB")
                nc.vector.tensor_tensor(sqB1[:], tqBf25, tqBf25, op=MUL)
                colsum1 = ps1.tile([1, CQ], F32, tag="psQA")
                nc.tensor.matmul(colsum1[:], onesF[:],
                                 tqA[:].rearrange("p c q -> p (c q)"),
                                 start=True, stop=False)
                nc.tensor.matmul(colsum1[:], onesF[0:25],
                                 tqB[0:25].rearrange("p c q -> p (c q)"),
                                 start=False, stop=True)
                tqB32w = tqB[32:33, :, :].rearrange("p c q -> p (c q)")
                nc.scalar.copy(tqB32w[:], colsum1[:])
                yn2r1 = ps1.tile([1, CQ], F32, tag="psQB")
                nc.tensor.matmul(yn2r1[:], onesF[:], sqA1[:], start=True, stop=False)
                nc.tensor.matmul(yn2r1[:], onesF[0:25], sqB1[:], start=False, stop=True)
                tqB32f = tqB[32:33, :, :].bitcast(F32).rearrange("p c q -> p (c q)")
                csqv1 = sb2.tile([1, CQ], F32, tag="csqv")
                nc.vector.tensor_tensor(csqv1[:], tqB32f, tqB32f, op=MUL)
                yn21 = sb2.tile([1, CQ], F32, tag="yn2")
                nc.vector.scalar_tensor_tensor(yn21[:], csqv1[:], -1.0 / D, yn2r1[:],
                                               op0=MUL, op1=ADD)
                lyn1 = sb2.tile([1, CQ], F32, tag="lyn")
                nc.scalar.activation(lyn1[:], yn21[:], ACT.Ln)
                inv_yn1 = sb2.tile([1, CQ], DT, tag="invyn")
                nc.scalar.activation(inv_yn1[:], lyn1[:], ACT.Exp, scale=-0.5)

                # ------------- m stats part 1: sum_d hat_m^2 ----------------
                xn2r = sb.tile([128, C], F32, tag="xn2r")
                sqs = sb.tile([128, D], F32, tag="sqs")
                for c in range(C):
                    nc.vector.scalar_tensor_tensor(
                        sqs[:], hm32[:, D * c:D * (c + 1)], 1.0,
                        hm32[:, D * c:D * (c + 1)], op0=MUL, op1=MUL,
                        accum_out=xn2r[:, c:c + 1])

            if dbg:
                nc.sync.dma_start(dbg_d["hatm"][:], hm32)
                nc.sync.dma_start(dbg_d["tqA0"][:], tqAf)
                nc.sync.dma_start(dbg_d["tqB0"][:], tqB[:].bitcast(F32).rearrange("p c q -> p (c q)"))
                nc.sync.dma_start(dbg_d["mTc1d"][:], mTc1[:].bitcast(F32).rearrange("p c n -> p (c n)"))
                nc.sync.dma_start(dbg_d["mTc2d"][:], mTc2[:].bitcast(F32).rearrange("p c n -> p (c n)"))

            inv_xn = sb.tile([128, C], F32, tag="invxn")
            ixb = inv_xn[:].rearrange("p (c a) -> p c a", a=1).broadcast_to((128, C, QL))

            # ---------------- routing --------------------------------------
            with tc.tile_pool(name="ps2", bufs=1, space="PSUM") as ps2:

                def pearson_late(inv_yn, lhs1, extra=None):
                    iyb = ps2.tile([128, CQ], F32, tag="bcast")
                    nc.tensor.matmul(iyb[:], lhs1, inv_yn[:], start=True, stop=True)
                    if extra is not None:
                        extra()  # setup-only work that must precede the num matmuls
                    # num[i,(c,q)]: per-class A/B pairs back-to-back (groups share
                    # one bank; start=True clears the bank's has_written bits)
                    num = ps2.tile([128, C, QL], F32, tag="num")
                    for c in range(C):
                        nc.tensor.matmul(num[:, c, :], mTc1[:, c, :], tqA[:, c, :],
                                         start=True, stop=False)
                        nc.tensor.matmul(num[:, c, :], mTc2[:, c, :], tqB[:, c, :],
                                         start=False, stop=True)
                    # p = tanh(num * inv_xn * inv_yn); tanh(x) = 1 - 2/(1+exp(2x))
                    pp1 = sb2.tile([128, C, QL], F32, tag="pp1")
                    nc.vector.tensor_tensor(pp1[:], num[:], ixb, op=MUL)
                    pp = sb2.tile([128, CQ], F32, tag="pp")
                    nc.vector.tensor_tensor(pp[:], pp1[:].rearrange("p c q -> p (c q)"),
                                            iyb[:], op=MUL)
                    e2 = sb2.tile([128, CQ], F32, tag="e2")
                    nc.scalar.activation(e2[:], pp[:], ACT.Exp, scale=2.0)
                    den = sb2.tile([128, CQ], F32, tag="den")
                    nc.vector.tensor_scalar(den[:], e2[:], 1.0, None, op0=ADD)
                    rr = sb2.tile([128, CQ], F32, tag="rr")
                    nc.vector.reciprocal(rr[:], den[:])
                    return rr

                def pearson(extra=None):
                    """p = tanh(centered-corr(mT, tq)); returns rr tile [128, CQ]."""
                    # squares (row 32 of tqB excluded; centering via csq)
                    sqA = sb2.tile([128, CQ], DT, tag="sqA")
                    nc.gpsimd.tensor_tensor(sqA[:], tqAf, tqAf, op=MUL)
                    sqB = sb2.tile([25, CQ], DT, tag="sqB")
                    nc.vector.tensor_tensor(sqB[:], tqBf25, tqBf25, op=MUL)
                    # colsum(tq) -> tqB row 32 (feeds the centered num matmul)
                    colsum = ps2.tile([1, CQ], F32, tag="colsum")
                    nc.tensor.matmul(colsum[:], onesF[:],
                                     tqA[:].rearrange("p c q -> p (c q)"),
                                     start=True, stop=False)
                    nc.tensor.matmul(colsum[:], onesF[0:25],
                                     tqB[0:25].rearrange("p c q -> p (c q)"),
                                     start=False, stop=True)
                    nc.scalar.copy(tqB[32:33, :, :].rearrange("p c q -> p (c q)"), colsum[:])
                    # yn2 = sum(tq^2) - colsum^2/D, centered variance of tq
                    yn2r = ps2.tile([1, CQ], F32, tag="cs2")
                    nc.tensor.matmul(yn2r[:], onesF[:], sqA[:], start=True, stop=False)
                    nc.tensor.matmul(yn2r[:], onesF[0:25], sqB[:], start=False, stop=True)
                    csqv = sb2.tile([1, CQ], F32, tag="csqv")
                    nc.vector.tensor_tensor(csqv[:], tqB32f, tqB32f, op=MUL)
                    yn2 = sb2.tile([1, CQ], F32, tag="yn2")
                    nc.vector.scalar_tensor_tensor(yn2[:], csqv[:], -1.0 / D, yn2r[:],
                                                   op0=MUL, op1=ADD)
                    lyn = sb2.tile([1, CQ], F32, tag="lyn")
                    nc.scalar.activation(lyn[:], yn2[:], ACT.Ln)
                    inv_yn = sb2.tile([1, CQ], DT, tag="invyn")
                    nc.scalar.activation(inv_yn[:], lyn[:], ACT.Exp, scale=-0.5)
                    return pearson_late(inv_yn, ones1[:])

                def stats_tail():
                    # m stats part 2: -mean[i,c] from mTc2 row 32 via 5 tiny
                    # transposes; then inv_xn = 1/sqrt(sum hm^2 - D*mean^2).
                    t_nm = ps2.tile([128, C], F32, tag="nm")
                    for c in range(C):
                        nc.tensor.transpose(t_nm[:, c:c + 1],
                                            mTc2[32:33, c, :].bitcast(F32), eye[32:33, 32:33])
                    nmean = sb.tile([128, C], F32, tag="nmean")
                    nc.vector.tensor_copy(nmean[:], t_nm[:])
                    nm2 = sb.tile([128, C], F32, tag="nm2")
                    nc.vector.tensor_tensor(nm2[:], nmean[:], nmean[:], op=MUL)
                    xn2 = sb.tile([128, C], F32, tag="xn2")
                    nc.vector.scalar_tensor_tensor(xn2[:], nm2[:], -float(D), xn2r[:],
                                                   op0=MUL, op1=ADD)
                    lxn = sb.tile([128, C], F32, tag="lxn")
                    nc.scalar.activation(lxn[:], xn2[:], ACT.Ln)
                    nc.scalar.activation(inv_xn[:], lxn[:], ACT.Exp, scale=-0.5)

                def p_from_rr(rr):
                    p_new = sb2.tile([128, CQ], F32, tag="p")
                    nc.vector.tensor_scalar(p_new[:], rr[:], -2.0, 1.0, op0=MUL, op1=ADD)
                    return p_new

                rr = pearson_late(inv_yn1, ones1[:], extra=stats_tail)
                a_t = None
                p_t = None

                for it in range(2):
                    dsp = sb2.tile([128, C, QL], DT, tag="dsp")
                    if it == 0:
                        # softmax(0) = 1/C exactly; dsp = p + 1/C straight from rr
                        nc.vector.tensor_scalar(dsp[:].rearrange("p c q -> p (c q)"),
                                                rr[:], -2.0, 1.0 + 1.0 / C, op0=MUL, op1=ADD)
                        p_t = p_from_rr(rr)
                        if dbg:
                            nc.sync.dma_start(dbg_d["p1"][:], p_t[:])
                            nc.sync.dma_start(dbg_d["invxn"][:], inv_xn[:])
                    else:
                        p_t = p_from_rr(rr)
                        if dbg:
                            nc.sync.dma_start(dbg_d["p2"][:], p_t[:])
                        ea = sb2.tile([128, CQ], F32, tag="ea")
                        nc.scalar.activation(ea[:], a_t[:], ACT.Exp)
                        asum = sb2.tile([128, QL], F32, tag="asum")
                        nc.vector.tensor_reduce(asum[:], ea[:].rearrange("p (c q) -> p q c", c=C),
                                                axis=AX, op=ADD)
                        rs = sb2.tile([128, QL], F32, tag="rs")
                        nc.vector.reciprocal(rs[:], asum[:])
                        dd = sb2.tile([128, C, QL], F32, tag="dd")
                        nc.vector.tensor_tensor(
                            dd[:], ea[:].rearrange("p (c q) -> p c q", c=C),
                            rs[:].rearrange("p (a q) -> p a q", a=1).broadcast_to((128, C, QL)),
                            op=MUL)
                        dd1 = sb2.tile([128, CQ], F32, tag="dd1")
                        nc.vector.tensor_scalar(dd1[:], dd[:].rearrange("p c q -> p (c q)"),
                                                1.0, None, op0=ADD)
                        nc.vector.scalar_tensor_tensor(dsp[:].rearrange("p c q -> p (c q)"),
                                                       rr[:], -2.0, dd1[:], op0=MUL, op1=ADD)

                    # hv[d,(c,q)] in PSUM (consumed in place; never copied to SBUF)
                    hvA = ps2.tile([128, C, QL], F32, tag="hvA")
                    hvB = ps2.tile([26, C, QL], F32, tag="hvB")
                    for c in range(C):
                        nc.tensor.matmul(hvA[:, c, :], hat_m_r[:, D * c:D * c + 128], dsp[:, c, :],
                                         start=True, stop=True)
                        nc.tensor.matmul(hvB[:, c, :], hat_m_r[:, D * c + 128:D * c + 154], dsp[:, c, :],
                                         start=True, stop=True)
                    hvAf = hvA[:].rearrange("p c q -> p (c q)")
                    hvBf25 = hvB[0:25].rearrange("p c q -> p (c q)")
                    # n2 = sum_d hv^2 (raw); squash scale applied later
                    # hv staged to SBUF so sv ops can read the sBh broadcast
                    # straight from PSUM (one-PSUM-input rule); 0.5*tqB is
                    # pre-scaled on Pool while DVE runs the squash chain
                    sqhA = sb2.tile([128, CQ], DT, tag="sqhA")
                    nc.scalar.activation(sqhA[:], hvAf, ACT.Square)
                    vB = sb2.tile([25, CQ], F32, tag="vB")
                    nc.vector.tensor_copy(vB[:], hvBf25)
                    sqhB = sb2.tile([25, CQ], DT, tag="sqhB")
                    nc.vector.tensor_tensor(sqhB[:], vB[:], vB[:], op=MUL)
                    vA = sb2.tile([128, CQ], F32, tag="vA")
                    nc.scalar.copy(vA[:], hvAf)
                    tqhB = sb2.tile([25, CQ], F32, tag="tqhB")
                    nc.gpsimd.tensor_scalar(tqhB[:], tqBf25, 0.5, None, op0=MUL)
                    n2 = ps2.tile([1, CQ], F32, tag="cs2")
                    nc.tensor.matmul(n2[:], onesF[:], sqhA[:], start=True, stop=False)
                    nc.tensor.matmul(n2[:], onesF[0:25], sqhB[:], start=False, stop=True)
                    # -s = (1/(1+n2) - 1) / sqrt(n2+eps)
                    n2p1 = sb2.tile([1, CQ], F32, tag="n2p1")
                    nc.vector.tensor_scalar(n2p1[:], n2[:], 1.0, None, op0=ADD)
                    r1 = sb2.tile([1, CQ], F32, tag="r1")
                    nc.vector.reciprocal(r1[:], n2p1[:])
                    ln2 = sb2.tile([1, CQ], F32, tag="ln2")
                    nc.scalar.activation(ln2[:], n2[:], ACT.Ln, bias=epsb[0:1, :])
                    r2 = sb2.tile([1, CQ], F32, tag="r2")
                    nc.scalar.activation(r2[:], ln2[:], ACT.Exp, scale=-0.5)
                    nsrow = sb2.tile([1, CQ], DT, tag="nsrow")
                    nc.vector.scalar_tensor_tensor(nsrow[:], r1[:], 1.0, r2[:],
                                                   op0=SUB, op1=MUL)
                    # 0.5*s broadcast to all partitions: (-0.5) x (-s)
                    sBh = ps2.tile([128, CQ], F32, tag="num")
                    nc.tensor.matmul(sBh[:], nhalf1[:], nsrow[:], start=True, stop=True)
                    # sv = (0.5*s)*hv; tq = 0.5*tq + sv
                    nc.vector.tensor_tensor(svA[:].rearrange("p c q -> p (c q)"),
                                            vA[:], sBh[:], op=MUL)
                    nc.vector.tensor_tensor(svB[0:25].rearrange("p c q -> p (c q)"),
                                            vB[:], sBh[0:25, :], op=MUL)
                    nc.vector.scalar_tensor_tensor(tqA[:].rearrange("p c q -> p (c q)"),
                                                   tqAf, 0.5,
                                                   svA[:].rearrange("p c q -> p (c q)").bitcast(F32),
                                                   op0=MUL, op1=ADD)
                    nc.gpsimd.tensor_tensor(tqB[0:25].rearrange("p c q -> p (c q)"),
                                            tqhB[:],
                                            svB[0:25].rearrange("p c q -> p (c q)").bitcast(F32),
                                            op=ADD)
                    # mdv' = mT . (0.5*s*v)  (rows 25..33 of svB stay 0 -> uncentered)
                    mdv = ps2.tile([128, C, QL], F32, tag="mdv")
                    for c in range(C):
                        nc.tensor.matmul(mdv[:, c, :], mTc1[:, c, :], svA[:, c, :],
                                         start=True, stop=False)
                        nc.tensor.matmul(mdv[:, c, :], mTc2[:, c, :], svB[:, c, :],
                                         start=False, stop=True)
                    # a += p * s * mdv_raw = p * (2*mdv')
                    if it == 0:
                        a_t = sb2.tile([128, CQ], F32, tag="a")
                        nc.vector.scalar_tensor_tensor(a_t[:], mdv[:].rearrange("p c q -> p (c q)"),
                                                       2.0, p_t[:], op0=MUL, op1=MUL)
                    else:
                        pm2 = sb2.tile([128, CQ], F32, tag="pm2")
                        nc.vector.scalar_tensor_tensor(pm2[:], mdv[:].rearrange("p c q -> p (c q)"),
                                                       2.0, p_t[:], op0=MUL, op1=MUL)
                        a_new = sb2.tile([128, CQ], F32, tag="a")
                        nc.vector.tensor_tensor(a_new[:], a_t[:], pm2[:], op=ADD)
                        a_t = a_new
                    if dbg:
                        nc.sync.dma_start(dbg_d["a1" if it == 0 else "a2"][:], a_t[:])

                    rr = pearson()

                # ---------------- final ------------------------------------
                p_t = p_from_rr(rr)
                if dbg:
                    nc.sync.dma_start(dbg_d["p3"][:], p_t[:])
                ea = sb2.tile([128, CQ], F32, tag="ea")
                nc.scalar.activation(ea[:], a_t[:], ACT.Exp)
                asum = sb2.tile([128, QL], F32, tag="asum")
                nc.vector.tensor_reduce(asum[:], ea[:].rearrange("p (c q) -> p q c", c=C),
                                        axis=AX, op=ADD)
                rs = sb2.tile([128, QL], F32, tag="rs")
                nc.vector.reciprocal(rs[:], asum[:])
                dd = sb2.tile([128, C, QL], F32, tag="dd")
                nc.vector.tensor_tensor(
                    dd[:], ea[:].rearrange("p (c q) -> p c q", c=C),
                    rs[:].rearrange("p (a q) -> p a q", a=1).broadcast_to((128, C, QL)), op=MUL)
                dd1 = sb2.tile([128, CQ], F32, tag="dd1")
                nc.vector.tensor_scalar(dd1[:], dd[:].rearrange("p c q -> p (c q)"),
                                        1.0, None, op0=ADD)
                dspF = sb2.tile([128, C, QL], DT, tag="dsp")
                nc.vector.scalar_tensor_tensor(dspF[:].rearrange("p c q -> p (c q)"),
                                               rr[:], -2.0, dd1[:], op0=MUL, op1=ADD)

                # final hv: per-class matmul, N=256 window (cols 765+ zero).
                # Each class gets its own PSUM bank (all free by now) so the
                # matmuls are not serialized by buffer rotation; n2q = Act
                # Square -> DVE accum; outT scales straight from PSUM.
                n2q = sb2.tile([QL, C], F32, tag="n2q")
                fps_l = []
                for c, tg in zip(range(C), ("hvA", "mdv", "num", "bcast", "colsum")):
                    fps = ps2.tile([QL, 256], F32, tag=tg)
                    nc.tensor.matmul(fps[:], dspF[:, c, :], hat_m_r[:, D * c:D * c + 256],
                                     start=True, stop=True)
                    fps_l.append(fps)
                    sqf = sb2.tile([QL, D], F32, tag="sqf")
                    nc.scalar.activation(sqf[:], fps[:, 0:D], ACT.Square)
                    jnk = sb2.tile([QL, D], F32, tag="jnk")
                    nc.vector.tensor_scalar(jnk[:], sqf[:], 1.0, 0.0, op0=MUL, op1=ADD,
                                            accum_out=n2q[:, c:c + 1])
                if dbg:
                    nc.sync.dma_start(dbg_d["n2qd"][:], n2q[:])
                # fs = squash scale [QL, C] (positive)
                fp1 = sb2.tile([QL, C], F32, tag="fp1")
                nc.vector.tensor_scalar(fp1[:], n2q[:], 1.0, None, op0=ADD)
                fr1 = sb2.tile([QL, C], F32, tag="fr1")
                nc.vector.reciprocal(fr1[:], fp1[:])
                fln = sb2.tile([QL, C], F32, tag="fln")
                nc.scalar.activation(fln[:], n2q[:], ACT.Ln, bias=epsb[0:QL, :])
                fr2 = sb2.tile([QL, C], F32, tag="fr2")
                nc.scalar.activation(fr2[:], fln[:], ACT.Exp, scale=-0.5)
                omr = sb2.tile([QL, C], F32, tag="omr")
                nc.vector.tensor_scalar(omr[:], fr1[:], -1.0, 1.0, op0=MUL, op1=ADD)
                fs = sb2.tile([QL, C], F32, tag="fs")
                nc.vector.tensor_tensor(fs[:], omr[:], fr2[:], op=MUL)
                outT = sb.tile([QL, CD], F32, tag="outT")
                for c in range(C):
                    if c % 2 == 0:
                        nc.vector.tensor_scalar(outT[:, D * c:D * (c + 1)],
                                                fps_l[c][:, 0:D],
                                                fs[:, c:c + 1], None, op0=MUL)
                    else:
                        nc.scalar.activation(outT[:, D * c:D * (c + 1)],
                                             fps_l[c][:, 0:D], ACT.Copy,
                                             scale=fs[:, c:c + 1])
                nc.sync.dma_start(out_d[:], outT[:])

    # All activations use only {Ln, Exp, Copy, Square}, which live together in
    # act func set 6 (natural_log_exp_and_others). The default solver alternates
    # sets, inserting table reloads (~1.3us each); one load suffices.
    def _single_act_table_load():
        inst = mybir.InstLoadActFuncSet(
            name=nc.get_next_instruction_name(), ins=[], outs=[],
            act_func_set_id=6,
        )
        inst.engine = mybir.EngineType.Activation
        nc.register_instruction(inst)
        for blk in nc.main_func.blocks:
            for idx, bi in enumerate(blk.instructions):
                if isinstance(bi, mybir.InstActivation):
                    blk.instructions.insert(idx, inst)
                    return
        raise AssertionError("no activation found")

    nc.insert_act_table_loads = _single_act_table_load
    nc.compile()
    return nc


_CACHE = {}
LAST_EXEC_NS = None
LAST_RESULTS = None


def kernel(m, q, W, b):
    m = np.asarray(m, dtype=np.float32)
    q = np.asarray(q, dtype=np.float32)
    W = np.asarray(W, dtype=np.float32)
    b = np.asarray(b, dtype=np.float32)
    assert m.shape == (I, K) and q.shape == (NCORES * QL, K) and W.shape == (K, CD)

    with_bias = bool(np.any(b))
    dbg = bool(int(os.environ.get("KERNEL_DBG", "0")))
    key = ("v2", with_bias, str(DT), dbg)
    if key not in _CACHE:
        _CACHE[key] = build(with_bias, dbg)
    nc = _CACHE[key]

    Wp = np.zeros((K, NPAD), dtype=np.float32)
    Wp[:, :CD] = W
    mT = np.ascontiguousarray(m.T)

    in_maps = []
    for i in range(NCORES):
        qT = np.ascontiguousarray(q[QL * i:QL * (i + 1)].T)
        im = {"mT": mT, "qT": qT, "Wp": Wp}
        if with_bias:
            im["b"] = b.reshape(1, CD)
        in_maps.append(im)

    res = run_bass_kernel_spmd(nc, in_maps, list(range(NCORES)))
    global LAST_EXEC_NS, LAST_RESULTS
    LAST_EXEC_NS = res.exec_time_ns
    LAST_RESULTS = res.results
    out = np.concatenate([res.results[i]["out"] for i in range(NCORES)], axis=0)
    return out.astype(np.float32)


if __name__ == "__main__":
    rng = np.random.default_rng(0)
    m = rng.standard_normal((I, K)).astype(np.float32)
    q = rng.standard_normal((NCORES * QL, K)).astype(np.float32)
    W = (rng.standard_normal((K, CD)) * 0.02).astype(np.float32)
    b = np.zeros((CD,), dtype=np.float32)
    out = kernel(m=m, q=q, W=W, b=b)
    print("out", out.shape, out.dtype, np.abs(out).mean())
